# revision 1
# baseline (speedup 1.0000x reference)
"""Performer (FAVOR+) encoder layer on 8 trn2 NeuronCores.

Sharding: data-parallel over sequence (512 positions per core x 4 batches).
The linear-attention summaries (A = E_k^T v per (batch, head), usum) and the
global key-feature max (via one-hot slots) are combined in ONE packed
AllReduce, overlapped with the Q-side feature compute.

Layout: activations feature-major (xT = [D, tokens]) so every GEMM's
stationary operand is a natural weight slice; per-token reductions and
broadcasts are small PE matmuls (selector / ones / f32r broadcast matmuls).
E_k and v are produced token-major directly by matmuls so the token-
contraction A-matmul needs no transposes.
"""
import os
import numpy as np
import ml_dtypes

B, N, D = 4, 4096, 1024
H, DH = 16, 64
DFF = 4096
M = 64
EPS_KERN = 1e-6
EPS_LN = 1e-6
NC = 8
NT = N // NC                # 512 positions per core per batch
PAIRS = H // 2              # 8 head-pairs
KT_D = D // 128             # 8
MT_FF = DFF // 128          # 32
TT = NT // 128              # 4
DN = 1.0 / np.sqrt(np.sqrt(DH))
DN2H = DN * DN / 2.0


def _emit(nc, tc):
    import concourse.mybir as mybir
    from contextlib import ExitStack
    F32 = mybir.dt.float32
    F32R = mybir.dt.float32r
    BF16 = mybir.dt.bfloat16
    AF = mybir.ActivationFunctionType
    ALU = mybir.AluOpType
    AX = mybir.AxisListType

    dram = lambda name, shape, dt, kind: nc.dram_tensor(name, shape, dt, kind=kind).ap()

    x_bf = dram("x_bf", [B, D, NT], BF16, "ExternalInput")
    x_f32 = dram("x_f32", [B, D, NT], F32, "ExternalInput")
    wqs = dram("wqs", [KT_D, 128, KT_D, 128], BF16, "ExternalInput")
    wks = dram("wks", [KT_D, 128, KT_D, 128], BF16, "ExternalInput")
    wv = dram("wv", [D, D], BF16, "ExternalInput")
    wos = dram("wos", [KT_D, 128, KT_D, 128], BF16, "ExternalInput")
    w1s = dram("w1s", [MT_FF, 128, KT_D, 128], BF16, "ExternalInput")
    w2s = dram("w2s", [KT_D, 128, MT_FF, 128], BF16, "ExternalInput")
    projbd = dram("projbd", [128, 128], BF16, "ExternalInput")
    negselF = dram("negselF", [2, 128], BF16, "ExternalInput")
    sel2 = dram("sel2", [128, 2], BF16, "ExternalInput")
    sel2b = dram("sel2b", [2, 128], F32, "ExternalInput")
    ones128 = dram("ones128", [128, 1], BF16, "ExternalInput")
    ones1x128 = dram("ones1x128", [1, 128], F32, "ExternalInput")
    mean1 = dram("mean1", [128, 1], F32, "ExternalInput")
    headmask2 = dram("headmask2", [128, 2], F32, "ExternalInput")
    epsvB = dram("epsvB", [128, B * D], BF16, "ExternalInput")
    onehot = dram("onehot", [1, NC], F32, "ExternalInput")
    b1c = dram("b1c", [128, MT_FF], F32, "ExternalInput")
    b1p1c = dram("b1p1c", [128, MT_FF], F32, "ExternalInput")
    b2adjc = dram("b2adjc", [128, KT_D], F32, "ExternalInput")
    g1c = dram("g1c", [128, KT_D], F32, "ExternalInput")
    be1c = dram("be1c", [128, KT_D], F32, "ExternalInput")
    g2c = dram("g2c", [128, KT_D], F32, "ExternalInput")
    be2c = dram("be2c", [128, KT_D], F32, "ExternalInput")
    out = dram("out", [B, D, NT], F32, "ExternalOutput")

    AC_A = B * PAIRS * 64
    AC_U = B * PAIRS
    AC = AC_A + AC_U + NC

    ctx = ExitStack()
    pconst = ctx.enter_context(tc.tile_pool(name="pconst", bufs=1))
    pwv = ctx.enter_context(tc.tile_pool(name="pwv", bufs=1))
    pstream = ctx.enter_context(tc.tile_pool(name="pstream", bufs=2))
    pw2s = ctx.enter_context(tc.tile_pool(name="pw2s", bufs=2))
    pxa = ctx.enter_context(tc.tile_pool(name="pxa", bufs=1))
    pmt = ctx.enter_context(tc.tile_pool(name="pmt", bufs=4))
    psm = ctx.enter_context(tc.tile_pool(name="psm", bufs=2))
    peq = ctx.enter_context(tc.tile_pool(name="peq", bufs=1))
    pbig = ctx.enter_context(tc.tile_pool(name="pbig", bufs=1))
    pone = ctx.enter_context(tc.tile_pool(name="pone", bufs=1))
    pdram = ctx.enter_context(tc.tile_pool(name="pdram", bufs=1, space="DRAM"))
    PP = ctx.enter_context(tc.tile_pool(name="PP", bufs=4, space="PSUM"))
    PD = ctx.enter_context(tc.tile_pool(name="PD", bufs=2, space="PSUM"))
    PA_ = ctx.enter_context(tc.tile_pool(name="PA", bufs=1, space="PSUM"))
    PR = ctx.enter_context(tc.tile_pool(name="PR", bufs=1, space="PSUM"))

    # ---- constants ----
    wv_sb = pwv.tile([128, KT_D, D], BF16, tag="wv")
    nc.sync.dma_start(wv_sb[:], wv.rearrange("(kt p) m -> p kt m", p=128))
    cAPs = {}
    for name, ap, shape, dt in (
        ("projbd", projbd, [128, 128], BF16), ("negselF", negselF, [2, 128], BF16),
        ("sel2", sel2, [128, 2], BF16), ("sel2b", sel2b, [2, 128], F32),
        ("ones128", ones128, [128, 1], BF16), ("ones1x128", ones1x128, [1, 128], F32),
        ("mean1", mean1, [128, 1], F32), ("headmask2", headmask2, [128, 2], F32),
        ("onehot", onehot, [1, NC], F32), ("b1c", b1c, [128, MT_FF], F32),
        ("b1p1c", b1p1c, [128, MT_FF], F32), ("b2adjc", b2adjc, [128, KT_D], F32),
        ("g1c", g1c, [128, KT_D], F32), ("be1c", be1c, [128, KT_D], F32),
        ("g2c", g2c, [128, KT_D], F32), ("be2c", be2c, [128, KT_D], F32),
    ):
        t = pconst.tile(shape, dt, tag=name)
        nc.sync.dma_start(t[:], ap[:])
        cAPs[name] = t
    sel2b_r = pconst.tile([2, 128], F32R, tag="sel2br")
    ones1x128_r = pconst.tile([1, 128], F32R, tag="ones1x128r")
    mean1_r = pconst.tile([128, 1], F32R, tag="mean1r")
    mean1_bf = pconst.tile([128, 1], BF16, tag="mean1bf")
    sel2b_bf = pconst.tile([2, 128], BF16, tag="sel2bbf")
    nc.vector.tensor_copy(sel2b_r[:], cAPs["sel2b"][:])
    nc.vector.tensor_copy(ones1x128_r[:], cAPs["ones1x128"][:])
    nc.vector.tensor_copy(mean1_r[:], cAPs["mean1"][:])
    nc.vector.tensor_copy(mean1_bf[:], cAPs["mean1"][:])
    nc.vector.tensor_copy(sel2b_bf[:], cAPs["sel2b"][:])

    epsln_c = pconst.tile([1, 1], F32, tag="epslnc")
    nc.vector.memset(epsln_c[:], float(EPS_LN))
    onesrow_c = pconst.tile([1, NT], F32, tag="onesrowc")
    nc.vector.memset(onesrow_c[:], 1.0)
    arstage = pone.tile([128, AC], F32, tag="arbuf")
    mxcols = pone.tile([128, B * PAIRS * TT], BF16, tag="mxcols")

    def ln(res, gc, bc, tag, odt, dma_out=None):
        """Feature-major LN: stats via f32r PE reductions, broadcasts via f32r
        K=1 matmuls. res is a [128, KT_D, NT] f32r tile. Returns out tile or,
        if odt is None, yields per-kt fp32 tiles via generator semantics."""
        pm0 = PR.tile([1, NT], F32, tag="prow")
        pm1 = PA_.tile([1, NT], F32, tag="pA")
        for kt in range(KT_D):
            sqt = psm.tile([128, NT], BF16, tag="lnsq")
            nc.scalar.square(sqt[:], res[:, kt, :].bitcast(F32))
            nc.tensor.matmul(pm0[:], mean1_r[:], res[:, kt, :],
                             start=kt == 0, stop=kt == KT_D - 1,
                             skip_group_check=True)
            nc.tensor.matmul(pm1[:], mean1_bf[:], sqt[:],
                             start=kt == 0, stop=kt == KT_D - 1,
                             skip_group_check=True)
        mu = psm.tile([1, NT], F32R, tag="lnmu")
        nc.vector.tensor_copy(mu[:], pm0[:])
        muf = mu[:].bitcast(F32)
        mu2 = psm.tile([1, NT], F32, tag="lnrow")
        nc.vector.tensor_tensor(mu2[:], muf, muf, op=ALU.mult)
        var = psm.tile([1, NT], F32, tag="lnrow")
        nc.vector.tensor_tensor(var[:], pm1[:], mu2[:], op=ALU.subtract)
        lnv = psm.tile([1, NT], F32, tag="lnrow")
        nc.scalar.activation(lnv[:], var[:], AF.Ln, bias=epsln_c[:])
        rstdf = psm.tile([1, NT], F32, tag="lnrow")
        nc.scalar.activation(rstdf[:], lnv[:], AF.Exp, scale=-0.5)
        rstd = psm.tile([1, NT], F32R, tag="lnrstd")
        nc.vector.tensor_copy(rstd[:], rstdf[:])
        pmu = PP.tile([128, NT], F32, tag="pbig")
        nc.tensor.matmul(pmu[:], ones1x128_r[:], mu[:], start=True, stop=True)
        prs = PP.tile([128, NT], F32, tag="pbig")
        nc.tensor.matmul(prs[:], ones1x128_r[:], rstd[:], start=True, stop=True)
        o = None
        if odt is not None:
            o = pbig.tile([128, KT_D, NT], odt, tag="bf8")
        cen = psm.tile([128, NT], F32, tag="lncen")
        nrm = psm.tile([128, NT], F32, tag="lnnrm")
        for kt in range(KT_D):
            nc.vector.tensor_tensor(cen[:], res[:, kt, :].bitcast(F32), pmu[:],
                                    op=ALU.subtract)
            nc.vector.tensor_tensor(nrm[:], cen[:], prs[:], op=ALU.mult)
            if o is not None:
                nc.vector.tensor_scalar(o[:, kt, :], nrm[:], gc[:, kt:kt + 1],
                                        bc[:, kt:kt + 1], op0=ALU.mult, op1=ALU.add)
            else:
                ot = psm.tile([128, NT], F32, tag="lnot")
                nc.vector.tensor_scalar(ot[:], nrm[:], gc[:, kt:kt + 1],
                                        bc[:, kt:kt + 1], op0=ALU.mult, op1=ALU.add)
                nc.sync.dma_start(dma_out[0][dma_out[1], kt * 128:(kt + 1) * 128, :],
                                  ot[:])
        return o

    # ================= stage A =================
    for b in range(B):
        xbf = pxa.tile([128, KT_D, NT], BF16, tag="xbf")
        nc.sync.dma_start(xbf[:], x_bf[b].rearrange("(kt p) t -> p kt t", p=128))

        vtok = pxa.tile([128, TT, PAIRS, 129], BF16, tag="vtok")
        nc.vector.memset(vtok[:], 1.0)
        for tt in range(TT):
            for nh in range(2):
                pv = PP.tile([128, NT], F32, tag="pbig")
                for kt in range(KT_D):
                    nc.tensor.matmul(pv[:], xbf[:, kt, tt * 128:(tt + 1) * 128],
                                     wv_sb[:, kt, nh * 512:(nh + 1) * 512],
                                     start=kt == 0, stop=kt == KT_D - 1)
                for i in range(4):
                    nc.scalar.activation(vtok[:, tt, nh * 4 + i, 0:128],
                                         pv[:, i * 128:(i + 1) * 128], AF.Copy)

        for pr in range(PAIRS):
            wkmt = pstream.tile([128, KT_D, 128], BF16, tag="wmt")
            nc.sync.dma_start(wkmt[:], wks[pr])
            pk = PP.tile([128, NT], F32, tag="pbig")
            for kt in range(KT_D):
                nc.tensor.matmul(pk[:], wkmt[:, kt, :], xbf[:, kt, :],
                                 start=kt == 0, stop=kt == KT_D - 1)
            kTmt = pmt.tile([128, NT], BF16, tag="mt512")
            nc.scalar.activation(kTmt[:], pk[:], AF.Copy)
            ksqmt = pmt.tile([128, NT], BF16, tag="mt512")
            nc.scalar.square(ksqmt[:], pk[:])
            pks = PR.tile([2, NT], F32, tag="prow")
            nc.tensor.matmul(pks[:], cAPs["sel2"][:], ksqmt[:], start=True, stop=True)
            ksq2 = psm.tile([2, NT], BF16, tag="ksq2")
            nc.scalar.activation(ksq2[:], pks[:], AF.Copy)

            Ek = psm.tile([128, TT, 128], BF16, tag="Ek")
            for tt in range(TT):
                pdd = PD.tile([128, 128], F32, tag="pdd")
                nc.tensor.matmul(pdd[:], kTmt[:, tt * 128:(tt + 1) * 128],
                                 cAPs["projbd"][:], start=True, stop=True)
                c = (b * PAIRS + pr) * TT + tt
                nc.vector.tensor_reduce(mxcols[:, c:c + 1], pdd[:], axis=AX.X,
                                        op=ALU.max)
                nc.tensor.matmul(pdd[:], ksq2[:, tt * 128:(tt + 1) * 128],
                                 cAPs["negselF"][:], start=False, stop=True,
                                 skip_group_check=True)
                nc.scalar.activation(Ek[:, tt, :], pdd[:], AF.Exp)

            pA = PA_.tile([128, 129], F32, tag="pA")
            for tt in range(TT):
                nc.tensor.matmul(pA[:], Ek[:, tt, :], vtok[:, tt, pr, :],
                                 start=tt == 0, stop=tt == TT - 1,
                                 skip_group_check=True)
            j = b * PAIRS + pr
            nc.vector.tensor_copy(arstage[0:64, j * 64:(j + 1) * 64], pA[0:64, 0:64])
            nc.vector.tensor_copy(arstage[64:128, j * 64:(j + 1) * 64],
                                  pA[64:128, 64:128])
            nc.vector.tensor_copy(arstage[:, AC_A + j:AC_A + j + 1], pA[:, 128:129])

    # ---- fire AllReduce ----
    mxr = pone.tile([128, 1], F32, tag="mxr")
    nc.vector.tensor_reduce(mxr[:], mxcols[:], axis=AX.X, op=ALU.max)
    mx1 = pone.tile([1, 1], F32, tag="mx1")
    nc.gpsimd.tensor_reduce(mx1[:], mxr[:], axis=AX.C, op=ALU.max)
    nc.vector.tensor_scalar(arstage[0:1, AC_A + AC_U:AC], cAPs["onehot"][:],
                            mx1[:], None, op0=ALU.mult)
    arin = pdram.tile([128, AC], F32, tag="arin")
    arout = pdram.tile([128, AC], F32, tag="arout")
    nc.sync.dma_start(arin[:], arstage[:])
    if os.environ.get("KERNEL_NOCOLL"):
        nc.sync.dma_start(arout[:], arin[:])
    else:
        nc.gpsimd.collective_compute("AllReduce", ALU.add,
                                     replica_groups=[list(range(NC))],
                                     ins=[arin[:]], outs=[arout[:]])
    arres = pone.tile([128, AC], F32, tag="arbuf")
    nc.sync.dma_start(arres[:], arout[:])

    # ================= q-side features (overlap AR) =================
    Eq_all = {}

    def emit_qside(b):
        qxbf = pxa.tile([128, KT_D, NT], BF16, tag="qxbf")
        nc.sync.dma_start(qxbf[:], x_bf[b].rearrange("(kt p) t -> p kt t", p=128))
        Eqs = []
        for pr in range(PAIRS):
            wqmt = pstream.tile([128, KT_D, 128], BF16, tag="wmt")
            nc.sync.dma_start(wqmt[:], wqs[pr])
            pq_ = PP.tile([128, NT], F32, tag="pbig")
            for kt in range(KT_D):
                nc.tensor.matmul(pq_[:], wqmt[:, kt, :], qxbf[:, kt, :],
                                 start=kt == 0, stop=kt == KT_D - 1)
            qTmt = pmt.tile([128, NT], BF16, tag="mt512")
            nc.scalar.activation(qTmt[:], pq_[:], AF.Copy)
            qsqmt = pmt.tile([128, NT], BF16, tag="mt512")
            nc.scalar.square(qsqmt[:], pq_[:])
            pqs = PR.tile([2, NT], F32, tag="prow")
            nc.tensor.matmul(pqs[:], cAPs["sel2"][:], qsqmt[:], start=True, stop=True)
            qsq2 = psm.tile([2, NT], BF16, tag="qsq2")
            nc.scalar.activation(qsq2[:], pqs[:], AF.Copy)

            pdq = PP.tile([128, NT], F32, tag="pbig")
            nc.tensor.matmul(pdq[:], cAPs["projbd"][:], qTmt[:], start=True,
                             stop=False)
            nc.tensor.matmul(pdq[:], cAPs["negselF"][:], qsq2[:], start=False,
                             stop=True, skip_group_check=True)
            Etmp = pmt.tile([128, NT], BF16, tag="t512bf")
            nc.scalar.activation(Etmp[:], pdq[:], AF.Exp)

            pS = PR.tile([2, NT], F32, tag="prow")
            nc.tensor.matmul(pS[:], cAPs["sel2"][:], Etmp[:], start=True, stop=True)
            ediag = psm.tile([2, NT], BF16, tag="ediag")
            nc.scalar.activation(ediag[:], qsq2[:], AF.Exp, scale=float(DN2H))
            wrow = psm.tile([2, NT], BF16, tag="wrow")
            nc.vector.scalar_tensor_tensor(wrow[:], ediag[:], EPS_KERN, pS[:],
                                           op0=ALU.mult, op1=ALU.mult)
            pwB = PP.tile([128, NT], F32, tag="pbig")
            nc.tensor.matmul(pwB[:], sel2b_bf[:], wrow[:], start=True, stop=True)
            Eq = peq.tile([128, NT], BF16, tag=f"Eq{pr}")
            nc.vector.tensor_tensor(Eq[:], Etmp[:], pwB[:], op=ALU.add)
            Eqs.append(Eq)
        Eq_all[b] = Eqs

    emit_qside(0)

    # ---- kv / ksum assembly ----
    mx8 = pone.tile([1, 1], F32, tag="mx8")
    nc.vector.tensor_reduce(mx8[:], arres[0:1, AC_A + AC_U:AC], axis=AX.X, op=ALU.max)
    emxf = pone.tile([1, 1], F32, tag="emxf")
    nc.scalar.activation(emxf[:], mx8[:], AF.Exp, scale=-1.0)
    emxrow = psm.tile([1, NT], F32R, tag="lnrow")
    nc.vector.tensor_scalar(emxrow[:], onesrow_c[:], emxf[:], None, op0=ALU.mult)
    pex = PP.tile([128, NT], F32, tag="pbig")
    nc.tensor.matmul(pex[:], ones1x128_r[:], emxrow[:], start=True, stop=True)
    emxc = pone.tile([128, 1], F32, tag="emxc")
    nc.vector.tensor_copy(emxc[:], pex[:, 0:1])

    epsv_sb = pbig.tile([128, B * D], BF16, tag="big32")
    nc.sync.dma_start(epsv_sb[:], epsvB[:])

    kvBall = pone.tile([128, B * PAIRS, 130], BF16, tag="kvBall")
    nc.vector.memset(kvBall[:], 0.0)
    kvB_all = {}
    for b in range(B):
        for pr in range(PAIRS):
            j = b * PAIRS + pr
            kvB = kvBall[:, j, :]
            nc.vector.scalar_tensor_tensor(
                kvB[0:64, 0:64], arres[0:64, j * 64:(j + 1) * 64], emxc[0:64, :],
                epsv_sb[0:64, b * D + pr * 128:b * D + pr * 128 + 64],
                op0=ALU.mult, op1=ALU.add)
            nc.vector.scalar_tensor_tensor(
                kvB[64:128, 64:128], arres[64:128, j * 64:(j + 1) * 64],
                emxc[64:128, :],
                epsv_sb[64:128, b * D + pr * 128 + 64:b * D + pr * 128 + 128],
                op0=ALU.mult, op1=ALU.add)
            ksf = psm.tile([128, 1], F32, tag="ksf")
            nc.vector.tensor_scalar(ksf[:], arres[:, AC_A + j:AC_A + j + 1],
                                    emxc[:], float(EPS_KERN * N),
                                    op0=ALU.mult, op1=ALU.add)
            nc.vector.tensor_scalar(kvB[:, 128:130], cAPs["headmask2"][:], ksf[:],
                                    None, op0=ALU.mult)
            kvB_all[j] = kvB

    # ================= stage B =================
    for b in range(B):
        Eqs = Eq_all[b]
        attnT = pbig.tile([128, KT_D, NT], BF16, tag="bf8")
        for pr in range(PAIRS):
            kvB = kvB_all[b * PAIRS + pr]
            pnum = PP.tile([128, NT], F32, tag="pbig")
            nc.tensor.matmul(pnum[:], kvB[:, 0:128], Eqs[pr][:], start=True,
                             stop=True)
            pden = PR.tile([2, NT], F32, tag="prow")
            nc.tensor.matmul(pden[:], kvB[:, 128:130], Eqs[pr][:], start=True,
                             stop=True)
            rdf = psm.tile([2, NT], F32, tag="rdf")
            nc.vector.reciprocal(rdf[:], pden[:])
            rden = psm.tile([2, NT], F32R, tag="rden")
            nc.vector.tensor_copy(rden[:], rdf[:])
            prdB = PP.tile([128, NT], F32, tag="pbig")
            nc.tensor.matmul(prdB[:], sel2b_r[:], rden[:], start=True, stop=True)
            numsb = psm.tile([128, NT], BF16, tag="numsb")
            nc.scalar.activation(numsb[:], pnum[:], AF.Copy)
            nc.vector.tensor_tensor(attnT[:, pr, :], numsb[:], prdB[:], op=ALU.mult)

        res1 = pbig.tile([128, KT_D, NT], F32R, tag="resX")
        for mt in range(KT_D):
            womt = pstream.tile([128, KT_D, 128], BF16, tag="wmt")
            nc.sync.dma_start(womt[:], wos[mt])
            po = PP.tile([128, NT], F32, tag="pbig")
            for kt in range(KT_D):
                nc.tensor.matmul(po[:], womt[:, kt, :], attnT[:, kt, :],
                                 start=kt == 0, stop=kt == KT_D - 1)
            xf = psm.tile([128, NT], F32, tag="xf")
            nc.sync.dma_start(xf[:], x_f32[b, mt * 128:(mt + 1) * 128, :])
            nc.vector.tensor_tensor(res1[:, mt, :], xf[:], po[:], op=ALU.add)

        out1 = ln(res1, cAPs["g1c"], cAPs["be1c"], "o1", mybir.dt.bfloat16)

        hsb = pbig.tile([128, MT_FF, NT], BF16, tag="big32")
        for mt in range(MT_FF):
            w1mt = pstream.tile([128, KT_D, 128], BF16, tag="wmt")
            nc.sync.dma_start(w1mt[:], w1s[mt])
            pz = PP.tile([128, NT], F32, tag="pbig")
            for kt in range(KT_D):
                nc.tensor.matmul(pz[:], w1mt[:, kt, :], out1[:, kt, :],
                                 start=kt == 0, stop=kt == KT_D - 1)
            eraw = pmt.tile([128, NT], BF16, tag="t512bf")
            nc.scalar.activation(eraw[:], pz[:], AF.Exp,
                                 bias=cAPs["b1c"][:, mt:mt + 1])
            emin = pmt.tile([128, NT], BF16, tag="t512bf")
            nc.gpsimd.tensor_scalar_min(emin[:], eraw[:], 1.0)
            nc.vector.scalar_tensor_tensor(hsb[:, mt, :], pz[:],
                                           cAPs["b1p1c"][:, mt:mt + 1], emin[:],
                                           op0=ALU.add, op1=ALU.max)

        res2 = pbig.tile([128, KT_D, NT], F32R, tag="resX")
        for mt in range(KT_D):
            w2mt = pw2s.tile([128, MT_FF, 128], BF16, tag="w2mt")
            nc.sync.dma_start(w2mt[:], w2s[mt])
            pf = PP.tile([128, NT], F32, tag="pbig")
            for kt in range(MT_FF):
                nc.tensor.matmul(pf[:], w2mt[:, kt, :], hsb[:, kt, :],
                                 start=kt == 0, stop=kt == MT_FF - 1)
            nc.vector.scalar_tensor_tensor(res2[:, mt, :], pf[:],
                                           cAPs["b2adjc"][:, mt:mt + 1],
                                           out1[:, mt, :], op0=ALU.add, op1=ALU.add)

        ln(res2, cAPs["g2c"], cAPs["be2c"], "o2", None, dma_out=(out, b))

        if b + 1 < B:
            emit_qside(b + 1)

    ctx.close()


_CACHE = {}


def _build():
    import concourse.tile as tile
    from concourse import bacc
    nc = bacc.Bacc("TRN2", target_bir_lowering=False, debug=False, num_devices=NC)
    with tile.TileContext(nc) as tc:
        _emit(nc, tc)
    nc.compile()
    return nc


def _host_inputs(x, Wq, Wk, Wv, Wo, proj, W1, b1, W2, b2,
                 ln1_g, ln1_b, ln2_g, ln2_b):
    bf = ml_dtypes.bfloat16
    f32 = np.float32
    d = {}

    def chunked(w):  # [D, X] -> [X/128 mt, 128 p, D/128 kt, 128]
        Dk, X = w.shape
        r = w.reshape(Dk // 128, 128, X // 128, 128)
        return np.ascontiguousarray(r.transpose(2, 1, 0, 3)).astype(bf)

    d["wqs"] = chunked(Wq.reshape(D, D))
    d["wks"] = chunked(Wk.reshape(D, D))
    d["wv"] = np.ascontiguousarray(Wv.reshape(D, D)).astype(bf)
    d["wos"] = chunked(Wo.reshape(D, D))
    d["w1s"] = chunked(W1)
    d["w2s"] = chunked(W2)

    projT_s = (proj * DN).T.astype(f32)
    pbd = np.zeros((128, 128), f32)
    pbd[0:64, 0:64] = projT_s
    pbd[64:128, 64:128] = projT_s
    d["projbd"] = pbd.astype(bf)
    nsF = np.zeros((2, 128), f32)
    nsF[0, 0:64] = -DN2H
    nsF[1, 64:128] = -DN2H
    d["negselF"] = nsF.astype(bf)
    s2 = np.zeros((128, 2), f32)
    s2[0:64, 0] = 1.0
    s2[64:128, 1] = 1.0
    d["sel2"] = s2.astype(bf)
    s2b = np.zeros((2, 128), f32)
    s2b[0, 0:64] = 1.0
    s2b[1, 64:128] = 1.0
    d["sel2b"] = s2b
    d["ones128"] = np.ones((128, 1), f32).astype(bf)
    d["ones1x128"] = np.ones((1, 128), f32)
    d["mean1"] = np.full((128, 1), 1.0 / D, f32)
    hm2 = np.zeros((128, 2), f32)
    hm2[0:64, 0] = 1.0
    hm2[64:128, 1] = 1.0
    d["headmask2"] = hm2

    xsum = x.sum(axis=1, dtype=np.float64)
    vsum = xsum @ Wv.reshape(D, D).astype(np.float64)
    epsv = (EPS_KERN * vsum).astype(f32)
    d["epsvB"] = np.ascontiguousarray(
        np.broadcast_to(epsv.reshape(1, B * D), (128, B * D))).astype(bf)

    d["b1c"] = np.ascontiguousarray(b1.reshape(MT_FF, 128).T).astype(f32)
    d["b1p1c"] = np.ascontiguousarray((b1 + 1.0).reshape(MT_FF, 128).T).astype(f32)
    b2adj = b2.astype(np.float64) - W2.astype(np.float64).sum(axis=0)
    d["b2adjc"] = np.ascontiguousarray(b2adj.reshape(KT_D, 128).T).astype(f32)
    d["g1c"] = np.ascontiguousarray(ln1_g.reshape(KT_D, 128).T).astype(f32)
    d["be1c"] = np.ascontiguousarray(ln1_b.reshape(KT_D, 128).T).astype(f32)
    d["g2c"] = np.ascontiguousarray(ln2_g.reshape(KT_D, 128).T).astype(f32)
    d["be2c"] = np.ascontiguousarray(ln2_b.reshape(KT_D, 128).T).astype(f32)
    return d


def kernel(x, Wq, Wk, Wv, Wo, proj, W1, b1, W2, b2, ln1_g, ln1_b, ln2_g, ln2_b):
    from concourse import bass_utils

    x = np.asarray(x, np.float32)
    shared = _host_inputs(x, np.asarray(Wq), np.asarray(Wk), np.asarray(Wv),
                          np.asarray(Wo), np.asarray(proj), np.asarray(W1),
                          np.asarray(b1), np.asarray(W2), np.asarray(b2),
                          np.asarray(ln1_g), np.asarray(ln1_b),
                          np.asarray(ln2_g), np.asarray(ln2_b))

    if "nc" not in _CACHE:
        _CACHE["nc"] = _build()
    nc = _CACHE["nc"]

    in_maps = []
    for c in range(NC):
        xs = x[:, c * NT:(c + 1) * NT, :]
        xT = np.ascontiguousarray(xs.transpose(0, 2, 1))
        oh = np.zeros((1, NC), np.float32)
        oh[0, c] = 1.0
        m = dict(shared)
        m["x_f32"] = xT
        m["x_bf"] = xT.astype(ml_dtypes.bfloat16)
        m["onehot"] = oh
        in_maps.append(m)

    trace = bool(int(os.environ.get("KERNEL_TRACE", "0")))
    res = bass_utils.run_bass_kernel_spmd(nc, in_maps, core_ids=list(range(NC)),
                                          trace=trace)
    if trace and res.exec_time_ns is not None:
        print(f"HW exec time: {res.exec_time_ns} ns")
        if res.instructions_and_trace is not None:
            print("trace:", res.instructions_and_trace[1])

    outp = np.empty((B, N, D), np.float32)
    for c in range(NC):
        oT = res.results[c]["out"]
        outp[:, c * NT:(c + 1) * NT, :] = oT.transpose(0, 2, 1)
    return outp



# revision 7
# speedup vs baseline: 1.5358x; 1.5358x over previous
"""Performer (FAVOR+) encoder layer on 8 trn2 NeuronCores.

Sharding: data-parallel over sequence (512 positions per core x 4 batches).
The linear-attention summaries (A = E_k^T v per (batch, head), usum) and the
global key-feature max (via one-hot slots) are combined in ONE packed
AllReduce, overlapped with the Q-side feature compute.

Layout: activations feature-major (xT = [D, tokens]) so every GEMM's
stationary operand is a natural weight slice; per-token reductions and
broadcasts are small PE matmuls (selector / ones / f32r broadcast matmuls).
E_k and v are produced token-major directly by matmuls so the token-
contraction A-matmul needs no transposes.
"""
import os
import numpy as np
import ml_dtypes

B, N, D = 4, 4096, 1024
H, DH = 16, 64
DFF = 4096
M = 64
EPS_KERN = 1e-6
EPS_LN = 1e-6
NC = 8
NT = N // NC                # 512 positions per core per batch
PAIRS = H // 2              # 8 head-pairs
KT_D = D // 128             # 8
MT_FF = DFF // 128          # 32
TT = NT // 128              # 4
DN = 1.0 / np.sqrt(np.sqrt(DH))
DN2H = DN * DN / 2.0


def _emit(nc, tc):
    import concourse.mybir as mybir
    from contextlib import ExitStack
    F32 = mybir.dt.float32
    F32R = mybir.dt.float32r
    BF16 = mybir.dt.bfloat16
    AF = mybir.ActivationFunctionType
    ALU = mybir.AluOpType
    AX = mybir.AxisListType

    dram = lambda name, shape, dt, kind: nc.dram_tensor(name, shape, dt, kind=kind).ap()

    x_bf = dram("x_bf", [B, D, NT], BF16, "ExternalInput")
    x_f32 = dram("x_f32", [B, D, NT], F32, "ExternalInput")
    wqs = dram("wqs", [KT_D, 128, KT_D, 128], BF16, "ExternalInput")
    wks = dram("wks", [KT_D, 128, KT_D, 128], BF16, "ExternalInput")
    wv = dram("wv", [D, D], BF16, "ExternalInput")
    wos = dram("wos", [KT_D, 128, KT_D, 128], BF16, "ExternalInput")
    w1s = dram("w1s", [MT_FF, 128, KT_D, 128], BF16, "ExternalInput")
    w2s = dram("w2s", [KT_D, 128, MT_FF, 128], BF16, "ExternalInput")
    projbd = dram("projbd", [128, 128], BF16, "ExternalInput")
    negselF = dram("negselF", [2, 128], BF16, "ExternalInput")
    sel2 = dram("sel2", [128, 2], BF16, "ExternalInput")
    sel2b = dram("sel2b", [2, 128], F32, "ExternalInput")
    ones128 = dram("ones128", [128, 1], BF16, "ExternalInput")
    ones1x128 = dram("ones1x128", [1, 128], F32, "ExternalInput")
    mean1 = dram("mean1", [128, 1], F32, "ExternalInput")
    headmask2 = dram("headmask2", [128, 2], F32, "ExternalInput")
    epsvB = dram("epsvB", [128, B * D], BF16, "ExternalInput")
    onehot = dram("onehot", [1, NC], F32, "ExternalInput")
    b1c = dram("b1c", [128, MT_FF], F32, "ExternalInput")
    nb1c = dram("nb1c", [128, MT_FF], F32, "ExternalInput")
    b1p1c = dram("b1p1c", [128, MT_FF], F32, "ExternalInput")
    b2adjc = dram("b2adjc", [128, KT_D], F32, "ExternalInput")
    g1c = dram("g1c", [128, KT_D], F32, "ExternalInput")
    be1c = dram("be1c", [128, KT_D], F32, "ExternalInput")
    g2c = dram("g2c", [128, KT_D], F32, "ExternalInput")
    be2c = dram("be2c", [128, KT_D], F32, "ExternalInput")
    out = dram("out", [B, D, NT], F32, "ExternalOutput")

    AC_A = B * PAIRS * 64
    AC_U = B * PAIRS
    AC = AC_A + AC_U + NC

    ctx = ExitStack()
    pconst = ctx.enter_context(tc.tile_pool(name="pconst", bufs=1))
    pwv = ctx.enter_context(tc.tile_pool(name="pwv", bufs=1))
    pstream = ctx.enter_context(tc.tile_pool(name="pstream", bufs=2))
    pw2s = ctx.enter_context(tc.tile_pool(name="pw2s", bufs=2))
    pxa = ctx.enter_context(tc.tile_pool(name="pxa", bufs=1))
    pmt = ctx.enter_context(tc.tile_pool(name="pmt", bufs=4))
    psm = ctx.enter_context(tc.tile_pool(name="psm", bufs=2))
    peq = ctx.enter_context(tc.tile_pool(name="peq", bufs=1))
    pbig = ctx.enter_context(tc.tile_pool(name="pbig", bufs=1))
    pone = ctx.enter_context(tc.tile_pool(name="pone", bufs=1))
    pdram = ctx.enter_context(tc.tile_pool(name="pdram", bufs=1, space="DRAM"))
    PP = ctx.enter_context(tc.tile_pool(name="PP", bufs=4, space="PSUM"))
    PD = ctx.enter_context(tc.tile_pool(name="PD", bufs=2, space="PSUM"))
    PA_ = ctx.enter_context(tc.tile_pool(name="PA", bufs=1, space="PSUM"))
    PR = ctx.enter_context(tc.tile_pool(name="PR", bufs=1, space="PSUM"))

    # ---- constants ----
    wv_sb = pwv.tile([128, KT_D, D], BF16, tag="wv")
    nc.sync.dma_start(wv_sb[:], wv.rearrange("(kt p) m -> p kt m", p=128))
    cAPs = {}
    for name, ap, shape, dt in (
        ("projbd", projbd, [128, 128], BF16), ("negselF", negselF, [2, 128], BF16),
        ("sel2", sel2, [128, 2], BF16), ("sel2b", sel2b, [2, 128], F32),
        ("ones128", ones128, [128, 1], BF16), ("ones1x128", ones1x128, [1, 128], F32),
        ("mean1", mean1, [128, 1], F32), ("headmask2", headmask2, [128, 2], F32),
        ("onehot", onehot, [1, NC], F32), ("b1c", b1c, [128, MT_FF], F32),
        ("nb1c", nb1c, [128, MT_FF], F32),
        ("b1p1c", b1p1c, [128, MT_FF], F32), ("b2adjc", b2adjc, [128, KT_D], F32),
        ("g1c", g1c, [128, KT_D], F32), ("be1c", be1c, [128, KT_D], F32),
        ("g2c", g2c, [128, KT_D], F32), ("be2c", be2c, [128, KT_D], F32),
    ):
        t = pconst.tile(shape, dt, tag=name)
        nc.sync.dma_start(t[:], ap[:])
        cAPs[name] = t
    sel2b_r = pconst.tile([2, 128], F32R, tag="sel2br")
    ones1x128_r = pconst.tile([1, 128], F32R, tag="ones1x128r")
    mean1_r = pconst.tile([128, 1], F32R, tag="mean1r")
    mean1_bf = pconst.tile([128, 1], BF16, tag="mean1bf")
    sel2b_bf = pconst.tile([2, 128], BF16, tag="sel2bbf")
    nc.vector.tensor_copy(sel2b_r[:], cAPs["sel2b"][:])
    nc.vector.tensor_copy(ones1x128_r[:], cAPs["ones1x128"][:])
    nc.vector.tensor_copy(mean1_r[:], cAPs["mean1"][:])
    nc.vector.tensor_copy(mean1_bf[:], cAPs["mean1"][:])
    nc.vector.tensor_copy(sel2b_bf[:], cAPs["sel2b"][:])

    epsln_c = pconst.tile([1, 1], F32, tag="epslnc")
    nc.vector.memset(epsln_c[:], float(EPS_LN))
    onesrow_c = pconst.tile([1, NT], F32, tag="onesrowc")
    nc.vector.memset(onesrow_c[:], 1.0)
    arstage = pone.tile([128, AC], F32, tag="arbuf")
    mxcols = pone.tile([128, B * PAIRS * TT], BF16, tag="mxcols")

    def ln(res, gc, bc, tag, odt, dma_out=None):
        """Feature-major LN: stats via f32r PE reductions, broadcasts via f32r
        K=1 matmuls. res is a [128, KT_D, NT] f32r tile. Returns out tile or,
        if odt is None, yields per-kt fp32 tiles via generator semantics."""
        pm0 = PR.tile([1, NT], F32, tag="prow")
        pm1 = PA_.tile([1, NT], F32, tag="pA")
        for kt in range(KT_D):
            sqt = psm.tile([128, NT], BF16, tag="lnsq")
            nc.scalar.square(sqt[:], res[:, kt, :].bitcast(F32))
            nc.tensor.matmul(pm0[:], mean1_r[:], res[:, kt, :],
                             start=kt == 0, stop=kt == KT_D - 1,
                             skip_group_check=True)
            nc.tensor.matmul(pm1[:], mean1_bf[:], sqt[:],
                             start=kt == 0, stop=kt == KT_D - 1,
                             skip_group_check=True)
        mu = psm.tile([1, NT], F32R, tag="lnmu")
        nc.vector.tensor_copy(mu[:], pm0[:])
        muf = mu[:].bitcast(F32)
        mu2 = psm.tile([1, NT], F32, tag="lnrow")
        nc.vector.tensor_tensor(mu2[:], muf, muf, op=ALU.mult)
        var = psm.tile([1, NT], F32, tag="lnrow")
        nc.vector.tensor_tensor(var[:], pm1[:], mu2[:], op=ALU.subtract)
        lnv = psm.tile([1, NT], F32, tag="lnrow")
        nc.scalar.activation(lnv[:], var[:], AF.Ln, bias=epsln_c[:])
        rstdf = psm.tile([1, NT], F32, tag="lnrow")
        nc.scalar.activation(rstdf[:], lnv[:], AF.Exp, scale=-0.5)
        rstd = psm.tile([1, NT], F32R, tag="lnrstd")
        nc.vector.tensor_copy(rstd[:], rstdf[:])
        pmu = PP.tile([128, NT], F32, tag="pbig")
        nc.tensor.matmul(pmu[:], ones1x128_r[:], mu[:], start=True, stop=True)
        prs = PP.tile([128, NT], F32, tag="pbig")
        nc.tensor.matmul(prs[:], ones1x128_r[:], rstd[:], start=True, stop=True)
        o = None
        if odt is not None:
            o = pbig.tile([128, KT_D, NT], odt, tag="bf8")
        cen = psm.tile([128, NT], F32, tag="lncen")
        nrm = psm.tile([128, NT], F32, tag="lnnrm")
        for kt in range(KT_D):
            nc.vector.tensor_tensor(cen[:], res[:, kt, :].bitcast(F32), pmu[:],
                                    op=ALU.subtract)
            nc.vector.tensor_tensor(nrm[:], cen[:], prs[:], op=ALU.mult)
            if o is not None:
                nc.vector.tensor_scalar(o[:, kt, :], nrm[:], gc[:, kt:kt + 1],
                                        bc[:, kt:kt + 1], op0=ALU.mult, op1=ALU.add)
            else:
                ot = psm.tile([128, NT], F32, tag="lnot")
                nc.vector.tensor_scalar(ot[:], nrm[:], gc[:, kt:kt + 1],
                                        bc[:, kt:kt + 1], op0=ALU.mult, op1=ALU.add)
                nc.sync.dma_start(dma_out[0][dma_out[1], kt * 128:(kt + 1) * 128, :],
                                  ot[:])
        return o

    # ================= stage A =================
    for b in range(B):
        xbf = pxa.tile([128, KT_D, NT], BF16, tag="xbf")
        nc.sync.dma_start(xbf[:], x_bf[b].rearrange("(kt p) t -> p kt t", p=128))

        vtok = pxa.tile([128, TT, PAIRS, 129], BF16, tag="vtok")
        nc.vector.memset(vtok[:, :, :, 128:129], 1.0)
        for tt in range(TT):
            for nh in range(2):
                pv = PP.tile([128, 4, 128], F32, tag="pbig")
                for kt in range(KT_D):
                    nc.tensor.matmul(pv[:], xbf[:, kt, tt * 128:(tt + 1) * 128],
                                     wv_sb[:, kt, nh * 512:(nh + 1) * 512],
                                     start=kt == 0, stop=kt == KT_D - 1)
                nc.vector.tensor_copy(vtok[:, tt, nh * 4:(nh + 1) * 4, 0:128],
                                      pv[:])

        for pr in range(PAIRS):
            wkmt = pstream.tile([128, KT_D, 128], BF16, tag="wmt")
            nc.sync.dma_start(wkmt[:], wks[pr])
            pk = PP.tile([128, NT], F32, tag="pbig")
            for kt in range(KT_D):
                nc.tensor.matmul(pk[:], wkmt[:, kt, :], xbf[:, kt, :],
                                 start=kt == 0, stop=kt == KT_D - 1)
            kTmt = pmt.tile([128, NT], BF16, tag="mt512")
            nc.scalar.activation(kTmt[:], pk[:], AF.Copy)
            ksqmt = pmt.tile([128, NT], BF16, tag="mt512")
            nc.scalar.square(ksqmt[:], pk[:])
            pks = PR.tile([2, NT], F32, tag="prow")
            nc.tensor.matmul(pks[:], cAPs["sel2"][:], ksqmt[:], start=True, stop=True)
            ksq2 = psm.tile([2, NT], BF16, tag="ksq2")
            nc.scalar.activation(ksq2[:], pks[:], AF.Copy)

            Ek = psm.tile([128, TT, 128], BF16, tag="Ek")
            for tt in range(TT):
                pdd = PD.tile([128, 128], F32, tag="pdd")
                nc.tensor.matmul(pdd[:], kTmt[:, tt * 128:(tt + 1) * 128],
                                 cAPs["projbd"][:], start=True, stop=True)
                c = (b * PAIRS + pr) * TT + tt
                nc.vector.tensor_reduce(mxcols[:, c:c + 1], pdd[:], axis=AX.X,
                                        op=ALU.max)
                nc.tensor.matmul(pdd[:], ksq2[:, tt * 128:(tt + 1) * 128],
                                 cAPs["negselF"][:], start=False, stop=True,
                                 skip_group_check=True)
                nc.scalar.activation(Ek[:, tt, :], pdd[:], AF.Exp)

            pA = PA_.tile([128, 129], F32, tag="pA")
            for tt in range(TT):
                nc.tensor.matmul(pA[:], Ek[:, tt, :], vtok[:, tt, pr, :],
                                 start=tt == 0, stop=tt == TT - 1,
                                 skip_group_check=True)
            j = b * PAIRS + pr
            nc.vector.tensor_copy(arstage[0:64, j * 64:(j + 1) * 64], pA[0:64, 0:64])
            nc.vector.tensor_copy(arstage[64:128, j * 64:(j + 1) * 64],
                                  pA[64:128, 64:128])
            nc.vector.tensor_copy(arstage[:, AC_A + j:AC_A + j + 1], pA[:, 128:129])

    # ---- fire AllReduce ----
    mxr = pone.tile([128, 1], F32, tag="mxr")
    nc.vector.tensor_reduce(mxr[:], mxcols[:], axis=AX.X, op=ALU.max)
    mx1 = pone.tile([1, 1], F32, tag="mx1")
    nc.gpsimd.tensor_reduce(mx1[:], mxr[:], axis=AX.C, op=ALU.max)
    nc.vector.tensor_scalar(arstage[0:1, AC_A + AC_U:AC], cAPs["onehot"][:],
                            mx1[:], None, op0=ALU.mult)
    arin = pdram.tile([128, AC], F32, tag="arin")
    arout = pdram.tile([128, AC], F32, tag="arout")
    nc.sync.dma_start(arin[:], arstage[:])
    if os.environ.get("KERNEL_NOCOLL"):
        nc.sync.dma_start(arout[:], arin[:])
    else:
        nc.gpsimd.collective_compute("AllReduce", ALU.add,
                                     replica_groups=[list(range(NC))],
                                     ins=[arin[:]], outs=[arout[:]])
    arres = pone.tile([128, AC], F32, tag="arbuf")
    nc.sync.dma_start(arres[:], arout[:])

    # ================= q-side features (overlap AR) =================
    Eq_all = {}

    def emit_qside(b):
        qxbf = pxa.tile([128, KT_D, NT], BF16, tag="qxbf")
        nc.sync.dma_start(qxbf[:], x_bf[b].rearrange("(kt p) t -> p kt t", p=128))
        Eqs = []
        for pr in range(PAIRS):
            wqmt = pstream.tile([128, KT_D, 128], BF16, tag="wmt")
            nc.sync.dma_start(wqmt[:], wqs[pr])
            pq_ = PP.tile([128, NT], F32, tag="pbig")
            for kt in range(KT_D):
                nc.tensor.matmul(pq_[:], wqmt[:, kt, :], qxbf[:, kt, :],
                                 start=kt == 0, stop=kt == KT_D - 1)
            qTmt = pmt.tile([128, NT], BF16, tag="mt512")
            nc.scalar.activation(qTmt[:], pq_[:], AF.Copy)
            qsqmt = pmt.tile([128, NT], BF16, tag="mt512")
            nc.scalar.square(qsqmt[:], pq_[:])
            pqs = PR.tile([2, NT], F32, tag="prow")
            nc.tensor.matmul(pqs[:], cAPs["sel2"][:], qsqmt[:], start=True, stop=True)
            qsq2 = psm.tile([2, NT], BF16, tag="qsq2")
            nc.scalar.activation(qsq2[:], pqs[:], AF.Copy)

            pdq = PP.tile([128, NT], F32, tag="pbig")
            nc.tensor.matmul(pdq[:], cAPs["projbd"][:], qTmt[:], start=True,
                             stop=False)
            nc.tensor.matmul(pdq[:], cAPs["negselF"][:], qsq2[:], start=False,
                             stop=True, skip_group_check=True)
            Etmp = pmt.tile([128, NT], BF16, tag="t512bf")
            nc.scalar.activation(Etmp[:], pdq[:], AF.Exp)

            pS = PR.tile([2, NT], F32, tag="prow")
            nc.tensor.matmul(pS[:], cAPs["sel2"][:], Etmp[:], start=True, stop=True)
            ediag = psm.tile([2, NT], BF16, tag="ediag")
            nc.scalar.activation(ediag[:], qsq2[:], AF.Exp, scale=float(DN2H))
            wrow = psm.tile([2, NT], BF16, tag="wrow")
            nc.vector.scalar_tensor_tensor(wrow[:], ediag[:], EPS_KERN, pS[:],
                                           op0=ALU.mult, op1=ALU.mult)
            pwB = PP.tile([128, NT], F32, tag="pbig")
            nc.tensor.matmul(pwB[:], sel2b_bf[:], wrow[:], start=True, stop=True)
            Eq = peq.tile([128, NT], BF16, tag=f"Eq{pr}")
            nc.vector.tensor_tensor(Eq[:], Etmp[:], pwB[:], op=ALU.add)
            Eqs.append(Eq)
        Eq_all[b] = Eqs

    emit_qside(0)

    # ---- kv / ksum assembly ----
    mx8 = pone.tile([1, 1], F32, tag="mx8")
    nc.vector.tensor_reduce(mx8[:], arres[0:1, AC_A + AC_U:AC], axis=AX.X, op=ALU.max)
    emxf = pone.tile([1, 1], F32, tag="emxf")
    nc.scalar.activation(emxf[:], mx8[:], AF.Exp, scale=-1.0)
    emxrow = psm.tile([1, NT], F32R, tag="lnrow")
    nc.vector.tensor_scalar(emxrow[:], onesrow_c[:], emxf[:], None, op0=ALU.mult)
    pex = PP.tile([128, NT], F32, tag="pbig")
    nc.tensor.matmul(pex[:], ones1x128_r[:], emxrow[:], start=True, stop=True)
    emxc = pone.tile([128, 1], F32, tag="emxc")
    nc.vector.tensor_copy(emxc[:], pex[:, 0:1])

    epsv_sb = pbig.tile([128, B * D], BF16, tag="big32")
    nc.sync.dma_start(epsv_sb[:], epsvB[:])

    kvBall = pone.tile([128, B * PAIRS, 130], BF16, tag="kvBall")
    nc.vector.memset(kvBall[:], 0.0)
    kvB_all = {}
    for b in range(B):
        for pr in range(PAIRS):
            j = b * PAIRS + pr
            kvB = kvBall[:, j, :]
            nc.vector.scalar_tensor_tensor(
                kvB[0:64, 0:64], arres[0:64, j * 64:(j + 1) * 64], emxc[0:64, :],
                epsv_sb[0:64, b * D + pr * 128:b * D + pr * 128 + 64],
                op0=ALU.mult, op1=ALU.add)
            nc.vector.scalar_tensor_tensor(
                kvB[64:128, 64:128], arres[64:128, j * 64:(j + 1) * 64],
                emxc[64:128, :],
                epsv_sb[64:128, b * D + pr * 128 + 64:b * D + pr * 128 + 128],
                op0=ALU.mult, op1=ALU.add)
            ksf = psm.tile([128, 1], F32, tag="ksf")
            nc.vector.tensor_scalar(ksf[:], arres[:, AC_A + j:AC_A + j + 1],
                                    emxc[:], float(EPS_KERN * N),
                                    op0=ALU.mult, op1=ALU.add)
            nc.vector.tensor_scalar(kvB[:, 128:130], cAPs["headmask2"][:], ksf[:],
                                    None, op0=ALU.mult)
            kvB_all[j] = kvB

    # ================= stage B =================
    for b in range(B):
        Eqs = Eq_all[b]
        attnT = pbig.tile([128, KT_D, NT], BF16, tag="bf8")
        for pr in range(PAIRS):
            kvB = kvB_all[b * PAIRS + pr]
            pnum = PP.tile([128, NT], F32, tag="pbig")
            nc.tensor.matmul(pnum[:], kvB[:, 0:128], Eqs[pr][:], start=True,
                             stop=True)
            pden = PR.tile([2, NT], F32, tag="prow")
            nc.tensor.matmul(pden[:], kvB[:, 128:130], Eqs[pr][:], start=True,
                             stop=True)
            rdf = psm.tile([2, NT], F32, tag="rdf")
            nc.vector.reciprocal_approx_fast(rdf[:], pden[:])
            rden = psm.tile([2, NT], F32R, tag="rden")
            nc.vector.tensor_copy(rden[:], rdf[:])
            prdB = PP.tile([128, NT], F32, tag="pbig")
            nc.tensor.matmul(prdB[:], sel2b_r[:], rden[:], start=True, stop=True)
            numsb = psm.tile([128, NT], BF16, tag="numsb")
            nc.scalar.activation(numsb[:], pnum[:], AF.Copy)
            nc.vector.tensor_tensor(attnT[:, pr, :], numsb[:], prdB[:], op=ALU.mult)

        res1 = pbig.tile([128, KT_D, NT], F32R, tag="resX")
        for mt in range(KT_D):
            womt = pstream.tile([128, KT_D, 128], BF16, tag="wmt")
            nc.sync.dma_start(womt[:], wos[mt])
            po = PP.tile([128, NT], F32, tag="pbig")
            for kt in range(KT_D):
                nc.tensor.matmul(po[:], womt[:, kt, :], attnT[:, kt, :],
                                 start=kt == 0, stop=kt == KT_D - 1)
            xf = psm.tile([128, NT], F32, tag="xf")
            nc.sync.dma_start(xf[:], x_f32[b, mt * 128:(mt + 1) * 128, :])
            nc.vector.tensor_tensor(res1[:, mt, :], xf[:], po[:], op=ALU.add)

        out1 = ln(res1, cAPs["g1c"], cAPs["be1c"], "o1", mybir.dt.bfloat16)

        hsb = pbig.tile([128, MT_FF, NT], BF16, tag="big32")
        for mt in range(MT_FF):
            w1mt = pstream.tile([128, KT_D, 128], BF16, tag="wmt")
            nc.sync.dma_start(w1mt[:], w1s[mt])
            pz = PP.tile([128, NT], F32, tag="pbig")
            for kt in range(KT_D):
                nc.tensor.matmul(pz[:], w1mt[:, kt, :], out1[:, kt, :],
                                 start=kt == 0, stop=kt == KT_D - 1)
            # elu(z')+1 = max(z'+1, exp(-relu(-z'))) — min(exp,1) via Relu+Exp
            # keeps everything on the scalar engine (Relu is in every table
            # set, so no ACT_TABLE_LOAD thrash) and off GpSimd entirely.
            nrelu = pmt.tile([128, NT], BF16, tag="t512bf")
            nc.scalar.activation(nrelu[:], pz[:], AF.Relu, scale=-1.0,
                                 bias=cAPs["nb1c"][:, mt:mt + 1])
            emin = pmt.tile([128, NT], BF16, tag="t512bf")
            nc.scalar.activation(emin[:], nrelu[:], AF.Exp, scale=-1.0)
            nc.vector.scalar_tensor_tensor(hsb[:, mt, :], pz[:],
                                           cAPs["b1p1c"][:, mt:mt + 1], emin[:],
                                           op0=ALU.add, op1=ALU.max)

        res2 = pbig.tile([128, KT_D, NT], F32R, tag="resX")
        for mt in range(KT_D):
            w2mt = pw2s.tile([128, MT_FF, 128], BF16, tag="w2mt")
            nc.sync.dma_start(w2mt[:], w2s[mt])
            pf = PP.tile([128, NT], F32, tag="pbig")
            for kt in range(MT_FF):
                nc.tensor.matmul(pf[:], w2mt[:, kt, :], hsb[:, kt, :],
                                 start=kt == 0, stop=kt == MT_FF - 1)
            nc.vector.scalar_tensor_tensor(res2[:, mt, :], pf[:],
                                           cAPs["b2adjc"][:, mt:mt + 1],
                                           out1[:, mt, :], op0=ALU.add, op1=ALU.add)

        ln(res2, cAPs["g2c"], cAPs["be2c"], "o2", None, dma_out=(out, b))

        if b + 1 < B:
            emit_qside(b + 1)

    ctx.close()


_CACHE = {}


def _build():
    import concourse.tile as tile
    from concourse import bacc
    nc = bacc.Bacc("TRN2", target_bir_lowering=False, debug=False, num_devices=NC)
    with tile.TileContext(nc) as tc:
        _emit(nc, tc)
    nc.compile()
    return nc


def _host_inputs(x, Wq, Wk, Wv, Wo, proj, W1, b1, W2, b2,
                 ln1_g, ln1_b, ln2_g, ln2_b):
    bf = ml_dtypes.bfloat16
    f32 = np.float32
    d = {}

    def chunked(w):  # [D, X] -> [X/128 mt, 128 p, D/128 kt, 128]
        Dk, X = w.shape
        r = w.reshape(Dk // 128, 128, X // 128, 128)
        return np.ascontiguousarray(r.transpose(2, 1, 0, 3)).astype(bf)

    d["wqs"] = chunked(Wq.reshape(D, D))
    d["wks"] = chunked(Wk.reshape(D, D))
    d["wv"] = np.ascontiguousarray(Wv.reshape(D, D)).astype(bf)
    d["wos"] = chunked(Wo.reshape(D, D))
    d["w1s"] = chunked(W1)
    d["w2s"] = chunked(W2)

    projT_s = (proj * DN).T.astype(f32)
    pbd = np.zeros((128, 128), f32)
    pbd[0:64, 0:64] = projT_s
    pbd[64:128, 64:128] = projT_s
    d["projbd"] = pbd.astype(bf)
    nsF = np.zeros((2, 128), f32)
    nsF[0, 0:64] = -DN2H
    nsF[1, 64:128] = -DN2H
    d["negselF"] = nsF.astype(bf)
    s2 = np.zeros((128, 2), f32)
    s2[0:64, 0] = 1.0
    s2[64:128, 1] = 1.0
    d["sel2"] = s2.astype(bf)
    s2b = np.zeros((2, 128), f32)
    s2b[0, 0:64] = 1.0
    s2b[1, 64:128] = 1.0
    d["sel2b"] = s2b
    d["ones128"] = np.ones((128, 1), f32).astype(bf)
    d["ones1x128"] = np.ones((1, 128), f32)
    d["mean1"] = np.full((128, 1), 1.0 / D, f32)
    hm2 = np.zeros((128, 2), f32)
    hm2[0:64, 0] = 1.0
    hm2[64:128, 1] = 1.0
    d["headmask2"] = hm2

    xsum = x.sum(axis=1, dtype=np.float64)
    vsum = xsum @ Wv.reshape(D, D).astype(np.float64)
    epsv = (EPS_KERN * vsum).astype(f32)
    d["epsvB"] = np.ascontiguousarray(
        np.broadcast_to(epsv.reshape(1, B * D), (128, B * D))).astype(bf)

    d["b1c"] = np.ascontiguousarray(b1.reshape(MT_FF, 128).T).astype(f32)
    d["nb1c"] = np.ascontiguousarray((-b1).reshape(MT_FF, 128).T).astype(f32)
    d["b1p1c"] = np.ascontiguousarray((b1 + 1.0).reshape(MT_FF, 128).T).astype(f32)
    b2adj = b2.astype(np.float64) - W2.astype(np.float64).sum(axis=0)
    d["b2adjc"] = np.ascontiguousarray(b2adj.reshape(KT_D, 128).T).astype(f32)
    d["g1c"] = np.ascontiguousarray(ln1_g.reshape(KT_D, 128).T).astype(f32)
    d["be1c"] = np.ascontiguousarray(ln1_b.reshape(KT_D, 128).T).astype(f32)
    d["g2c"] = np.ascontiguousarray(ln2_g.reshape(KT_D, 128).T).astype(f32)
    d["be2c"] = np.ascontiguousarray(ln2_b.reshape(KT_D, 128).T).astype(f32)
    return d


def kernel(x, Wq, Wk, Wv, Wo, proj, W1, b1, W2, b2, ln1_g, ln1_b, ln2_g, ln2_b):
    from concourse import bass_utils

    x = np.asarray(x, np.float32)
    shared = _host_inputs(x, np.asarray(Wq), np.asarray(Wk), np.asarray(Wv),
                          np.asarray(Wo), np.asarray(proj), np.asarray(W1),
                          np.asarray(b1), np.asarray(W2), np.asarray(b2),
                          np.asarray(ln1_g), np.asarray(ln1_b),
                          np.asarray(ln2_g), np.asarray(ln2_b))

    if "nc" not in _CACHE:
        _CACHE["nc"] = _build()
    nc = _CACHE["nc"]

    in_maps = []
    for c in range(NC):
        xs = x[:, c * NT:(c + 1) * NT, :]
        xT = np.ascontiguousarray(xs.transpose(0, 2, 1))
        oh = np.zeros((1, NC), np.float32)
        oh[0, c] = 1.0
        m = dict(shared)
        m["x_f32"] = xT
        m["x_bf"] = xT.astype(ml_dtypes.bfloat16)
        m["onehot"] = oh
        in_maps.append(m)

    trace = bool(int(os.environ.get("KERNEL_TRACE", "0")))
    res = bass_utils.run_bass_kernel_spmd(nc, in_maps, core_ids=list(range(NC)),
                                          trace=trace)
    if trace and res.exec_time_ns is not None:
        print(f"HW exec time: {res.exec_time_ns} ns")
        if res.instructions_and_trace is not None:
            print("trace:", res.instructions_and_trace[1])

    outp = np.empty((B, N, D), np.float32)
    for c in range(NC):
        oT = res.results[c]["out"]
        outp[:, c * NT:(c + 1) * NT, :] = oT.transpose(0, 2, 1)
    return outp



# revision 29
# speedup vs baseline: 1.6903x; 1.1006x over previous
"""Performer (FAVOR+) encoder layer on 8 trn2 NeuronCores.

Sharding: data-parallel over sequence (512 positions per core x 4 batches).
The linear-attention summaries (A = E_k^T v per (batch, head), usum) and the
global key-feature max (via one-hot slots) are combined in ONE packed
AllReduce, overlapped with the Q-side feature compute.

Layout: activations feature-major (xT = [D, tokens]) so every GEMM's
stationary operand is a natural weight slice; per-token reductions and
broadcasts are small PE matmuls (selector / ones / f32r broadcast matmuls).
E_k and v are produced token-major directly by matmuls so the token-
contraction A-matmul needs no transposes.
"""
import os
import numpy as np
import ml_dtypes

B, N, D = 4, 4096, 1024
H, DH = 16, 64
DFF = 4096
M = 64
EPS_KERN = 1e-6
EPS_LN = 1e-6
NC = 8
NT = N // NC                # 512 positions per core per batch
PAIRS = H // 2              # 8 head-pairs
KT_D = D // 128             # 8
MT_FF = DFF // 128          # 32
TT = NT // 128              # 4
DN = 1.0 / np.sqrt(np.sqrt(DH))
DN2H = DN * DN / 2.0


def _emit(nc, tc):
    import concourse.mybir as mybir
    from contextlib import ExitStack
    F32 = mybir.dt.float32
    F32R = mybir.dt.float32r
    BF16 = mybir.dt.bfloat16
    AF = mybir.ActivationFunctionType
    ALU = mybir.AluOpType
    AX = mybir.AxisListType

    dram = lambda name, shape, dt, kind: nc.dram_tensor(name, shape, dt, kind=kind).ap()

    x_bf = dram("x_bf", [B, D, NT], BF16, "ExternalInput")
    x_f32 = dram("x_f32", [B, D, NT], F32, "ExternalInput")
    wqs = dram("wqs", [KT_D, 128, KT_D, 128], BF16, "ExternalInput")
    wks = dram("wks", [KT_D, 128, KT_D, 128], BF16, "ExternalInput")
    wv = dram("wv", [D, D], BF16, "ExternalInput")
    wos = dram("wos", [KT_D, 128, KT_D, 128], BF16, "ExternalInput")
    w1s = dram("w1s", [MT_FF, 128, KT_D, 128], BF16, "ExternalInput")
    w2s = dram("w2s", [KT_D, 128, MT_FF, 128], BF16, "ExternalInput")
    projbd = dram("projbd", [128, 128], BF16, "ExternalInput")
    negselF = dram("negselF", [2, 128], BF16, "ExternalInput")
    sel2 = dram("sel2", [128, 2], BF16, "ExternalInput")
    sel2b = dram("sel2b", [2, 128], F32, "ExternalInput")
    ones128 = dram("ones128", [128, 1], BF16, "ExternalInput")
    ones1x128 = dram("ones1x128", [1, 128], F32, "ExternalInput")
    mean1 = dram("mean1", [128, 1], F32, "ExternalInput")
    headmask2 = dram("headmask2", [128, 2], F32, "ExternalInput")
    epsvB = dram("epsvB", [128, B * D], BF16, "ExternalInput")
    onehot = dram("onehot", [1, NC], F32, "ExternalInput")
    b1c = dram("b1c", [128, MT_FF], F32, "ExternalInput")
    nb1c = dram("nb1c", [128, MT_FF], F32, "ExternalInput")
    b1p1c = dram("b1p1c", [128, MT_FF], F32, "ExternalInput")
    b2adjc = dram("b2adjc", [128, KT_D], F32, "ExternalInput")
    g1c = dram("g1c", [128, KT_D], F32, "ExternalInput")
    be1c = dram("be1c", [128, KT_D], F32, "ExternalInput")
    g2c = dram("g2c", [128, KT_D], F32, "ExternalInput")
    be2c = dram("be2c", [128, KT_D], F32, "ExternalInput")
    out = dram("out", [B, D, NT], F32, "ExternalOutput")

    AC_A = B * PAIRS * 64
    AC_U = B * PAIRS
    AC = AC_A + AC_U + NC

    ctx = ExitStack()
    pconst = ctx.enter_context(tc.tile_pool(name="pconst", bufs=1))
    pstream = ctx.enter_context(tc.tile_pool(name="pstream", bufs=2))
    pw2s = ctx.enter_context(tc.tile_pool(name="pw2s", bufs=2))
    pxa = ctx.enter_context(tc.tile_pool(name="pxa", bufs=1))
    pmt = ctx.enter_context(tc.tile_pool(name="pmt", bufs=4))
    psm = ctx.enter_context(tc.tile_pool(name="psm", bufs=2))
    peq = ctx.enter_context(tc.tile_pool(name="peq", bufs=1))
    pbig = ctx.enter_context(tc.tile_pool(name="pbig", bufs=1))
    pone = ctx.enter_context(tc.tile_pool(name="pone", bufs=1))
    pdram = ctx.enter_context(tc.tile_pool(name="pdram", bufs=1, space="DRAM"))
    PP = ctx.enter_context(tc.tile_pool(name="PP", bufs=4, space="PSUM"))
    PD = ctx.enter_context(tc.tile_pool(name="PD", bufs=2, space="PSUM"))
    PA_ = ctx.enter_context(tc.tile_pool(name="PA", bufs=1, space="PSUM"))
    PR = ctx.enter_context(tc.tile_pool(name="PR", bufs=1, space="PSUM"))

    # ---- constants ----
    # wv shares the big32 slot with epsv/hsb: wv is only read in stage A,
    # epsv only in kv assembly, hsb only from FFN1 onward — disjoint uses.
    wv_sb = pbig.tile([128, KT_D, D], BF16, tag="big32")
    nc.sync.dma_start(wv_sb[:], wv.rearrange("(kt p) m -> p kt m", p=128))
    cAPs = {}
    for name, ap, shape, dt in (
        ("projbd", projbd, [128, 128], BF16), ("negselF", negselF, [2, 128], BF16),
        ("sel2", sel2, [128, 2], BF16), ("sel2b", sel2b, [2, 128], F32),
        ("ones128", ones128, [128, 1], BF16), ("ones1x128", ones1x128, [1, 128], F32),
        ("mean1", mean1, [128, 1], F32), ("headmask2", headmask2, [128, 2], F32),
        ("onehot", onehot, [1, NC], F32), ("b1c", b1c, [128, MT_FF], F32),
        ("nb1c", nb1c, [128, MT_FF], F32),
        ("b1p1c", b1p1c, [128, MT_FF], F32), ("b2adjc", b2adjc, [128, KT_D], F32),
        ("g1c", g1c, [128, KT_D], F32), ("be1c", be1c, [128, KT_D], F32),
        ("g2c", g2c, [128, KT_D], F32), ("be2c", be2c, [128, KT_D], F32),
    ):
        t = pconst.tile(shape, dt, tag=name)
        nc.sync.dma_start(t[:], ap[:])
        cAPs[name] = t
    sel2b_r = pconst.tile([2, 128], F32R, tag="sel2br")
    ones1x128_r = pconst.tile([1, 128], F32R, tag="ones1x128r")
    mean1_r = pconst.tile([128, 1], F32R, tag="mean1r")
    mean1_bf = pconst.tile([128, 1], BF16, tag="mean1bf")
    sel2b_bf = pconst.tile([2, 128], BF16, tag="sel2bbf")
    nc.vector.tensor_copy(sel2b_r[:], cAPs["sel2b"][:])
    nc.vector.tensor_copy(ones1x128_r[:], cAPs["ones1x128"][:])
    nc.vector.tensor_copy(mean1_r[:], cAPs["mean1"][:])
    nc.vector.tensor_copy(mean1_bf[:], cAPs["mean1"][:])
    nc.vector.tensor_copy(sel2b_bf[:], cAPs["sel2b"][:])

    epsln_c = pconst.tile([1, 1], F32, tag="epslnc")
    nc.vector.memset(epsln_c[:], float(EPS_LN))
    onesrow_c = pconst.tile([1, NT], F32, tag="onesrowc")
    nc.vector.memset(onesrow_c[:], 1.0)
    arstage = pone.tile([128, AC], F32, tag="arbuf")
    mxcols = pone.tile([128, B * PAIRS * TT], BF16, tag="mxcols")

    def ln(res, gc, bc, tag, odt, dma_out=None):
        """Feature-major LN: stats via f32r PE reductions, broadcasts via f32r
        K=1 matmuls. res is a [128, KT_D, NT] f32r tile. Apply is fused to two
        DVE ops per kt (cen, then (cen*g)*S) with the +b on the scalar engine."""
        pm0 = PR.tile([1, NT], F32, tag="prow")
        pm1 = PA_.tile([1, NT], F32, tag="pA")
        for kt in range(KT_D):
            sqt = psm.tile([128, NT], BF16, tag="lnsq")
            nc.scalar.square(sqt[:], res[:, kt, :].bitcast(F32))
            nc.tensor.matmul(pm0[:], mean1_r[:], res[:, kt, :],
                             start=kt == 0, stop=kt == KT_D - 1,
                             skip_group_check=True)
            nc.tensor.matmul(pm1[:], mean1_bf[:], sqt[:],
                             start=kt == 0, stop=kt == KT_D - 1,
                             skip_group_check=True)
        mu = psm.tile([1, NT], F32R, tag="lnmu")
        nc.vector.tensor_copy(mu[:], pm0[:])
        muf = mu[:].bitcast(F32)
        mu2 = psm.tile([1, NT], F32, tag="lnrow")
        nc.vector.tensor_tensor(mu2[:], muf, muf, op=ALU.mult)
        var = psm.tile([1, NT], F32, tag="lnrow")
        nc.vector.tensor_tensor(var[:], pm1[:], mu2[:], op=ALU.subtract)
        lnv = psm.tile([1, NT], F32, tag="lnrow")
        nc.scalar.activation(lnv[:], var[:], AF.Ln, bias=epsln_c[:])
        rstdf = psm.tile([1, NT], F32, tag="lnrow")
        nc.scalar.activation(rstdf[:], lnv[:], AF.Exp, scale=-0.5)
        rstd = psm.tile([1, NT], F32R, tag="lnrstd")
        nc.vector.tensor_copy(rstd[:], rstdf[:])
        pmu = PP.tile([128, NT], F32, tag="pbig")
        nc.tensor.matmul(pmu[:], ones1x128_r[:], mu[:], start=True, stop=True)
        prs = PP.tile([128, NT], F32, tag="pbig")
        nc.tensor.matmul(prs[:], ones1x128_r[:], rstd[:], start=True, stop=True)
        o = None
        if odt is not None:
            o = pbig.tile([128, KT_D, NT], odt, tag="bf8")
        cen = psm.tile([128, NT], F32, tag="lncen")
        nrm = psm.tile([128, NT], BF16 if odt is not None else F32, tag="lnnrm")
        for kt in range(KT_D):
            nc.vector.tensor_tensor(cen[:], res[:, kt, :].bitcast(F32), pmu[:],
                                    op=ALU.subtract)
            nc.vector.scalar_tensor_tensor(nrm[:], cen[:], gc[:, kt:kt + 1],
                                           prs[:], op0=ALU.mult, op1=ALU.mult)
            if o is not None:
                nc.scalar.activation(o[:, kt, :], nrm[:], AF.Identity,
                                     bias=bc[:, kt:kt + 1])
            else:
                ot = psm.tile([128, NT], F32, tag="lnot")
                nc.scalar.activation(ot[:], nrm[:], AF.Identity,
                                     bias=bc[:, kt:kt + 1])
                nc.sync.dma_start(dma_out[0][dma_out[1], kt * 128:(kt + 1) * 128, :],
                                  ot[:])
        return o

    # ================= stage A =================
    for b in range(B):
        xbf = pxa.tile([128, KT_D, NT], BF16, tag="xbf")
        nc.sync.dma_start(xbf[:], x_bf[b].rearrange("(kt p) t -> p kt t", p=128))

        vtok = pxa.tile([128, TT, PAIRS, 129], BF16, tag="vtok")
        nc.vector.memset(vtok[:, :, :, 128:129], 1.0)
        for tt in range(TT):
            for nh in range(2):
                pv = PP.tile([128, 4, 128], F32, tag="pbig")
                for kt in range(KT_D):
                    nc.tensor.matmul(pv[:], xbf[:, kt, tt * 128:(tt + 1) * 128],
                                     wv_sb[:, kt, nh * 512:(nh + 1) * 512],
                                     start=kt == 0, stop=kt == KT_D - 1)
                nc.vector.tensor_copy(vtok[:, tt, nh * 4:(nh + 1) * 4, 0:128],
                                      pv[:])

        for pr in range(PAIRS):
            wkmt = pstream.tile([128, KT_D, 128], BF16, tag="wmt")
            nc.sync.dma_start(wkmt[:], wks[pr])
            pk = PP.tile([128, NT], F32, tag="pbig")
            for kt in range(KT_D):
                nc.tensor.matmul(pk[:], wkmt[:, kt, :], xbf[:, kt, :],
                                 start=kt == 0, stop=kt == KT_D - 1)
            kTmt = pmt.tile([128, NT], BF16, tag="mt512")
            nc.scalar.activation(kTmt[:], pk[:], AF.Copy)
            ksqmt = pmt.tile([128, NT], BF16, tag="mt512")
            nc.scalar.square(ksqmt[:], pk[:])
            pks = PR.tile([2, NT], F32, tag="prow")
            nc.tensor.matmul(pks[:], cAPs["sel2"][:], ksqmt[:], start=True, stop=True)
            ksq2 = psm.tile([2, NT], BF16, tag="ksq2")
            nc.scalar.activation(ksq2[:], pks[:], AF.Copy)

            Ek = psm.tile([128, TT, 128], BF16, tag="Ek")
            pdd = PD.tile([128, NT], F32, tag="pdd")
            for tt in range(TT):
                sl = slice(tt * 128, (tt + 1) * 128)
                nc.tensor.matmul(pdd[:, sl], kTmt[:, sl],
                                 cAPs["projbd"][:], start=True, stop=True)
                c = (b * PAIRS + pr) * TT + tt
                nc.vector.tensor_reduce(mxcols[:, c:c + 1], pdd[:, sl],
                                        axis=AX.X, op=ALU.max)
                nc.tensor.matmul(pdd[:, sl], ksq2[:, sl],
                                 cAPs["negselF"][:], start=False, stop=True,
                                 skip_group_check=True)
            nc.scalar.activation(Ek[:], pdd[:], AF.Exp)

            pA = PA_.tile([128, 129], F32, tag="pA")
            for tt in range(TT):
                nc.tensor.matmul(pA[:], Ek[:, tt, :], vtok[:, tt, pr, :],
                                 start=tt == 0, stop=tt == TT - 1,
                                 skip_group_check=True)
            j = b * PAIRS + pr
            nc.vector.tensor_copy(arstage[0:64, j * 64:(j + 1) * 64], pA[0:64, 0:64])
            nc.vector.tensor_copy(arstage[64:128, j * 64:(j + 1) * 64],
                                  pA[64:128, 64:128])
            nc.vector.tensor_copy(arstage[:, AC_A + j:AC_A + j + 1], pA[:, 128:129])

    # ---- fire AllReduce ----
    mxr = pone.tile([128, 1], F32, tag="mxr")
    nc.vector.tensor_reduce(mxr[:], mxcols[:], axis=AX.X, op=ALU.max)
    mx1 = pone.tile([1, 1], F32, tag="mx1")
    nc.gpsimd.tensor_reduce(mx1[:], mxr[:], axis=AX.C, op=ALU.max)
    nc.vector.tensor_scalar(arstage[0:1, AC_A + AC_U:AC], cAPs["onehot"][:],
                            mx1[:], None, op0=ALU.mult)
    arin = pdram.tile([128, AC], F32, tag="arin")
    arout = pdram.tile([128, AC], F32, tag="arout")
    nc.sync.dma_start(arin[:], arstage[:])
    if os.environ.get("KERNEL_NOCOLL"):
        nc.sync.dma_start(arout[:], arin[:])
    else:
        nc.gpsimd.collective_compute("AllReduce", ALU.add,
                                     replica_groups=[list(range(NC))],
                                     ins=[arin[:]], outs=[arout[:]])
    arres = pone.tile([128, AC], F32, tag="arbuf")
    nc.sync.dma_start(arres[:], arout[:])

    # ================= q-side features (overlap AR) =================
    Eq_all = {}

    def emit_qside(b):
        qxbf = pxa.tile([128, KT_D, NT], BF16, tag="qxbf")
        nc.sync.dma_start(qxbf[:], x_bf[b].rearrange("(kt p) t -> p kt t", p=128))
        Eqs = []
        for pr in range(PAIRS):
            wqmt = pstream.tile([128, KT_D, 128], BF16, tag="wmt")
            nc.sync.dma_start(wqmt[:], wqs[pr])
            pq_ = PP.tile([128, NT], F32, tag="pbig")
            for kt in range(KT_D):
                nc.tensor.matmul(pq_[:], wqmt[:, kt, :], qxbf[:, kt, :],
                                 start=kt == 0, stop=kt == KT_D - 1)
            qTmt = pmt.tile([128, NT], BF16, tag="mt512")
            nc.scalar.activation(qTmt[:], pq_[:], AF.Copy)
            qsqmt = pmt.tile([128, NT], BF16, tag="mt512")
            nc.scalar.square(qsqmt[:], pq_[:])
            pqs = PR.tile([2, NT], F32, tag="prow")
            nc.tensor.matmul(pqs[:], cAPs["sel2"][:], qsqmt[:], start=True, stop=True)
            qsq2 = psm.tile([2, NT], BF16, tag="qsq2")
            nc.scalar.activation(qsq2[:], pqs[:], AF.Copy)

            pdq = PP.tile([128, NT], F32, tag="pbig")
            nc.tensor.matmul(pdq[:], cAPs["projbd"][:], qTmt[:], start=True,
                             stop=False)
            nc.tensor.matmul(pdq[:], cAPs["negselF"][:], qsq2[:], start=False,
                             stop=True, skip_group_check=True)
            Etmp = pmt.tile([128, NT], BF16, tag="t512bf")
            nc.scalar.activation(Etmp[:], pdq[:], AF.Exp)

            pS = PR.tile([2, NT], F32, tag="prow")
            nc.tensor.matmul(pS[:], cAPs["sel2"][:], Etmp[:], start=True, stop=True)
            ediag = psm.tile([2, NT], BF16, tag="ediag")
            nc.scalar.activation(ediag[:], qsq2[:], AF.Exp, scale=float(DN2H))
            wrow = psm.tile([2, NT], BF16, tag="wrow")
            nc.vector.scalar_tensor_tensor(wrow[:], ediag[:], EPS_KERN, pS[:],
                                           op0=ALU.mult, op1=ALU.mult)
            pwB = PP.tile([128, NT], F32, tag="pbig")
            nc.tensor.matmul(pwB[:], sel2b_bf[:], wrow[:], start=True, stop=True)
            Eq = peq.tile([128, NT], BF16, tag=f"Eq{b % 2}_{pr}")
            nc.vector.tensor_tensor(Eq[:], Etmp[:], pwB[:], op=ALU.add)
            Eqs.append(Eq)
        Eq_all[b] = Eqs

    emit_qside(0)
    emit_qside(1)

    # ---- kv / ksum assembly ----
    mx8 = pone.tile([1, 1], F32, tag="mx8")
    nc.vector.tensor_reduce(mx8[:], arres[0:1, AC_A + AC_U:AC], axis=AX.X, op=ALU.max)
    emxf = pone.tile([1, 1], F32, tag="emxf")
    nc.scalar.activation(emxf[:], mx8[:], AF.Exp, scale=-1.0)
    emxrow = psm.tile([1, NT], F32R, tag="lnrow")
    nc.vector.tensor_scalar(emxrow[:], onesrow_c[:], emxf[:], None, op0=ALU.mult)
    pex = PP.tile([128, NT], F32, tag="pbig")
    nc.tensor.matmul(pex[:], ones1x128_r[:], emxrow[:], start=True, stop=True)
    emxc = pone.tile([128, 1], F32, tag="emxc")
    nc.vector.tensor_copy(emxc[:], pex[:, 0:1])

    epsv_sb = pbig.tile([128, B * D], BF16, tag="big32")
    nc.sync.dma_start(epsv_sb[:], epsvB[:])

    kvBall = pone.tile([128, B * PAIRS, 130], BF16, tag="kvBall")
    nc.vector.memset(kvBall[:], 0.0)
    kvB_all = {}
    for b in range(B):
        for pr in range(PAIRS):
            j = b * PAIRS + pr
            kvB = kvBall[:, j, :]
            nc.vector.scalar_tensor_tensor(
                kvB[0:64, 0:64], arres[0:64, j * 64:(j + 1) * 64], emxc[0:64, :],
                epsv_sb[0:64, b * D + pr * 128:b * D + pr * 128 + 64],
                op0=ALU.mult, op1=ALU.add)
            nc.vector.scalar_tensor_tensor(
                kvB[64:128, 64:128], arres[64:128, j * 64:(j + 1) * 64],
                emxc[64:128, :],
                epsv_sb[64:128, b * D + pr * 128 + 64:b * D + pr * 128 + 128],
                op0=ALU.mult, op1=ALU.add)
            ksf = psm.tile([128, 1], F32, tag="ksf")
            nc.vector.tensor_scalar(ksf[:], arres[:, AC_A + j:AC_A + j + 1],
                                    emxc[:], float(EPS_KERN * N),
                                    op0=ALU.mult, op1=ALU.add)
            nc.vector.tensor_scalar(kvB[:, 128:130], cAPs["headmask2"][:], ksf[:],
                                    None, op0=ALU.mult)
            kvB_all[j] = kvB

    # ================= stage B =================
    for b in range(B):
        Eqs = Eq_all[b]
        attnT = pbig.tile([128, KT_D, NT], BF16, tag="attnT")
        # pass 1: dense PE stream of num/den matmuls; scalar+vector trail
        numsbs, rdens = [], []
        for pr in range(PAIRS):
            kvB = kvB_all[b * PAIRS + pr]
            pnum = PP.tile([128, NT], F32, tag="pbig")
            nc.tensor.matmul(pnum[:], kvB[:, 0:128], Eqs[pr][:], start=True,
                             stop=True)
            pden = PD.tile([2, NT], F32, tag="pdd")
            nc.tensor.matmul(pden[:], kvB[:, 128:130], Eqs[pr][:], start=True,
                             stop=True)
            nc.scalar.activation(attnT[:, pr, :], pnum[:], AF.Copy)
            rdf = psm.tile([2, NT], F32, tag="rdf")
            nc.vector.reciprocal_approx_fast(rdf[:], pden[:])
            rden = psm.tile([2, NT], F32R, tag=f"rden{pr}", bufs=1)
            nc.vector.tensor_copy(rden[:], rdf[:])
            rdens.append(rden)
        # pass 2: broadcast 1/den and scale numerators in place
        for pr in range(PAIRS):
            prdB = PP.tile([128, NT], F32, tag="pbig")
            nc.tensor.matmul(prdB[:], sel2b_r[:], rdens[pr][:], start=True,
                             stop=True)
            nc.vector.tensor_tensor(attnT[:, pr, :], attnT[:, pr, :], prdB[:],
                                    op=ALU.mult)

        res1 = pbig.tile([128, KT_D, NT], F32R, tag="resX")
        for mt in range(KT_D):
            womt = pstream.tile([128, KT_D, 128], BF16, tag="wmt")
            nc.sync.dma_start(womt[:], wos[mt])
            po = PP.tile([128, NT], F32, tag="pbig")
            for kt in range(KT_D):
                nc.tensor.matmul(po[:], womt[:, kt, :], attnT[:, kt, :],
                                 start=kt == 0, stop=kt == KT_D - 1)
            xf = psm.tile([128, NT], F32, tag="xf")
            nc.sync.dma_start(xf[:], x_f32[b, mt * 128:(mt + 1) * 128, :])
            nc.vector.tensor_tensor(res1[:, mt, :], xf[:], po[:], op=ALU.add)

        out1 = ln(res1, cAPs["g1c"], cAPs["be1c"], "o1", mybir.dt.bfloat16)

        hsb = pbig.tile([128, MT_FF, NT], BF16, tag="big32")
        for mt in range(MT_FF):
            w1mt = pstream.tile([128, KT_D, 128], BF16, tag="wmt")
            nc.sync.dma_start(w1mt[:], w1s[mt])
            pz = PP.tile([128, NT], F32, tag="pbig")
            for kt in range(KT_D):
                nc.tensor.matmul(pz[:], w1mt[:, kt, :], out1[:, kt, :],
                                 start=kt == 0, stop=kt == KT_D - 1)
            # elu(z')+1 = max(z'+1, exp(-relu(-z'))) — min(exp,1) via Relu+Exp
            # keeps everything on the scalar engine (Relu is in every table
            # set, so no ACT_TABLE_LOAD thrash) and off GpSimd entirely.
            nrelu = pmt.tile([128, NT], BF16, tag="t512bf")
            nc.scalar.activation(nrelu[:], pz[:], AF.Relu, scale=-1.0,
                                 bias=cAPs["nb1c"][:, mt:mt + 1])
            emin = pmt.tile([128, NT], BF16, tag="t512bf")
            nc.scalar.activation(emin[:], nrelu[:], AF.Exp, scale=-1.0)
            nc.vector.scalar_tensor_tensor(hsb[:, mt, :], pz[:],
                                           cAPs["b1p1c"][:, mt:mt + 1], emin[:],
                                           op0=ALU.add, op1=ALU.max)

        res2 = pbig.tile([128, KT_D, NT], F32R, tag="resX")
        for mt in range(KT_D):
            w2a = pw2s.tile([128, MT_FF // 2, 128], BF16, tag="w2mt")
            nc.sync.dma_start(w2a[:], w2s[mt, :, 0:MT_FF // 2])
            w2b = pw2s.tile([128, MT_FF // 2, 128], BF16, tag="w2mt")
            nc.sync.dma_start(w2b[:], w2s[mt, :, MT_FF // 2:MT_FF])
            pf = PP.tile([128, NT], F32, tag="pbig")
            for kt in range(MT_FF):
                w2h = w2a if kt < MT_FF // 2 else w2b
                nc.tensor.matmul(pf[:], w2h[:, kt % (MT_FF // 2), :],
                                 hsb[:, kt, :],
                                 start=kt == 0, stop=kt == MT_FF - 1)
            nc.vector.scalar_tensor_tensor(res2[:, mt, :], pf[:],
                                           cAPs["b2adjc"][:, mt:mt + 1],
                                           out1[:, mt, :], op0=ALU.add, op1=ALU.add)

        ln(res2, cAPs["g2c"], cAPs["be2c"], "o2", None, dma_out=(out, b))

        if b + 2 < B:
            emit_qside(b + 2)

    ctx.close()


_CACHE = {}


def _build():
    import concourse.tile as tile
    from concourse import bacc
    nc = bacc.Bacc("TRN2", target_bir_lowering=False, debug=False, num_devices=NC)
    with tile.TileContext(nc) as tc:
        _emit(nc, tc)
    nc.compile()
    return nc


def _host_inputs(x, Wq, Wk, Wv, Wo, proj, W1, b1, W2, b2,
                 ln1_g, ln1_b, ln2_g, ln2_b):
    bf = ml_dtypes.bfloat16
    f32 = np.float32
    d = {}

    def chunked(w):  # [D, X] -> [X/128 mt, 128 p, D/128 kt, 128]
        Dk, X = w.shape
        r = w.reshape(Dk // 128, 128, X // 128, 128)
        return np.ascontiguousarray(r.transpose(2, 1, 0, 3)).astype(bf)

    d["wqs"] = chunked(Wq.reshape(D, D))
    d["wks"] = chunked(Wk.reshape(D, D))
    d["wv"] = np.ascontiguousarray(Wv.reshape(D, D)).astype(bf)
    d["wos"] = chunked(Wo.reshape(D, D))
    d["w1s"] = chunked(W1)
    d["w2s"] = chunked(W2)

    projT_s = (proj * DN).T.astype(f32)
    pbd = np.zeros((128, 128), f32)
    pbd[0:64, 0:64] = projT_s
    pbd[64:128, 64:128] = projT_s
    d["projbd"] = pbd.astype(bf)
    nsF = np.zeros((2, 128), f32)
    nsF[0, 0:64] = -DN2H
    nsF[1, 64:128] = -DN2H
    d["negselF"] = nsF.astype(bf)
    s2 = np.zeros((128, 2), f32)
    s2[0:64, 0] = 1.0
    s2[64:128, 1] = 1.0
    d["sel2"] = s2.astype(bf)
    s2b = np.zeros((2, 128), f32)
    s2b[0, 0:64] = 1.0
    s2b[1, 64:128] = 1.0
    d["sel2b"] = s2b
    d["ones128"] = np.ones((128, 1), f32).astype(bf)
    d["ones1x128"] = np.ones((1, 128), f32)
    d["mean1"] = np.full((128, 1), 1.0 / D, f32)
    hm2 = np.zeros((128, 2), f32)
    hm2[0:64, 0] = 1.0
    hm2[64:128, 1] = 1.0
    d["headmask2"] = hm2

    xsum = x.sum(axis=1, dtype=np.float64)
    vsum = xsum @ Wv.reshape(D, D).astype(np.float64)
    epsv = (EPS_KERN * vsum).astype(f32)
    d["epsvB"] = np.ascontiguousarray(
        np.broadcast_to(epsv.reshape(1, B * D), (128, B * D))).astype(bf)

    d["b1c"] = np.ascontiguousarray(b1.reshape(MT_FF, 128).T).astype(f32)
    d["nb1c"] = np.ascontiguousarray((-b1).reshape(MT_FF, 128).T).astype(f32)
    d["b1p1c"] = np.ascontiguousarray((b1 + 1.0).reshape(MT_FF, 128).T).astype(f32)
    b2adj = b2.astype(np.float64) - W2.astype(np.float64).sum(axis=0)
    d["b2adjc"] = np.ascontiguousarray(b2adj.reshape(KT_D, 128).T).astype(f32)
    d["g1c"] = np.ascontiguousarray(ln1_g.reshape(KT_D, 128).T).astype(f32)
    d["be1c"] = np.ascontiguousarray(ln1_b.reshape(KT_D, 128).T).astype(f32)
    d["g2c"] = np.ascontiguousarray(ln2_g.reshape(KT_D, 128).T).astype(f32)
    d["be2c"] = np.ascontiguousarray(ln2_b.reshape(KT_D, 128).T).astype(f32)
    return d


def kernel(x, Wq, Wk, Wv, Wo, proj, W1, b1, W2, b2, ln1_g, ln1_b, ln2_g, ln2_b):
    from concourse import bass_utils

    x = np.asarray(x, np.float32)
    shared = _host_inputs(x, np.asarray(Wq), np.asarray(Wk), np.asarray(Wv),
                          np.asarray(Wo), np.asarray(proj), np.asarray(W1),
                          np.asarray(b1), np.asarray(W2), np.asarray(b2),
                          np.asarray(ln1_g), np.asarray(ln1_b),
                          np.asarray(ln2_g), np.asarray(ln2_b))

    if "nc" not in _CACHE:
        _CACHE["nc"] = _build()
    nc = _CACHE["nc"]

    in_maps = []
    for c in range(NC):
        xs = x[:, c * NT:(c + 1) * NT, :]
        xT = np.ascontiguousarray(xs.transpose(0, 2, 1))
        oh = np.zeros((1, NC), np.float32)
        oh[0, c] = 1.0
        m = dict(shared)
        m["x_f32"] = xT
        m["x_bf"] = xT.astype(ml_dtypes.bfloat16)
        m["onehot"] = oh
        in_maps.append(m)

    trace = bool(int(os.environ.get("KERNEL_TRACE", "0")))
    res = bass_utils.run_bass_kernel_spmd(nc, in_maps, core_ids=list(range(NC)),
                                          trace=trace)
    if trace and res.exec_time_ns is not None:
        print(f"HW exec time: {res.exec_time_ns} ns")
        if res.instructions_and_trace is not None:
            print("trace:", res.instructions_and_trace[1])

    outp = np.empty((B, N, D), np.float32)
    for c in range(NC):
        oT = res.results[c]["out"]
        outp[:, c * NT:(c + 1) * NT, :] = oT.transpose(0, 2, 1)
    return outp



# revision 34
# speedup vs baseline: 1.6967x; 1.0038x over previous
"""Performer (FAVOR+) encoder layer on 8 trn2 NeuronCores.

Sharding: data-parallel over sequence (512 positions per core x 4 batches).
The linear-attention summaries (A = E_k^T v per (batch, head), usum) and the
global key-feature max (via one-hot slots) are combined in ONE packed
AllReduce, overlapped with the Q-side feature compute.

Layout: activations feature-major (xT = [D, tokens]) so every GEMM's
stationary operand is a natural weight slice; per-token reductions and
broadcasts are small PE matmuls (selector / ones / f32r broadcast matmuls).
E_k and v are produced token-major directly by matmuls so the token-
contraction A-matmul needs no transposes.
"""
import os
import numpy as np
import ml_dtypes

B, N, D = 4, 4096, 1024
H, DH = 16, 64
DFF = 4096
M = 64
EPS_KERN = 1e-6
EPS_LN = 1e-6
NC = 8
NT = N // NC                # 512 positions per core per batch
PAIRS = H // 2              # 8 head-pairs
KT_D = D // 128             # 8
MT_FF = DFF // 128          # 32
TT = NT // 128              # 4
DN = 1.0 / np.sqrt(np.sqrt(DH))
DN2H = DN * DN / 2.0


def _emit(nc, tc):
    import concourse.mybir as mybir
    from contextlib import ExitStack
    F32 = mybir.dt.float32
    F32R = mybir.dt.float32r
    BF16 = mybir.dt.bfloat16
    AF = mybir.ActivationFunctionType
    ALU = mybir.AluOpType
    AX = mybir.AxisListType

    dram = lambda name, shape, dt, kind: nc.dram_tensor(name, shape, dt, kind=kind).ap()

    x_bf = dram("x_bf", [B, D, NT], BF16, "ExternalInput")
    x_f32 = dram("x_f32", [B, D, NT], F32, "ExternalInput")
    wqs = dram("wqs", [KT_D, 128, KT_D, 128], BF16, "ExternalInput")
    wks = dram("wks", [KT_D, 128, KT_D, 128], BF16, "ExternalInput")
    wv = dram("wv", [D, D], BF16, "ExternalInput")
    wos = dram("wos", [KT_D, 128, KT_D, 128], BF16, "ExternalInput")
    w1s = dram("w1s", [MT_FF, 128, KT_D, 128], BF16, "ExternalInput")
    w2s = dram("w2s", [KT_D, 128, MT_FF, 128], BF16, "ExternalInput")
    projbd = dram("projbd", [128, 128], BF16, "ExternalInput")
    negselF = dram("negselF", [2, 128], BF16, "ExternalInput")
    sel2 = dram("sel2", [128, 2], BF16, "ExternalInput")
    sel2b = dram("sel2b", [2, 128], F32, "ExternalInput")
    ones128 = dram("ones128", [128, 1], BF16, "ExternalInput")
    ones1x128 = dram("ones1x128", [1, 128], F32, "ExternalInput")
    mean1 = dram("mean1", [128, 1], F32, "ExternalInput")
    headmask2 = dram("headmask2", [128, 2], F32, "ExternalInput")
    epsvB = dram("epsvB", [128, B * D], BF16, "ExternalInput")
    onehot = dram("onehot", [1, NC], F32, "ExternalInput")
    b1c = dram("b1c", [128, MT_FF], F32, "ExternalInput")
    nb1c = dram("nb1c", [128, MT_FF], F32, "ExternalInput")
    b1p1c = dram("b1p1c", [128, MT_FF], F32, "ExternalInput")
    b2adjc = dram("b2adjc", [128, KT_D], F32, "ExternalInput")
    g1c = dram("g1c", [128, KT_D], F32, "ExternalInput")
    be1c = dram("be1c", [128, KT_D], F32, "ExternalInput")
    g2c = dram("g2c", [128, KT_D], F32, "ExternalInput")
    be2c = dram("be2c", [128, KT_D], F32, "ExternalInput")
    out = dram("out", [B, D, NT], F32, "ExternalOutput")

    AC_A = B * PAIRS * 64
    AC_U = B * PAIRS
    AC = AC_A + AC_U + NC

    ctx = ExitStack()
    pconst = ctx.enter_context(tc.tile_pool(name="pconst", bufs=1))
    pstream = ctx.enter_context(tc.tile_pool(name="pstream", bufs=2))
    pw2s = ctx.enter_context(tc.tile_pool(name="pw2s", bufs=2))
    pxa = ctx.enter_context(tc.tile_pool(name="pxa", bufs=1))
    pmt = ctx.enter_context(tc.tile_pool(name="pmt", bufs=4))
    psm = ctx.enter_context(tc.tile_pool(name="psm", bufs=2))
    peq = ctx.enter_context(tc.tile_pool(name="peq", bufs=1))
    pbig = ctx.enter_context(tc.tile_pool(name="pbig", bufs=1))
    pone = ctx.enter_context(tc.tile_pool(name="pone", bufs=1))
    pdram = ctx.enter_context(tc.tile_pool(name="pdram", bufs=1, space="DRAM"))
    PP = ctx.enter_context(tc.tile_pool(name="PP", bufs=4, space="PSUM"))
    PD = ctx.enter_context(tc.tile_pool(name="PD", bufs=2, space="PSUM"))
    PA_ = ctx.enter_context(tc.tile_pool(name="PA", bufs=1, space="PSUM"))
    PR = ctx.enter_context(tc.tile_pool(name="PR", bufs=1, space="PSUM"))

    # ---- constants ----
    # wv shares the big32 slot with epsv/hsb: wv is only read in stage A,
    # epsv only in kv assembly, hsb only from FFN1 onward — disjoint uses.
    wv_sb = pbig.tile([128, KT_D, D], BF16, tag="big32")
    nc.sync.dma_start(wv_sb[:], wv.rearrange("(kt p) m -> p kt m", p=128))
    cAPs = {}
    for name, ap, shape, dt in (
        ("projbd", projbd, [128, 128], BF16), ("negselF", negselF, [2, 128], BF16),
        ("sel2", sel2, [128, 2], BF16), ("sel2b", sel2b, [2, 128], F32),
        ("ones128", ones128, [128, 1], BF16), ("ones1x128", ones1x128, [1, 128], F32),
        ("mean1", mean1, [128, 1], F32), ("headmask2", headmask2, [128, 2], F32),
        ("onehot", onehot, [1, NC], F32), ("b1c", b1c, [128, MT_FF], F32),
        ("nb1c", nb1c, [128, MT_FF], F32),
        ("b1p1c", b1p1c, [128, MT_FF], F32), ("b2adjc", b2adjc, [128, KT_D], F32),
        ("g1c", g1c, [128, KT_D], F32), ("be1c", be1c, [128, KT_D], F32),
        ("g2c", g2c, [128, KT_D], F32), ("be2c", be2c, [128, KT_D], F32),
    ):
        t = pconst.tile(shape, dt, tag=name)
        nc.sync.dma_start(t[:], ap[:])
        cAPs[name] = t
    sel2b_r = pconst.tile([2, 128], F32R, tag="sel2br")
    ones1x128_r = pconst.tile([1, 128], F32R, tag="ones1x128r")
    mean1_r = pconst.tile([128, 1], F32R, tag="mean1r")
    mean1_bf = pconst.tile([128, 1], BF16, tag="mean1bf")
    sel2b_bf = pconst.tile([2, 128], BF16, tag="sel2bbf")
    nc.vector.tensor_copy(sel2b_r[:], cAPs["sel2b"][:])
    nc.vector.tensor_copy(ones1x128_r[:], cAPs["ones1x128"][:])
    nc.vector.tensor_copy(mean1_r[:], cAPs["mean1"][:])
    nc.vector.tensor_copy(mean1_bf[:], cAPs["mean1"][:])
    nc.vector.tensor_copy(sel2b_bf[:], cAPs["sel2b"][:])

    epsln_c = pconst.tile([1, 1], F32, tag="epslnc")
    nc.vector.memset(epsln_c[:], float(EPS_LN))
    onesrow_c = pconst.tile([1, NT], F32, tag="onesrowc")
    nc.vector.memset(onesrow_c[:], 1.0)
    arstage = pone.tile([128, AC], F32, tag="arbuf")
    mxcols = pone.tile([128, B * PAIRS], BF16, tag="mxcols")

    def ln(res, gc, bc, tag, odt, dma_out=None):
        """Feature-major LN: stats via f32r PE reductions, broadcasts via f32r
        K=1 matmuls. res is a [128, KT_D, NT] f32r tile. Apply is fused to two
        DVE ops per kt (cen, then (cen*g)*S) with the +b on the scalar engine."""
        pm0 = PR.tile([1, NT], F32, tag="prow")
        pm1 = PA_.tile([1, NT], F32, tag="pA")
        for kt in range(KT_D):
            sqt = psm.tile([128, NT], BF16, tag="lnsq")
            nc.scalar.square(sqt[:], res[:, kt, :].bitcast(F32))
            nc.tensor.matmul(pm0[:], mean1_r[:], res[:, kt, :],
                             start=kt == 0, stop=kt == KT_D - 1,
                             skip_group_check=True)
            nc.tensor.matmul(pm1[:], mean1_bf[:], sqt[:],
                             start=kt == 0, stop=kt == KT_D - 1,
                             skip_group_check=True)
        mu = psm.tile([1, NT], F32R, tag="lnmu")
        nc.vector.tensor_copy(mu[:], pm0[:])
        muf = mu[:].bitcast(F32)
        mu2 = psm.tile([1, NT], F32, tag="lnrow")
        nc.vector.tensor_tensor(mu2[:], muf, muf, op=ALU.mult)
        var = psm.tile([1, NT], F32, tag="lnrow")
        nc.vector.tensor_tensor(var[:], pm1[:], mu2[:], op=ALU.subtract)
        lnv = psm.tile([1, NT], F32, tag="lnrow")
        nc.scalar.activation(lnv[:], var[:], AF.Ln, bias=epsln_c[:])
        rstdf = psm.tile([1, NT], F32, tag="lnrow")
        nc.scalar.activation(rstdf[:], lnv[:], AF.Exp, scale=-0.5)
        rstd = psm.tile([1, NT], F32R, tag="lnrstd")
        nc.vector.tensor_copy(rstd[:], rstdf[:])
        pmu = PP.tile([128, NT], F32, tag="pbig")
        nc.tensor.matmul(pmu[:], ones1x128_r[:], mu[:], start=True, stop=True)
        prs = PP.tile([128, NT], F32, tag="pbig")
        nc.tensor.matmul(prs[:], ones1x128_r[:], rstd[:], start=True, stop=True)
        o = None
        if odt is not None:
            o = pbig.tile([128, KT_D, NT], odt, tag="bf8")
        cen = psm.tile([128, NT], F32, tag="lncen")
        nrm = psm.tile([128, NT], BF16 if odt is not None else F32, tag="lnnrm")
        for kt in range(KT_D):
            nc.vector.tensor_tensor(cen[:], res[:, kt, :].bitcast(F32), pmu[:],
                                    op=ALU.subtract)
            nc.vector.scalar_tensor_tensor(nrm[:], cen[:], gc[:, kt:kt + 1],
                                           prs[:], op0=ALU.mult, op1=ALU.mult)
            if o is not None:
                nc.scalar.activation(o[:, kt, :], nrm[:], AF.Identity,
                                     bias=bc[:, kt:kt + 1])
            else:
                ot = psm.tile([128, NT], F32, tag="lnot")
                nc.scalar.activation(ot[:], nrm[:], AF.Identity,
                                     bias=bc[:, kt:kt + 1])
                nc.sync.dma_start(dma_out[0][dma_out[1], kt * 128:(kt + 1) * 128, :],
                                  ot[:])
        return o

    # ================= stage A =================
    for b in range(B):
        xbf = pxa.tile([128, KT_D, NT], BF16, tag="xbf")
        nc.sync.dma_start(xbf[:], x_bf[b].rearrange("(kt p) t -> p kt t", p=128))

        vtok = pxa.tile([128, TT, PAIRS, 129], BF16, tag="vtok")
        nc.vector.memset(vtok[:, :, :, 128:129], 1.0)
        for tt in range(TT):
            for nh in range(2):
                pv = PP.tile([128, 4, 128], F32, tag="pbig")
                for kt in range(KT_D):
                    nc.tensor.matmul(pv[:], xbf[:, kt, tt * 128:(tt + 1) * 128],
                                     wv_sb[:, kt, nh * 512:(nh + 1) * 512],
                                     start=kt == 0, stop=kt == KT_D - 1)
                nc.vector.tensor_copy(vtok[:, tt, nh * 4:(nh + 1) * 4, 0:128],
                                      pv[:])

        # per-pair chain software-pipelined across pairs (depth 2) so the PE
        # never waits on the scalar/vector singletons between sub-phases.
        kTs, ksq2s, Eks = {}, {}, {}

        def s1(pr):  # K projection -> token-major k + k^2 (scalar trails)
            wkmt = pstream.tile([128, KT_D, 128], BF16, tag="wmt")
            nc.sync.dma_start(wkmt[:], wks[pr])
            pk = PP.tile([128, NT], F32, tag="pbig")
            for kt in range(KT_D):
                nc.tensor.matmul(pk[:], wkmt[:, kt, :], xbf[:, kt, :],
                                 start=kt == 0, stop=kt == KT_D - 1)
            kTmt = pmt.tile([128, NT], BF16, tag="mt512")
            nc.scalar.activation(kTmt[:], pk[:], AF.Copy)
            ksqmt = pmt.tile([128, NT], BF16, tag="mt512")
            nc.scalar.square(ksqmt[:], pk[:])
            kTs[pr] = (kTmt, ksqmt)

        def s2(pr):  # squared-norm row + FAVOR features
            kTmt, ksqmt = kTs[pr]
            pks = PR.tile([2, NT], F32, tag="prow")
            nc.tensor.matmul(pks[:], cAPs["sel2"][:], ksqmt[:], start=True,
                             stop=True)
            ksq2 = psm.tile([2, NT], BF16, tag="ksq2")
            nc.scalar.activation(ksq2[:], pks[:], AF.Copy)
            ksq2s[pr] = ksq2

            # One accumulation group for the whole bank: only the FIRST matmul
            # carries start=True (whole-bank has_written clear); later slices
            # overwrite-and-set, negsels then accumulate onto set bits. The
            # raw-projection max is one whole-bank reduce between the phases.
            Ek = psm.tile([128, TT, 128], BF16, tag="Ek")
            pdd = PD.tile([128, NT], F32, tag="pdd")
            sls = [slice(tt * 128, (tt + 1) * 128) for tt in range(TT)]
            for tt in range(TT):
                nc.tensor.matmul(pdd[:, sls[tt]], kTmt[:, sls[tt]],
                                 cAPs["projbd"][:], start=tt == 0, stop=False,
                                 skip_group_check=True)
            c = b * PAIRS + pr
            nc.vector.tensor_reduce(mxcols[:, c:c + 1], pdd[:], axis=AX.X,
                                    op=ALU.max)
            for tt in range(TT):
                nc.tensor.matmul(pdd[:, sls[tt]], ksq2[:, sls[tt]],
                                 cAPs["negselF"][:], start=False,
                                 stop=tt == TT - 1, skip_group_check=True)
            nc.scalar.activation(Ek[:], pdd[:], AF.Exp)
            Eks[pr] = Ek

        def s3(pr):  # token-contraction A matmuls + AR staging
            Ek = Eks[pr]
            pA = PA_.tile([128, 129], F32, tag="pA")
            for tt in range(TT):
                nc.tensor.matmul(pA[:], Ek[:, tt, :], vtok[:, tt, pr, :],
                                 start=tt == 0, stop=tt == TT - 1,
                                 skip_group_check=True)
            j = b * PAIRS + pr
            nc.vector.tensor_copy(arstage[0:64, j * 64:(j + 1) * 64], pA[0:64, 0:64])
            nc.vector.tensor_copy(arstage[64:128, j * 64:(j + 1) * 64],
                                  pA[64:128, 64:128])
            nc.vector.tensor_copy(arstage[:, AC_A + j:AC_A + j + 1], pA[:, 128:129])

        for step in range(PAIRS + 2):
            if step < PAIRS:
                s1(step)
            if 1 <= step <= PAIRS:
                s2(step - 1)
            if step >= 2:
                s3(step - 2)

    # ---- fire AllReduce ----
    mxr = pone.tile([128, 1], F32, tag="mxr")
    nc.vector.tensor_reduce(mxr[:], mxcols[:], axis=AX.X, op=ALU.max)
    mx1 = pone.tile([1, 1], F32, tag="mx1")
    nc.gpsimd.tensor_reduce(mx1[:], mxr[:], axis=AX.C, op=ALU.max)
    nc.vector.tensor_scalar(arstage[0:1, AC_A + AC_U:AC], cAPs["onehot"][:],
                            mx1[:], None, op0=ALU.mult)
    arin = pdram.tile([128, AC], F32, tag="arin")
    arout = pdram.tile([128, AC], F32, tag="arout")
    nc.sync.dma_start(arin[:], arstage[:])
    if os.environ.get("KERNEL_NOCOLL"):
        nc.sync.dma_start(arout[:], arin[:])
    else:
        nc.gpsimd.collective_compute("AllReduce", ALU.add,
                                     replica_groups=[list(range(NC))],
                                     ins=[arin[:]], outs=[arout[:]])
    arres = pone.tile([128, AC], F32, tag="arbuf")
    nc.sync.dma_start(arres[:], arout[:])

    # ================= q-side features (overlap AR) =================
    Eq_all = {}

    def emit_qside(b):
        qxbf = pxa.tile([128, KT_D, NT], BF16, tag="qxbf")
        nc.sync.dma_start(qxbf[:], x_bf[b].rearrange("(kt p) t -> p kt t", p=128))
        Eqs = []
        for pr in range(PAIRS):
            wqmt = pstream.tile([128, KT_D, 128], BF16, tag="wmt")
            nc.sync.dma_start(wqmt[:], wqs[pr])
            pq_ = PP.tile([128, NT], F32, tag="pbig")
            for kt in range(KT_D):
                nc.tensor.matmul(pq_[:], wqmt[:, kt, :], qxbf[:, kt, :],
                                 start=kt == 0, stop=kt == KT_D - 1)
            qTmt = pmt.tile([128, NT], BF16, tag="mt512")
            nc.scalar.activation(qTmt[:], pq_[:], AF.Copy)
            qsqmt = pmt.tile([128, NT], BF16, tag="mt512")
            nc.scalar.square(qsqmt[:], pq_[:])
            pqs = PR.tile([2, NT], F32, tag="prow")
            nc.tensor.matmul(pqs[:], cAPs["sel2"][:], qsqmt[:], start=True, stop=True)
            qsq2 = psm.tile([2, NT], BF16, tag="qsq2")
            nc.scalar.activation(qsq2[:], pqs[:], AF.Copy)

            pdq = PP.tile([128, NT], F32, tag="pbig")
            nc.tensor.matmul(pdq[:], cAPs["projbd"][:], qTmt[:], start=True,
                             stop=False)
            nc.tensor.matmul(pdq[:], cAPs["negselF"][:], qsq2[:], start=False,
                             stop=True, skip_group_check=True)
            Etmp = pmt.tile([128, NT], BF16, tag="t512bf")
            nc.scalar.activation(Etmp[:], pdq[:], AF.Exp)

            pS = PR.tile([2, NT], F32, tag="prow")
            nc.tensor.matmul(pS[:], cAPs["sel2"][:], Etmp[:], start=True, stop=True)
            ediag = psm.tile([2, NT], BF16, tag="ediag")
            nc.scalar.activation(ediag[:], qsq2[:], AF.Exp, scale=float(DN2H))
            wrow = psm.tile([2, NT], BF16, tag="wrow")
            nc.vector.scalar_tensor_tensor(wrow[:], ediag[:], EPS_KERN, pS[:],
                                           op0=ALU.mult, op1=ALU.mult)
            pwB = PP.tile([128, NT], F32, tag="pbig")
            nc.tensor.matmul(pwB[:], sel2b_bf[:], wrow[:], start=True, stop=True)
            Eq = peq.tile([128, NT], BF16, tag=f"Eq{b % 2}_{pr}")
            nc.vector.tensor_tensor(Eq[:], Etmp[:], pwB[:], op=ALU.add)
            Eqs.append(Eq)
        Eq_all[b] = Eqs

    emit_qside(0)
    emit_qside(1)

    # ---- kv / ksum assembly ----
    mx8 = pone.tile([1, 1], F32, tag="mx8")
    nc.vector.tensor_reduce(mx8[:], arres[0:1, AC_A + AC_U:AC], axis=AX.X, op=ALU.max)
    emxf = pone.tile([1, 1], F32, tag="emxf")
    nc.scalar.activation(emxf[:], mx8[:], AF.Exp, scale=-1.0)
    emxrow = psm.tile([1, NT], F32R, tag="lnrow")
    nc.vector.tensor_scalar(emxrow[:], onesrow_c[:], emxf[:], None, op0=ALU.mult)
    pex = PP.tile([128, NT], F32, tag="pbig")
    nc.tensor.matmul(pex[:], ones1x128_r[:], emxrow[:], start=True, stop=True)
    emxc = pone.tile([128, 1], F32, tag="emxc")
    nc.vector.tensor_copy(emxc[:], pex[:, 0:1])

    epsv_sb = pbig.tile([128, B * D], BF16, tag="big32")
    nc.sync.dma_start(epsv_sb[:], epsvB[:])

    kvBall = pone.tile([128, B * PAIRS, 130], BF16, tag="kvBall")
    nc.vector.memset(kvBall[:], 0.0)
    kvB_all = {}
    for b in range(B):
        for pr in range(PAIRS):
            j = b * PAIRS + pr
            kvB = kvBall[:, j, :]
            nc.vector.scalar_tensor_tensor(
                kvB[0:64, 0:64], arres[0:64, j * 64:(j + 1) * 64], emxc[0:64, :],
                epsv_sb[0:64, b * D + pr * 128:b * D + pr * 128 + 64],
                op0=ALU.mult, op1=ALU.add)
            nc.vector.scalar_tensor_tensor(
                kvB[64:128, 64:128], arres[64:128, j * 64:(j + 1) * 64],
                emxc[64:128, :],
                epsv_sb[64:128, b * D + pr * 128 + 64:b * D + pr * 128 + 128],
                op0=ALU.mult, op1=ALU.add)
            ksf = psm.tile([128, 1], F32, tag="ksf")
            nc.vector.tensor_scalar(ksf[:], arres[:, AC_A + j:AC_A + j + 1],
                                    emxc[:], float(EPS_KERN * N),
                                    op0=ALU.mult, op1=ALU.add)
            nc.vector.tensor_scalar(kvB[:, 128:130], cAPs["headmask2"][:], ksf[:],
                                    None, op0=ALU.mult)
            kvB_all[j] = kvB

    # ================= stage B =================
    for b in range(B):
        Eqs = Eq_all[b]
        attnT = pbig.tile([128, KT_D, NT], BF16, tag="attnT")
        # pass 1: dense PE stream of num/den matmuls; scalar+vector trail
        numsbs, rdens = [], []
        for pr in range(PAIRS):
            kvB = kvB_all[b * PAIRS + pr]
            pnum = PP.tile([128, NT], F32, tag="pbig")
            nc.tensor.matmul(pnum[:], kvB[:, 0:128], Eqs[pr][:], start=True,
                             stop=True)
            pden = PD.tile([2, NT], F32, tag="pdd")
            nc.tensor.matmul(pden[:], kvB[:, 128:130], Eqs[pr][:], start=True,
                             stop=True)
            nc.scalar.activation(attnT[:, pr, :], pnum[:], AF.Copy)
            rdf = psm.tile([2, NT], F32, tag="rdf")
            nc.vector.reciprocal_approx_fast(rdf[:], pden[:])
            rden = psm.tile([2, NT], F32R, tag=f"rden{pr}", bufs=1)
            nc.vector.tensor_copy(rden[:], rdf[:])
            rdens.append(rden)
        # pass 2: broadcast 1/den and scale numerators in place
        for pr in range(PAIRS):
            prdB = PP.tile([128, NT], F32, tag="pbig")
            nc.tensor.matmul(prdB[:], sel2b_r[:], rdens[pr][:], start=True,
                             stop=True)
            nc.vector.tensor_tensor(attnT[:, pr, :], attnT[:, pr, :], prdB[:],
                                    op=ALU.mult)

        res1 = pbig.tile([128, KT_D, NT], F32R, tag="resX")
        for mt in range(KT_D):
            womt = pstream.tile([128, KT_D, 128], BF16, tag="wmt")
            nc.sync.dma_start(womt[:], wos[mt])
            po = PP.tile([128, NT], F32, tag="pbig")
            for kt in range(KT_D):
                nc.tensor.matmul(po[:], womt[:, kt, :], attnT[:, kt, :],
                                 start=kt == 0, stop=kt == KT_D - 1)
            xf = psm.tile([128, NT], F32, tag="xf")
            nc.sync.dma_start(xf[:], x_f32[b, mt * 128:(mt + 1) * 128, :])
            nc.vector.tensor_tensor(res1[:, mt, :], xf[:], po[:], op=ALU.add)

        out1 = ln(res1, cAPs["g1c"], cAPs["be1c"], "o1", mybir.dt.bfloat16)

        hsb = pbig.tile([128, MT_FF, NT], BF16, tag="big32")
        for mt in range(MT_FF):
            w1mt = pstream.tile([128, KT_D, 128], BF16, tag="wmt")
            nc.sync.dma_start(w1mt[:], w1s[mt])
            # 6-deep pz rotation (4 PP banks + the 2 PD banks, idle here) so
            # the matmul stream never waits on the scalar/vector ELU drain.
            if mt % 3 != 2:
                pz = PP.tile([128, NT], F32, tag="pbig", name="pz")
            else:
                pz = PD.tile([128, NT], F32, tag="pdd", name="pz")
            for kt in range(KT_D):
                nc.tensor.matmul(pz[:], w1mt[:, kt, :], out1[:, kt, :],
                                 start=kt == 0, stop=kt == KT_D - 1)
            # elu(z')+1 = max(z'+1, exp(-relu(-z'))) — min(exp,1) via Relu+Exp
            # keeps everything on the scalar engine (Relu is in every table
            # set, so no ACT_TABLE_LOAD thrash) and off GpSimd entirely.
            nrelu = pmt.tile([128, NT], BF16, tag="t512bf")
            nc.scalar.activation(nrelu[:], pz[:], AF.Relu, scale=-1.0,
                                 bias=cAPs["nb1c"][:, mt:mt + 1])
            emin = pmt.tile([128, NT], BF16, tag="t512bf")
            nc.scalar.activation(emin[:], nrelu[:], AF.Exp, scale=-1.0)
            nc.vector.scalar_tensor_tensor(hsb[:, mt, :], pz[:],
                                           cAPs["b1p1c"][:, mt:mt + 1], emin[:],
                                           op0=ALU.add, op1=ALU.max)

        res2 = pbig.tile([128, KT_D, NT], F32R, tag="resX")
        for mt in range(KT_D):
            w2a = pw2s.tile([128, MT_FF // 2, 128], BF16, tag="w2mt")
            nc.sync.dma_start(w2a[:], w2s[mt, :, 0:MT_FF // 2])
            w2b = pw2s.tile([128, MT_FF // 2, 128], BF16, tag="w2mt")
            nc.sync.dma_start(w2b[:], w2s[mt, :, MT_FF // 2:MT_FF])
            pf = PP.tile([128, NT], F32, tag="pbig")
            for kt in range(MT_FF):
                w2h = w2a if kt < MT_FF // 2 else w2b
                nc.tensor.matmul(pf[:], w2h[:, kt % (MT_FF // 2), :],
                                 hsb[:, kt, :],
                                 start=kt == 0, stop=kt == MT_FF - 1)
            nc.vector.scalar_tensor_tensor(res2[:, mt, :], pf[:],
                                           cAPs["b2adjc"][:, mt:mt + 1],
                                           out1[:, mt, :], op0=ALU.add, op1=ALU.add)

        ln(res2, cAPs["g2c"], cAPs["be2c"], "o2", None, dma_out=(out, b))

        if b + 2 < B:
            emit_qside(b + 2)

    ctx.close()


_CACHE = {}


def _build():
    import concourse.tile as tile
    from concourse import bacc
    nc = bacc.Bacc("TRN2", target_bir_lowering=False, debug=False, num_devices=NC)
    with tile.TileContext(nc) as tc:
        _emit(nc, tc)
    nc.compile()
    return nc


def _host_inputs(x, Wq, Wk, Wv, Wo, proj, W1, b1, W2, b2,
                 ln1_g, ln1_b, ln2_g, ln2_b):
    bf = ml_dtypes.bfloat16
    f32 = np.float32
    d = {}

    def chunked(w):  # [D, X] -> [X/128 mt, 128 p, D/128 kt, 128]
        Dk, X = w.shape
        r = w.reshape(Dk // 128, 128, X // 128, 128)
        return np.ascontiguousarray(r.transpose(2, 1, 0, 3)).astype(bf)

    d["wqs"] = chunked(Wq.reshape(D, D))
    d["wks"] = chunked(Wk.reshape(D, D))
    d["wv"] = np.ascontiguousarray(Wv.reshape(D, D)).astype(bf)
    d["wos"] = chunked(Wo.reshape(D, D))
    d["w1s"] = chunked(W1)
    d["w2s"] = chunked(W2)

    projT_s = (proj * DN).T.astype(f32)
    pbd = np.zeros((128, 128), f32)
    pbd[0:64, 0:64] = projT_s
    pbd[64:128, 64:128] = projT_s
    d["projbd"] = pbd.astype(bf)
    nsF = np.zeros((2, 128), f32)
    nsF[0, 0:64] = -DN2H
    nsF[1, 64:128] = -DN2H
    d["negselF"] = nsF.astype(bf)
    s2 = np.zeros((128, 2), f32)
    s2[0:64, 0] = 1.0
    s2[64:128, 1] = 1.0
    d["sel2"] = s2.astype(bf)
    s2b = np.zeros((2, 128), f32)
    s2b[0, 0:64] = 1.0
    s2b[1, 64:128] = 1.0
    d["sel2b"] = s2b
    d["ones128"] = np.ones((128, 1), f32).astype(bf)
    d["ones1x128"] = np.ones((1, 128), f32)
    d["mean1"] = np.full((128, 1), 1.0 / D, f32)
    hm2 = np.zeros((128, 2), f32)
    hm2[0:64, 0] = 1.0
    hm2[64:128, 1] = 1.0
    d["headmask2"] = hm2

    xsum = x.sum(axis=1, dtype=np.float64)
    vsum = xsum @ Wv.reshape(D, D).astype(np.float64)
    epsv = (EPS_KERN * vsum).astype(f32)
    d["epsvB"] = np.ascontiguousarray(
        np.broadcast_to(epsv.reshape(1, B * D), (128, B * D))).astype(bf)

    d["b1c"] = np.ascontiguousarray(b1.reshape(MT_FF, 128).T).astype(f32)
    d["nb1c"] = np.ascontiguousarray((-b1).reshape(MT_FF, 128).T).astype(f32)
    d["b1p1c"] = np.ascontiguousarray((b1 + 1.0).reshape(MT_FF, 128).T).astype(f32)
    b2adj = b2.astype(np.float64) - W2.astype(np.float64).sum(axis=0)
    d["b2adjc"] = np.ascontiguousarray(b2adj.reshape(KT_D, 128).T).astype(f32)
    d["g1c"] = np.ascontiguousarray(ln1_g.reshape(KT_D, 128).T).astype(f32)
    d["be1c"] = np.ascontiguousarray(ln1_b.reshape(KT_D, 128).T).astype(f32)
    d["g2c"] = np.ascontiguousarray(ln2_g.reshape(KT_D, 128).T).astype(f32)
    d["be2c"] = np.ascontiguousarray(ln2_b.reshape(KT_D, 128).T).astype(f32)
    return d


def kernel(x, Wq, Wk, Wv, Wo, proj, W1, b1, W2, b2, ln1_g, ln1_b, ln2_g, ln2_b):
    from concourse import bass_utils

    x = np.asarray(x, np.float32)
    shared = _host_inputs(x, np.asarray(Wq), np.asarray(Wk), np.asarray(Wv),
                          np.asarray(Wo), np.asarray(proj), np.asarray(W1),
                          np.asarray(b1), np.asarray(W2), np.asarray(b2),
                          np.asarray(ln1_g), np.asarray(ln1_b),
                          np.asarray(ln2_g), np.asarray(ln2_b))

    if "nc" not in _CACHE:
        _CACHE["nc"] = _build()
    nc = _CACHE["nc"]

    in_maps = []
    for c in range(NC):
        xs = x[:, c * NT:(c + 1) * NT, :]
        xT = np.ascontiguousarray(xs.transpose(0, 2, 1))
        oh = np.zeros((1, NC), np.float32)
        oh[0, c] = 1.0
        m = dict(shared)
        m["x_f32"] = xT
        m["x_bf"] = xT.astype(ml_dtypes.bfloat16)
        m["onehot"] = oh
        in_maps.append(m)

    trace = bool(int(os.environ.get("KERNEL_TRACE", "0")))
    res = bass_utils.run_bass_kernel_spmd(nc, in_maps, core_ids=list(range(NC)),
                                          trace=trace)
    if trace and res.exec_time_ns is not None:
        print(f"HW exec time: {res.exec_time_ns} ns")
        if res.instructions_and_trace is not None:
            print("trace:", res.instructions_and_trace[1])

    outp = np.empty((B, N, D), np.float32)
    for c in range(NC):
        oT = res.results[c]["out"]
        outp[:, c * NT:(c + 1) * NT, :] = oT.transpose(0, 2, 1)
    return outp



# revision 36
# speedup vs baseline: 1.7307x; 1.0200x over previous
"""Performer (FAVOR+) encoder layer on 8 trn2 NeuronCores.

Sharding: data-parallel over sequence (512 positions per core x 4 batches).
The linear-attention summaries (A = E_k^T v per (batch, head), usum) and the
global key-feature max (via one-hot slots) are combined in ONE packed
AllReduce, overlapped with the Q-side feature compute.

Layout: activations feature-major (xT = [D, tokens]) so every GEMM's
stationary operand is a natural weight slice; per-token reductions and
broadcasts are small PE matmuls (selector / ones / f32r broadcast matmuls).
E_k and v are produced token-major directly by matmuls so the token-
contraction A-matmul needs no transposes.
"""
import os
import numpy as np
import ml_dtypes

B, N, D = 4, 4096, 1024
H, DH = 16, 64
DFF = 4096
M = 64
EPS_KERN = 1e-6
EPS_LN = 1e-6
NC = 8
NT = N // NC                # 512 positions per core per batch
PAIRS = H // 2              # 8 head-pairs
KT_D = D // 128             # 8
MT_FF = DFF // 128          # 32
TT = NT // 128              # 4
DN = 1.0 / np.sqrt(np.sqrt(DH))
DN2H = DN * DN / 2.0


def _emit(nc, tc):
    import concourse.mybir as mybir
    from contextlib import ExitStack
    F32 = mybir.dt.float32
    F32R = mybir.dt.float32r
    BF16 = mybir.dt.bfloat16
    AF = mybir.ActivationFunctionType
    ALU = mybir.AluOpType
    AX = mybir.AxisListType

    dram = lambda name, shape, dt, kind: nc.dram_tensor(name, shape, dt, kind=kind).ap()

    x_bf = dram("x_bf", [B, D, NT], BF16, "ExternalInput")
    x_f32 = dram("x_f32", [B, D, NT], F32, "ExternalInput")
    wqs = dram("wqs", [KT_D, 128, KT_D, 128], BF16, "ExternalInput")
    wks = dram("wks", [KT_D, 128, KT_D, 128], BF16, "ExternalInput")
    wv = dram("wv", [D, D], BF16, "ExternalInput")
    wos = dram("wos", [KT_D, 128, KT_D, 128], BF16, "ExternalInput")
    w1s = dram("w1s", [MT_FF, 128, KT_D, 128], BF16, "ExternalInput")
    w2s = dram("w2s", [KT_D, 128, MT_FF, 128], BF16, "ExternalInput")
    projbd = dram("projbd", [128, 128], BF16, "ExternalInput")
    negselF = dram("negselF", [2, 128], BF16, "ExternalInput")
    sel2 = dram("sel2", [128, 2], BF16, "ExternalInput")
    sel2b = dram("sel2b", [2, 128], F32, "ExternalInput")
    ones128 = dram("ones128", [128, 1], BF16, "ExternalInput")
    ones1x128 = dram("ones1x128", [1, 128], F32, "ExternalInput")
    mean1 = dram("mean1", [128, 1], F32, "ExternalInput")
    headmask2 = dram("headmask2", [128, 2], F32, "ExternalInput")
    epsvB = dram("epsvB", [128, B * D], BF16, "ExternalInput")
    onehot = dram("onehot", [1, NC], F32, "ExternalInput")
    b1c = dram("b1c", [128, MT_FF], F32, "ExternalInput")
    nb1c = dram("nb1c", [128, MT_FF], F32, "ExternalInput")
    b1p1c = dram("b1p1c", [128, MT_FF], F32, "ExternalInput")
    b2adjc = dram("b2adjc", [128, KT_D], F32, "ExternalInput")
    g1c = dram("g1c", [128, KT_D], F32, "ExternalInput")
    be1c = dram("be1c", [128, KT_D], F32, "ExternalInput")
    g2c = dram("g2c", [128, KT_D], F32, "ExternalInput")
    be2c = dram("be2c", [128, KT_D], F32, "ExternalInput")
    out = dram("out", [B, D, NT], F32, "ExternalOutput")

    AC_A = B * PAIRS * 64
    AC_U = B * PAIRS
    AC = AC_A + AC_U + NC

    ctx = ExitStack()
    pconst = ctx.enter_context(tc.tile_pool(name="pconst", bufs=1))
    pstream = ctx.enter_context(tc.tile_pool(name="pstream", bufs=2))
    pw2s = ctx.enter_context(tc.tile_pool(name="pw2s", bufs=2))
    pxa = ctx.enter_context(tc.tile_pool(name="pxa", bufs=1))
    pmt = ctx.enter_context(tc.tile_pool(name="pmt", bufs=4))
    psm = ctx.enter_context(tc.tile_pool(name="psm", bufs=2))
    peq = ctx.enter_context(tc.tile_pool(name="peq", bufs=1))
    pbig = ctx.enter_context(tc.tile_pool(name="pbig", bufs=1))
    pone = ctx.enter_context(tc.tile_pool(name="pone", bufs=1))
    pdram = ctx.enter_context(tc.tile_pool(name="pdram", bufs=1, space="DRAM"))
    PP = ctx.enter_context(tc.tile_pool(name="PP", bufs=4, space="PSUM"))
    PD = ctx.enter_context(tc.tile_pool(name="PD", bufs=2, space="PSUM"))
    PA_ = ctx.enter_context(tc.tile_pool(name="PA", bufs=1, space="PSUM"))
    PR = ctx.enter_context(tc.tile_pool(name="PR", bufs=1, space="PSUM"))

    # ---- constants ----
    # wv shares the big32 slot with epsv/hsb: wv is only read in stage A,
    # epsv only in kv assembly, hsb only from FFN1 onward — disjoint uses.
    wv_sb = pbig.tile([128, KT_D, D], BF16, tag="big32")
    nc.sync.dma_start(wv_sb[:], wv.rearrange("(kt p) m -> p kt m", p=128))
    cAPs = {}
    for name, ap, shape, dt in (
        ("projbd", projbd, [128, 128], BF16), ("negselF", negselF, [2, 128], BF16),
        ("sel2", sel2, [128, 2], BF16), ("sel2b", sel2b, [2, 128], F32),
        ("ones128", ones128, [128, 1], BF16), ("ones1x128", ones1x128, [1, 128], F32),
        ("mean1", mean1, [128, 1], F32), ("headmask2", headmask2, [128, 2], F32),
        ("onehot", onehot, [1, NC], F32), ("b1c", b1c, [128, MT_FF], F32),
        ("nb1c", nb1c, [128, MT_FF], F32),
        ("b1p1c", b1p1c, [128, MT_FF], F32), ("b2adjc", b2adjc, [128, KT_D], F32),
        ("g1c", g1c, [128, KT_D], F32), ("be1c", be1c, [128, KT_D], F32),
        ("g2c", g2c, [128, KT_D], F32), ("be2c", be2c, [128, KT_D], F32),
    ):
        t = pconst.tile(shape, dt, tag=name)
        nc.sync.dma_start(t[:], ap[:])
        cAPs[name] = t
    sel2b_r = pconst.tile([2, 128], F32R, tag="sel2br")
    ones1x128_r = pconst.tile([1, 128], F32R, tag="ones1x128r")
    mean1_r = pconst.tile([128, 1], F32R, tag="mean1r")
    mean1_bf = pconst.tile([128, 1], BF16, tag="mean1bf")
    sel2b_bf = pconst.tile([2, 128], BF16, tag="sel2bbf")
    nc.vector.tensor_copy(sel2b_r[:], cAPs["sel2b"][:])
    nc.vector.tensor_copy(ones1x128_r[:], cAPs["ones1x128"][:])
    nc.vector.tensor_copy(mean1_r[:], cAPs["mean1"][:])
    nc.vector.tensor_copy(mean1_bf[:], cAPs["mean1"][:])
    nc.vector.tensor_copy(sel2b_bf[:], cAPs["sel2b"][:])

    epsln_c = pconst.tile([1, 1], F32, tag="epslnc")
    nc.vector.memset(epsln_c[:], float(EPS_LN))
    onesrow_c = pconst.tile([1, NT], F32, tag="onesrowc")
    nc.vector.memset(onesrow_c[:], 1.0)
    arstage = pone.tile([128, AC], F32, tag="arbuf")
    mxcols = pone.tile([128, B * PAIRS], BF16, tag="mxcols")

    def ln(res, gc, bc, tag, odt, dma_out=None):
        """Feature-major LN: stats via f32r PE reductions, broadcasts via f32r
        K=1 matmuls. res is a [128, KT_D, NT] f32r tile. Apply is fused to two
        DVE ops per kt (cen, then (cen*g)*S) with the +b on the scalar engine."""
        pm0 = PR.tile([1, NT], F32, tag="prow")
        pm1 = PA_.tile([1, NT], F32, tag="pA")
        for kt in range(KT_D):
            sqt = psm.tile([128, NT], BF16, tag="lnsq")
            nc.scalar.square(sqt[:], res[:, kt, :].bitcast(F32))
            nc.tensor.matmul(pm0[:], mean1_r[:], res[:, kt, :],
                             start=kt == 0, stop=kt == KT_D - 1,
                             skip_group_check=True)
            nc.tensor.matmul(pm1[:], mean1_bf[:], sqt[:],
                             start=kt == 0, stop=kt == KT_D - 1,
                             skip_group_check=True)
        mu = psm.tile([1, NT], F32R, tag="lnmu")
        nc.vector.tensor_copy(mu[:], pm0[:])
        muf = mu[:].bitcast(F32)
        mu2 = psm.tile([1, NT], F32, tag="lnrow")
        nc.vector.tensor_tensor(mu2[:], muf, muf, op=ALU.mult)
        var = psm.tile([1, NT], F32, tag="lnrow")
        nc.vector.tensor_tensor(var[:], pm1[:], mu2[:], op=ALU.subtract)
        lnv = psm.tile([1, NT], F32, tag="lnrow")
        nc.scalar.activation(lnv[:], var[:], AF.Ln, bias=epsln_c[:])
        rstdf = psm.tile([1, NT], F32, tag="lnrow")
        nc.scalar.activation(rstdf[:], lnv[:], AF.Exp, scale=-0.5)
        rstd = psm.tile([1, NT], F32R, tag="lnrstd")
        nc.vector.tensor_copy(rstd[:], rstdf[:])
        pmu = PP.tile([128, NT], F32, tag="pbig")
        nc.tensor.matmul(pmu[:], ones1x128_r[:], mu[:], start=True, stop=True)
        prs = PP.tile([128, NT], F32, tag="pbig")
        nc.tensor.matmul(prs[:], ones1x128_r[:], rstd[:], start=True, stop=True)
        o = None
        if odt is not None:
            o = pbig.tile([128, KT_D, NT], odt, tag="bf8")
        cen = psm.tile([128, NT], F32, tag="lncen")
        nrm = psm.tile([128, NT], BF16 if odt is not None else F32, tag="lnnrm")
        for kt in range(KT_D):
            nc.vector.tensor_tensor(cen[:], res[:, kt, :].bitcast(F32), pmu[:],
                                    op=ALU.subtract)
            nc.vector.scalar_tensor_tensor(nrm[:], cen[:], gc[:, kt:kt + 1],
                                           prs[:], op0=ALU.mult, op1=ALU.mult)
            if o is not None:
                nc.scalar.activation(o[:, kt, :], nrm[:], AF.Identity,
                                     bias=bc[:, kt:kt + 1])
            else:
                ot = psm.tile([128, NT], F32, tag="lnot")
                nc.scalar.activation(ot[:], nrm[:], AF.Identity,
                                     bias=bc[:, kt:kt + 1])
                nc.sync.dma_start(dma_out[0][dma_out[1], kt * 128:(kt + 1) * 128, :],
                                  ot[:])
        return o

    # ================= stage A =================
    for b in range(B):
        xbf = pxa.tile([128, KT_D, NT], BF16, tag="xbf")
        nc.sync.dma_start(xbf[:], x_bf[b].rearrange("(kt p) t -> p kt t", p=128))

        vtok = pxa.tile([128, TT, PAIRS, 129], BF16, tag="vtok")
        nc.vector.memset(vtok[:, :, :, 128:129], 1.0)
        for tt in range(TT):
            for nh in range(2):
                pv = PP.tile([128, 4, 128], F32, tag="pbig")
                for kt in range(KT_D):
                    nc.tensor.matmul(pv[:], xbf[:, kt, tt * 128:(tt + 1) * 128],
                                     wv_sb[:, kt, nh * 512:(nh + 1) * 512],
                                     start=kt == 0, stop=kt == KT_D - 1)
                nc.vector.tensor_copy(vtok[:, tt, nh * 4:(nh + 1) * 4, 0:128],
                                      pv[:])

        # per-pair chain software-pipelined across pairs (depth 2) so the PE
        # never waits on the scalar/vector singletons between sub-phases.
        kTs, ksq2s, Eks = {}, {}, {}

        def s1(pr):  # K projection -> token-major k + k^2 (scalar trails)
            wkmt = pstream.tile([128, KT_D, 128], BF16, tag="wmt")
            nc.sync.dma_start(wkmt[:], wks[pr])
            pk = PP.tile([128, NT], F32, tag="pbig")
            for kt in range(KT_D):
                nc.tensor.matmul(pk[:], wkmt[:, kt, :], xbf[:, kt, :],
                                 start=kt == 0, stop=kt == KT_D - 1)
            kTmt = pmt.tile([128, NT], BF16, tag="mt512")
            nc.scalar.activation(kTmt[:], pk[:], AF.Copy)
            ksqmt = pmt.tile([128, NT], BF16, tag="mt512")
            nc.scalar.square(ksqmt[:], pk[:])
            kTs[pr] = (kTmt, ksqmt)

        def s2(pr):  # squared-norm row + FAVOR features
            kTmt, ksqmt = kTs[pr]
            pks = PR.tile([2, NT], F32, tag="prow")
            nc.tensor.matmul(pks[:], cAPs["sel2"][:], ksqmt[:], start=True,
                             stop=True)
            ksq2 = psm.tile([2, NT], BF16, tag="ksq2")
            nc.scalar.activation(ksq2[:], pks[:], AF.Copy)
            ksq2s[pr] = ksq2

            # One accumulation group for the whole bank: only the FIRST matmul
            # carries start=True (whole-bank has_written clear); later slices
            # overwrite-and-set, negsels then accumulate onto set bits. The
            # raw-projection max is one whole-bank reduce between the phases.
            Ek = psm.tile([128, TT, 128], BF16, tag="Ek")
            pdd = PD.tile([128, NT], F32, tag="pdd")
            sls = [slice(tt * 128, (tt + 1) * 128) for tt in range(TT)]
            for tt in range(TT):
                nc.tensor.matmul(pdd[:, sls[tt]], kTmt[:, sls[tt]],
                                 cAPs["projbd"][:], start=tt == 0, stop=False,
                                 skip_group_check=True)
            c = b * PAIRS + pr
            nc.vector.tensor_reduce(mxcols[:, c:c + 1], pdd[:], axis=AX.X,
                                    op=ALU.max)
            for tt in range(TT):
                nc.tensor.matmul(pdd[:, sls[tt]], ksq2[:, sls[tt]],
                                 cAPs["negselF"][:], start=False,
                                 stop=tt == TT - 1, skip_group_check=True)
            nc.scalar.activation(Ek[:], pdd[:], AF.Exp)
            Eks[pr] = Ek

        def s3(pr):  # token-contraction A matmuls + AR staging
            Ek = Eks[pr]
            pA = PA_.tile([128, 129], F32, tag="pA")
            for tt in range(TT):
                nc.tensor.matmul(pA[:], Ek[:, tt, :], vtok[:, tt, pr, :],
                                 start=tt == 0, stop=tt == TT - 1,
                                 skip_group_check=True)
            j = b * PAIRS + pr
            nc.vector.tensor_copy(arstage[0:64, j * 64:(j + 1) * 64], pA[0:64, 0:64])
            nc.vector.tensor_copy(arstage[64:128, j * 64:(j + 1) * 64],
                                  pA[64:128, 64:128])
            nc.vector.tensor_copy(arstage[:, AC_A + j:AC_A + j + 1], pA[:, 128:129])

        for step in range(PAIRS + 2):
            if step < PAIRS:
                s1(step)
            if 1 <= step <= PAIRS:
                s2(step - 1)
            if step >= 2:
                s3(step - 2)

    # ---- fire AllReduce ----
    mxr = pone.tile([128, 1], F32, tag="mxr")
    nc.vector.tensor_reduce(mxr[:], mxcols[:], axis=AX.X, op=ALU.max)
    mx1 = pone.tile([1, 1], F32, tag="mx1")
    nc.gpsimd.tensor_reduce(mx1[:], mxr[:], axis=AX.C, op=ALU.max)
    nc.vector.tensor_scalar(arstage[0:1, AC_A + AC_U:AC], cAPs["onehot"][:],
                            mx1[:], None, op0=ALU.mult)
    arin = pdram.tile([128, AC], F32, tag="arin")
    arout = pdram.tile([128, AC], F32, tag="arout")
    nc.sync.dma_start(arin[:], arstage[:])
    if os.environ.get("KERNEL_NOCOLL"):
        nc.sync.dma_start(arout[:], arin[:])
    else:
        nc.gpsimd.collective_compute("AllReduce", ALU.add,
                                     replica_groups=[list(range(NC))],
                                     ins=[arin[:]], outs=[arout[:]])
    arres = pone.tile([128, AC], F32, tag="arbuf")
    nc.sync.dma_start(arres[:], arout[:])

    # ================= q-side features (overlap AR) =================
    Eq_all = {}

    def emit_qside(b):
        qxbf = pxa.tile([128, KT_D, NT], BF16, tag="qxbf")
        nc.sync.dma_start(qxbf[:], x_bf[b].rearrange("(kt p) t -> p kt t", p=128))
        Eqs = []
        for pr in range(PAIRS):
            wqmt = pstream.tile([128, KT_D, 128], BF16, tag="wmt")
            nc.sync.dma_start(wqmt[:], wqs[pr])
            pq_ = PP.tile([128, NT], F32, tag="pbig")
            for kt in range(KT_D):
                nc.tensor.matmul(pq_[:], wqmt[:, kt, :], qxbf[:, kt, :],
                                 start=kt == 0, stop=kt == KT_D - 1)
            qTmt = pmt.tile([128, NT], BF16, tag="mt512")
            nc.scalar.activation(qTmt[:], pq_[:], AF.Copy)
            qsqmt = pmt.tile([128, NT], BF16, tag="mt512")
            nc.scalar.square(qsqmt[:], pq_[:])
            pqs = PR.tile([2, NT], F32, tag="prow")
            nc.tensor.matmul(pqs[:], cAPs["sel2"][:], qsqmt[:], start=True, stop=True)
            qsq2 = psm.tile([2, NT], BF16, tag="qsq2")
            nc.scalar.activation(qsq2[:], pqs[:], AF.Copy)

            pdq = PP.tile([128, NT], F32, tag="pbig")
            nc.tensor.matmul(pdq[:], cAPs["projbd"][:], qTmt[:], start=True,
                             stop=False)
            nc.tensor.matmul(pdq[:], cAPs["negselF"][:], qsq2[:], start=False,
                             stop=True, skip_group_check=True)
            Etmp = pmt.tile([128, NT], BF16, tag="t512bf")
            nc.scalar.activation(Etmp[:], pdq[:], AF.Exp)

            pS = PR.tile([2, NT], F32, tag="prow")
            nc.tensor.matmul(pS[:], cAPs["sel2"][:], Etmp[:], start=True, stop=True)
            ediag = psm.tile([2, NT], BF16, tag="ediag")
            nc.scalar.activation(ediag[:], qsq2[:], AF.Exp, scale=float(DN2H))
            wrow = psm.tile([2, NT], BF16, tag="wrow")
            nc.vector.scalar_tensor_tensor(wrow[:], ediag[:], EPS_KERN, pS[:],
                                           op0=ALU.mult, op1=ALU.mult)
            pwB = PP.tile([128, NT], F32, tag="pbig")
            nc.tensor.matmul(pwB[:], sel2b_bf[:], wrow[:], start=True, stop=True)
            Eq = peq.tile([128, NT], BF16, tag=f"Eq{b % 2}_{pr}")
            nc.vector.tensor_tensor(Eq[:], Etmp[:], pwB[:], op=ALU.add)
            Eqs.append(Eq)
        Eq_all[b] = Eqs

    emit_qside(0)
    emit_qside(1)

    # ---- kv / ksum assembly ----
    mx8 = pone.tile([1, 1], F32, tag="mx8")
    nc.vector.tensor_reduce(mx8[:], arres[0:1, AC_A + AC_U:AC], axis=AX.X, op=ALU.max)
    emxf = pone.tile([1, 1], F32, tag="emxf")
    nc.scalar.activation(emxf[:], mx8[:], AF.Exp, scale=-1.0)
    emxrow = psm.tile([1, NT], F32R, tag="lnrow")
    nc.vector.tensor_scalar(emxrow[:], onesrow_c[:], emxf[:], None, op0=ALU.mult)
    pex = PP.tile([128, NT], F32, tag="pbig")
    nc.tensor.matmul(pex[:], ones1x128_r[:], emxrow[:], start=True, stop=True)
    emxc = pone.tile([128, 1], F32, tag="emxc")
    nc.vector.tensor_copy(emxc[:], pex[:, 0:1])

    epsv_sb = pbig.tile([128, B * PAIRS, 128], BF16, tag="big32")
    nc.sync.dma_start(epsv_sb[:], epsvB[:])

    # bulk kv assembly: one strided op per quadrant across all 32 (b, pair)
    # summaries instead of 128 tiny vector ops — this chain gates stage B.
    kvBall = pone.tile([128, B * PAIRS, 130], BF16, tag="kvBall")
    nc.vector.memset(kvBall[:], 0.0)
    arA0 = arres[0:64, 0:AC_A].rearrange("p (j c) -> p j c", j=B * PAIRS)
    nc.vector.scalar_tensor_tensor(kvBall[0:64, :, 0:64], arA0, emxc[0:64, :],
                                   epsv_sb[0:64, :, 0:64],
                                   op0=ALU.mult, op1=ALU.add)
    arA1 = arres[64:128, 0:AC_A].rearrange("p (j c) -> p j c", j=B * PAIRS)
    nc.vector.scalar_tensor_tensor(kvBall[64:128, :, 64:128], arA1,
                                   emxc[64:128, :], epsv_sb[64:128, :, 64:128],
                                   op0=ALU.mult, op1=ALU.add)
    ksfall = pone.tile([128, B * PAIRS], F32, tag="ksfall")
    nc.vector.tensor_scalar(ksfall[:], arres[:, AC_A:AC_A + B * PAIRS],
                            emxc[:], float(EPS_KERN * N),
                            op0=ALU.mult, op1=ALU.add)
    kvB_all = {}
    for j in range(B * PAIRS):
        nc.vector.tensor_scalar(kvBall[:, j, 128:130], cAPs["headmask2"][:],
                                ksfall[:, j:j + 1], None, op0=ALU.mult)
        kvB_all[j] = kvBall[:, j, :]

    # ================= stage B =================
    for b in range(B):
        Eqs = Eq_all[b]
        attnT = pbig.tile([128, KT_D, NT], BF16, tag="attnT")
        # pass 1: dense PE stream of num/den matmuls; scalar+vector trail
        numsbs, rdens = [], []
        for pr in range(PAIRS):
            kvB = kvB_all[b * PAIRS + pr]
            pnum = PP.tile([128, NT], F32, tag="pbig")
            nc.tensor.matmul(pnum[:], kvB[:, 0:128], Eqs[pr][:], start=True,
                             stop=True)
            pden = PD.tile([2, NT], F32, tag="pdd")
            nc.tensor.matmul(pden[:], kvB[:, 128:130], Eqs[pr][:], start=True,
                             stop=True)
            nc.scalar.activation(attnT[:, pr, :], pnum[:], AF.Copy)
            rdf = psm.tile([2, NT], F32, tag="rdf")
            nc.vector.reciprocal_approx_fast(rdf[:], pden[:])
            rden = psm.tile([2, NT], F32R, tag=f"rden{pr}", bufs=1)
            nc.vector.tensor_copy(rden[:], rdf[:])
            rdens.append(rden)
        # pass 2: broadcast 1/den and scale numerators in place
        for pr in range(PAIRS):
            prdB = PP.tile([128, NT], F32, tag="pbig")
            nc.tensor.matmul(prdB[:], sel2b_r[:], rdens[pr][:], start=True,
                             stop=True)
            nc.vector.tensor_tensor(attnT[:, pr, :], attnT[:, pr, :], prdB[:],
                                    op=ALU.mult)

        res1 = pbig.tile([128, KT_D, NT], F32R, tag="resX")
        for mt in range(KT_D):
            womt = pstream.tile([128, KT_D, 128], BF16, tag="wmt")
            nc.sync.dma_start(womt[:], wos[mt])
            po = PP.tile([128, NT], F32, tag="pbig")
            for kt in range(KT_D):
                nc.tensor.matmul(po[:], womt[:, kt, :], attnT[:, kt, :],
                                 start=kt == 0, stop=kt == KT_D - 1)
            xf = psm.tile([128, NT], F32, tag="xf")
            nc.sync.dma_start(xf[:], x_f32[b, mt * 128:(mt + 1) * 128, :])
            nc.vector.tensor_tensor(res1[:, mt, :], xf[:], po[:], op=ALU.add)

        out1 = ln(res1, cAPs["g1c"], cAPs["be1c"], "o1", mybir.dt.bfloat16)

        hsb = pbig.tile([128, MT_FF, NT], BF16, tag="big32")
        for mt in range(MT_FF):
            w1mt = pstream.tile([128, KT_D, 128], BF16, tag="wmt")
            nc.sync.dma_start(w1mt[:], w1s[mt])
            # 6-deep pz rotation (4 PP banks + the 2 PD banks, idle here) so
            # the matmul stream never waits on the scalar/vector ELU drain.
            if mt % 3 != 2:
                pz = PP.tile([128, NT], F32, tag="pbig", name="pz")
            else:
                pz = PD.tile([128, NT], F32, tag="pdd", name="pz")
            for kt in range(KT_D):
                nc.tensor.matmul(pz[:], w1mt[:, kt, :], out1[:, kt, :],
                                 start=kt == 0, stop=kt == KT_D - 1)
            # elu(z')+1 = max(z'+1, min(exp(z'), 1)): one scalar Exp, with the
            # min on the (FFN-idle) vector engine in 16-bit 2x mode.
            eraw = pmt.tile([128, NT], BF16, tag="t512bf")
            nc.scalar.activation(eraw[:], pz[:], AF.Exp,
                                 bias=cAPs["b1c"][:, mt:mt + 1])
            emin = pmt.tile([128, NT], BF16, tag="t512bf")
            nc.vector.tensor_scalar(emin[:], eraw[:], 1.0, None, op0=ALU.min)
            nc.vector.scalar_tensor_tensor(hsb[:, mt, :], pz[:],
                                           cAPs["b1p1c"][:, mt:mt + 1], emin[:],
                                           op0=ALU.add, op1=ALU.max)

        res2 = pbig.tile([128, KT_D, NT], F32R, tag="resX")
        for mt in range(KT_D):
            w2a = pw2s.tile([128, MT_FF // 2, 128], BF16, tag="w2mt")
            nc.sync.dma_start(w2a[:], w2s[mt, :, 0:MT_FF // 2])
            w2b = pw2s.tile([128, MT_FF // 2, 128], BF16, tag="w2mt")
            nc.sync.dma_start(w2b[:], w2s[mt, :, MT_FF // 2:MT_FF])
            pf = PP.tile([128, NT], F32, tag="pbig")
            for kt in range(MT_FF):
                w2h = w2a if kt < MT_FF // 2 else w2b
                nc.tensor.matmul(pf[:], w2h[:, kt % (MT_FF // 2), :],
                                 hsb[:, kt, :],
                                 start=kt == 0, stop=kt == MT_FF - 1)
            nc.vector.scalar_tensor_tensor(res2[:, mt, :], pf[:],
                                           cAPs["b2adjc"][:, mt:mt + 1],
                                           out1[:, mt, :], op0=ALU.add, op1=ALU.add)

        ln(res2, cAPs["g2c"], cAPs["be2c"], "o2", None, dma_out=(out, b))

        if b + 2 < B:
            emit_qside(b + 2)

    ctx.close()


_CACHE = {}


def _build():
    import concourse.tile as tile
    from concourse import bacc
    nc = bacc.Bacc("TRN2", target_bir_lowering=False, debug=False, num_devices=NC)
    with tile.TileContext(nc) as tc:
        _emit(nc, tc)
    nc.compile()
    return nc


def _host_inputs(x, Wq, Wk, Wv, Wo, proj, W1, b1, W2, b2,
                 ln1_g, ln1_b, ln2_g, ln2_b):
    bf = ml_dtypes.bfloat16
    f32 = np.float32
    d = {}

    def chunked(w):  # [D, X] -> [X/128 mt, 128 p, D/128 kt, 128]
        Dk, X = w.shape
        r = w.reshape(Dk // 128, 128, X // 128, 128)
        return np.ascontiguousarray(r.transpose(2, 1, 0, 3)).astype(bf)

    d["wqs"] = chunked(Wq.reshape(D, D))
    d["wks"] = chunked(Wk.reshape(D, D))
    d["wv"] = np.ascontiguousarray(Wv.reshape(D, D)).astype(bf)
    d["wos"] = chunked(Wo.reshape(D, D))
    d["w1s"] = chunked(W1)
    d["w2s"] = chunked(W2)

    projT_s = (proj * DN).T.astype(f32)
    pbd = np.zeros((128, 128), f32)
    pbd[0:64, 0:64] = projT_s
    pbd[64:128, 64:128] = projT_s
    d["projbd"] = pbd.astype(bf)
    nsF = np.zeros((2, 128), f32)
    nsF[0, 0:64] = -DN2H
    nsF[1, 64:128] = -DN2H
    d["negselF"] = nsF.astype(bf)
    s2 = np.zeros((128, 2), f32)
    s2[0:64, 0] = 1.0
    s2[64:128, 1] = 1.0
    d["sel2"] = s2.astype(bf)
    s2b = np.zeros((2, 128), f32)
    s2b[0, 0:64] = 1.0
    s2b[1, 64:128] = 1.0
    d["sel2b"] = s2b
    d["ones128"] = np.ones((128, 1), f32).astype(bf)
    d["ones1x128"] = np.ones((1, 128), f32)
    d["mean1"] = np.full((128, 1), 1.0 / D, f32)
    hm2 = np.zeros((128, 2), f32)
    hm2[0:64, 0] = 1.0
    hm2[64:128, 1] = 1.0
    d["headmask2"] = hm2

    xsum = x.sum(axis=1, dtype=np.float64)
    vsum = xsum @ Wv.reshape(D, D).astype(np.float64)
    epsv = (EPS_KERN * vsum).astype(f32)
    d["epsvB"] = np.ascontiguousarray(
        np.broadcast_to(epsv.reshape(1, B * D), (128, B * D))).astype(bf)

    d["b1c"] = np.ascontiguousarray(b1.reshape(MT_FF, 128).T).astype(f32)
    d["nb1c"] = np.ascontiguousarray((-b1).reshape(MT_FF, 128).T).astype(f32)
    d["b1p1c"] = np.ascontiguousarray((b1 + 1.0).reshape(MT_FF, 128).T).astype(f32)
    b2adj = b2.astype(np.float64) - W2.astype(np.float64).sum(axis=0)
    d["b2adjc"] = np.ascontiguousarray(b2adj.reshape(KT_D, 128).T).astype(f32)
    d["g1c"] = np.ascontiguousarray(ln1_g.reshape(KT_D, 128).T).astype(f32)
    d["be1c"] = np.ascontiguousarray(ln1_b.reshape(KT_D, 128).T).astype(f32)
    d["g2c"] = np.ascontiguousarray(ln2_g.reshape(KT_D, 128).T).astype(f32)
    d["be2c"] = np.ascontiguousarray(ln2_b.reshape(KT_D, 128).T).astype(f32)
    return d


def kernel(x, Wq, Wk, Wv, Wo, proj, W1, b1, W2, b2, ln1_g, ln1_b, ln2_g, ln2_b):
    from concourse import bass_utils

    x = np.asarray(x, np.float32)
    shared = _host_inputs(x, np.asarray(Wq), np.asarray(Wk), np.asarray(Wv),
                          np.asarray(Wo), np.asarray(proj), np.asarray(W1),
                          np.asarray(b1), np.asarray(W2), np.asarray(b2),
                          np.asarray(ln1_g), np.asarray(ln1_b),
                          np.asarray(ln2_g), np.asarray(ln2_b))

    if "nc" not in _CACHE:
        _CACHE["nc"] = _build()
    nc = _CACHE["nc"]

    in_maps = []
    for c in range(NC):
        xs = x[:, c * NT:(c + 1) * NT, :]
        xT = np.ascontiguousarray(xs.transpose(0, 2, 1))
        oh = np.zeros((1, NC), np.float32)
        oh[0, c] = 1.0
        m = dict(shared)
        m["x_f32"] = xT
        m["x_bf"] = xT.astype(ml_dtypes.bfloat16)
        m["onehot"] = oh
        in_maps.append(m)

    trace = bool(int(os.environ.get("KERNEL_TRACE", "0")))
    res = bass_utils.run_bass_kernel_spmd(nc, in_maps, core_ids=list(range(NC)),
                                          trace=trace)
    if trace and res.exec_time_ns is not None:
        print(f"HW exec time: {res.exec_time_ns} ns")
        if res.instructions_and_trace is not None:
            print("trace:", res.instructions_and_trace[1])

    outp = np.empty((B, N, D), np.float32)
    for c in range(NC):
        oT = res.results[c]["out"]
        outp[:, c * NT:(c + 1) * NT, :] = oT.transpose(0, 2, 1)
    return outp



# revision 42
# speedup vs baseline: 1.7464x; 1.0091x over previous
"""Performer (FAVOR+) encoder layer on 8 trn2 NeuronCores.

Sharding: data-parallel over sequence (512 positions per core x 4 batches).
The linear-attention summaries (A = E_k^T v per (batch, head), usum) and the
global key-feature max (via one-hot slots) are combined in ONE packed
AllReduce, overlapped with the Q-side feature compute.

Layout: activations feature-major (xT = [D, tokens]) so every GEMM's
stationary operand is a natural weight slice; per-token reductions and
broadcasts are small PE matmuls (selector / ones / f32r broadcast matmuls).
E_k and v are produced token-major directly by matmuls so the token-
contraction A-matmul needs no transposes.
"""
import os
import numpy as np
import ml_dtypes

B, N, D = 4, 4096, 1024
H, DH = 16, 64
DFF = 4096
M = 64
EPS_KERN = 1e-6
EPS_LN = 1e-6
NC = 8
NT = N // NC                # 512 positions per core per batch
PAIRS = H // 2              # 8 head-pairs
KT_D = D // 128             # 8
MT_FF = DFF // 128          # 32
TT = NT // 128              # 4
DN = 1.0 / np.sqrt(np.sqrt(DH))
DN2H = DN * DN / 2.0


def _emit(nc, tc):
    import concourse.mybir as mybir
    from contextlib import ExitStack
    F32 = mybir.dt.float32
    F32R = mybir.dt.float32r
    BF16 = mybir.dt.bfloat16
    AF = mybir.ActivationFunctionType
    ALU = mybir.AluOpType
    AX = mybir.AxisListType

    dram = lambda name, shape, dt, kind: nc.dram_tensor(name, shape, dt, kind=kind).ap()

    x_bf = dram("x_bf", [B, D, NT], BF16, "ExternalInput")
    x_f32 = dram("x_f32", [B, D, NT], F32, "ExternalInput")
    wqs = dram("wqs", [KT_D, 128, KT_D, 128], BF16, "ExternalInput")
    wks = dram("wks", [KT_D, 128, KT_D, 128], BF16, "ExternalInput")
    wv = dram("wv", [D, D], BF16, "ExternalInput")
    wos = dram("wos", [KT_D, 128, KT_D, 128], BF16, "ExternalInput")
    w1s = dram("w1s", [MT_FF, 128, KT_D, 128], BF16, "ExternalInput")
    w2s = dram("w2s", [KT_D, 128, MT_FF, 128], BF16, "ExternalInput")
    projbd = dram("projbd", [128, 128], BF16, "ExternalInput")
    negselF = dram("negselF", [2, 128], BF16, "ExternalInput")
    sel2 = dram("sel2", [128, 2], BF16, "ExternalInput")
    sel2b = dram("sel2b", [2, 128], F32, "ExternalInput")
    ones128 = dram("ones128", [128, 1], BF16, "ExternalInput")
    ones1x128 = dram("ones1x128", [1, 128], F32, "ExternalInput")
    mean1 = dram("mean1", [128, 1], F32, "ExternalInput")
    headmask2 = dram("headmask2", [128, 2], F32, "ExternalInput")
    epsvB = dram("epsvB", [128, B * D], BF16, "ExternalInput")
    onehot = dram("onehot", [1, NC], F32, "ExternalInput")
    b1c = dram("b1c", [128, MT_FF], F32, "ExternalInput")
    nb1c = dram("nb1c", [128, MT_FF], F32, "ExternalInput")
    b1p1c = dram("b1p1c", [128, MT_FF], F32, "ExternalInput")
    b2adjc = dram("b2adjc", [128, KT_D], F32, "ExternalInput")
    g1c = dram("g1c", [128, KT_D], F32, "ExternalInput")
    be1c = dram("be1c", [128, KT_D], F32, "ExternalInput")
    g2c = dram("g2c", [128, KT_D], F32, "ExternalInput")
    be2c = dram("be2c", [128, KT_D], F32, "ExternalInput")
    out = dram("out", [B, D, NT], F32, "ExternalOutput")

    AC_A = B * PAIRS * 64
    AC_U = B * PAIRS
    AC = AC_A + AC_U + NC

    ctx = ExitStack()
    pconst = ctx.enter_context(tc.tile_pool(name="pconst", bufs=1))
    pstream = ctx.enter_context(tc.tile_pool(name="pstream", bufs=2))
    pw2s = ctx.enter_context(tc.tile_pool(name="pw2s", bufs=2))
    pxa = ctx.enter_context(tc.tile_pool(name="pxa", bufs=1))
    pmt = ctx.enter_context(tc.tile_pool(name="pmt", bufs=4))
    psm = ctx.enter_context(tc.tile_pool(name="psm", bufs=2))
    peq = ctx.enter_context(tc.tile_pool(name="peq", bufs=1))
    pbig = ctx.enter_context(tc.tile_pool(name="pbig", bufs=1))
    pone = ctx.enter_context(tc.tile_pool(name="pone", bufs=1))
    pdram = ctx.enter_context(tc.tile_pool(name="pdram", bufs=1, space="DRAM"))
    PP = ctx.enter_context(tc.tile_pool(name="PP", bufs=4, space="PSUM"))
    PD = ctx.enter_context(tc.tile_pool(name="PD", bufs=2, space="PSUM"))
    PA_ = ctx.enter_context(tc.tile_pool(name="PA", bufs=1, space="PSUM"))
    PR = ctx.enter_context(tc.tile_pool(name="PR", bufs=1, space="PSUM"))

    # ---- constants ----
    # wv shares the big32 slot with epsv/hsb: wv is only read in stage A,
    # epsv only in kv assembly, hsb only from FFN1 onward — disjoint uses.
    wv_sb = pbig.tile([128, KT_D, D], BF16, tag="big32")
    nc.sync.dma_start(wv_sb[:], wv.rearrange("(kt p) m -> p kt m", p=128))
    cAPs = {}
    for name, ap, shape, dt in (
        ("projbd", projbd, [128, 128], BF16), ("negselF", negselF, [2, 128], BF16),
        ("sel2", sel2, [128, 2], BF16), ("sel2b", sel2b, [2, 128], F32),
        ("ones128", ones128, [128, 1], BF16), ("ones1x128", ones1x128, [1, 128], F32),
        ("mean1", mean1, [128, 1], F32), ("headmask2", headmask2, [128, 2], F32),
        ("onehot", onehot, [1, NC], F32), ("b1c", b1c, [128, MT_FF], F32),
        ("nb1c", nb1c, [128, MT_FF], F32),
        ("b1p1c", b1p1c, [128, MT_FF], F32), ("b2adjc", b2adjc, [128, KT_D], F32),
        ("g1c", g1c, [128, KT_D], F32), ("be1c", be1c, [128, KT_D], F32),
        ("g2c", g2c, [128, KT_D], F32), ("be2c", be2c, [128, KT_D], F32),
    ):
        t = pconst.tile(shape, dt, tag=name)
        nc.sync.dma_start(t[:], ap[:])
        cAPs[name] = t
    sel2b_r = pconst.tile([2, 128], F32R, tag="sel2br")
    ones1x128_r = pconst.tile([1, 128], F32R, tag="ones1x128r")
    mean1_r = pconst.tile([128, 1], F32R, tag="mean1r")
    mean1_bf = pconst.tile([128, 1], BF16, tag="mean1bf")
    sel2b_bf = pconst.tile([2, 128], BF16, tag="sel2bbf")
    nc.vector.tensor_copy(sel2b_r[:], cAPs["sel2b"][:])
    nc.vector.tensor_copy(ones1x128_r[:], cAPs["ones1x128"][:])
    nc.vector.tensor_copy(mean1_r[:], cAPs["mean1"][:])
    nc.vector.tensor_copy(mean1_bf[:], cAPs["mean1"][:])
    nc.vector.tensor_copy(sel2b_bf[:], cAPs["sel2b"][:])

    epsln_c = pconst.tile([1, 1], F32, tag="epslnc")
    nc.vector.memset(epsln_c[:], float(EPS_LN))
    onesrow_c = pconst.tile([1, NT], F32, tag="onesrowc")
    nc.vector.memset(onesrow_c[:], 1.0)
    magicrow = pconst.tile([1, NT], mybir.dt.int32, tag="magicrow")
    nc.vector.memset(magicrow[:], 0x5f3759df)
    oneirow = pconst.tile([1, NT], mybir.dt.int32, tag="oneirow")
    nc.vector.memset(oneirow[:], 1)
    arstage = pone.tile([128, AC], F32, tag="arbuf")
    mxcols = pone.tile([128, B * PAIRS], BF16, tag="mxcols")

    def ln_stats(res):
        """Feature-major LN stats for a [128, KT_D, NT] f32r residual.
        rstd = 1/sqrt(var+eps) entirely on the vector engine (bit-magic seed +
        2 Newton steps) — the scalar Ln/Exp pair forced an ACT_TABLE_LOAD per
        LN. Returns (mu f32r row, rstd f32r row)."""
        pm0 = PR.tile([1, NT], F32, tag="prow")
        pm1 = PA_.tile([1, NT], F32, tag="pA")
        for kt in range(KT_D):
            sqt = psm.tile([128, NT], BF16, tag="lnsq")
            nc.scalar.square(sqt[:], res[:, kt, :].bitcast(F32))
            nc.tensor.matmul(pm0[:], mean1_r[:], res[:, kt, :],
                             start=kt == 0, stop=kt == KT_D - 1,
                             skip_group_check=True)
            nc.tensor.matmul(pm1[:], mean1_bf[:], sqt[:],
                             start=kt == 0, stop=kt == KT_D - 1,
                             skip_group_check=True)
        mu = psm.tile([1, NT], F32R, tag="lnmu", bufs=1)
        nc.vector.tensor_copy(mu[:], pm0[:])
        muf = mu[:].bitcast(F32)
        mu2 = psm.tile([1, NT], F32, tag="lnrow")
        nc.vector.tensor_tensor(mu2[:], muf, muf, op=ALU.mult)
        var = psm.tile([1, NT], F32, tag="lnrow")
        nc.vector.tensor_tensor(var[:], pm1[:], mu2[:], op=ALU.subtract)
        vare = psm.tile([1, NT], F32, tag="lnvare", bufs=1)
        nc.vector.tensor_scalar(vare[:], var[:], float(EPS_LN), None,
                                op0=ALU.add)
        I32 = mybir.dt.int32
        sh = psm.tile([1, NT], I32, tag="lnrow")
        nc.vector.tensor_tensor(sh[:], vare[:].bitcast(I32), oneirow[:],
                                op=ALU.arith_shift_right)
        y0 = psm.tile([1, NT], I32, tag="lnya", bufs=1)
        nc.vector.tensor_tensor(y0[:], magicrow[:], sh[:], op=ALU.subtract)
        y = y0[:].bitcast(F32)
        for it in range(2):
            t1 = psm.tile([1, NT], F32, tag="lnrow")
            nc.vector.tensor_tensor(t1[:], y, y, op=ALU.mult)
            t2 = psm.tile([1, NT], F32, tag="lnrow")
            nc.vector.tensor_tensor(t2[:], t1[:], vare[:], op=ALU.mult)
            t3 = psm.tile([1, NT], F32, tag="lnrow")
            nc.vector.tensor_scalar(t3[:], t2[:], -0.5, 1.5,
                                    op0=ALU.mult, op1=ALU.add)
            yn = psm.tile([1, NT], F32, tag=f"lny{it}", bufs=1)
            nc.vector.tensor_tensor(yn[:], y, t3[:], op=ALU.mult)
            y = yn[:]
        rstd = psm.tile([1, NT], F32R, tag="lnrstd", bufs=1)
        nc.vector.tensor_copy(rstd[:], y)
        return mu, rstd

    def ln_apply(mu, rstd, res, gc, bc, odt, dma_out=None):
        # pmu/prs live in the PD banks (idle during LN windows) so the PP
        # rotation used by the surrounding matmul streams never stalls on
        # the long apply reads.
        pmu = PD.tile([128, NT], F32, tag="pdd")
        nc.tensor.matmul(pmu[:], ones1x128_r[:], mu[:], start=True, stop=True)
        prs = PD.tile([128, NT], F32, tag="pdd")
        nc.tensor.matmul(prs[:], ones1x128_r[:], rstd[:], start=True, stop=True)
        o = None
        if odt is not None:
            o = pbig.tile([128, KT_D, NT], odt, tag="bf8")
        cen = psm.tile([128, NT], F32, tag="lncen")
        nrm = psm.tile([128, NT], BF16 if odt is not None else F32, tag="lnnrm")
        for kt in range(KT_D):
            nc.vector.tensor_tensor(cen[:], res[:, kt, :].bitcast(F32), pmu[:],
                                    op=ALU.subtract)
            nc.vector.scalar_tensor_tensor(nrm[:], cen[:], gc[:, kt:kt + 1],
                                           prs[:], op0=ALU.mult, op1=ALU.mult)
            if o is not None:
                nc.scalar.activation(o[:, kt, :], nrm[:], AF.Identity,
                                     bias=bc[:, kt:kt + 1])
            else:
                ot = psm.tile([128, NT], F32, tag="lnot")
                nc.scalar.activation(ot[:], nrm[:], AF.Identity,
                                     bias=bc[:, kt:kt + 1])
                nc.sync.dma_start(dma_out[0][dma_out[1], kt * 128:(kt + 1) * 128, :],
                                  ot[:])
        return o

    # ================= stage A =================
    for b in range(B):
        xbf = pxa.tile([128, KT_D, NT], BF16, tag="xbf")
        nc.sync.dma_start(xbf[:], x_bf[b].rearrange("(kt p) t -> p kt t", p=128))

        vtok = pxa.tile([128, TT, PAIRS, 129], BF16, tag="vtok")
        nc.vector.memset(vtok[:, :, :, 128:129], 1.0)
        for tt in range(TT):
            for nh in range(2):
                pv = PP.tile([128, 4, 128], F32, tag="pbig")
                for kt in range(KT_D):
                    nc.tensor.matmul(pv[:], xbf[:, kt, tt * 128:(tt + 1) * 128],
                                     wv_sb[:, kt, nh * 512:(nh + 1) * 512],
                                     start=kt == 0, stop=kt == KT_D - 1)
                nc.vector.tensor_copy(vtok[:, tt, nh * 4:(nh + 1) * 4, 0:128],
                                      pv[:])

        # per-pair chain software-pipelined across pairs (depth 2) so the PE
        # never waits on the scalar/vector singletons between sub-phases.
        kTs, ksq2s, Eks = {}, {}, {}

        def s1(pr):  # K projection -> token-major k + k^2 (scalar trails)
            wkmt = pstream.tile([128, KT_D, 128], BF16, tag="wmt")
            nc.sync.dma_start(wkmt[:], wks[pr])
            pk = PP.tile([128, NT], F32, tag="pbig")
            for kt in range(KT_D):
                nc.tensor.matmul(pk[:], wkmt[:, kt, :], xbf[:, kt, :],
                                 start=kt == 0, stop=kt == KT_D - 1)
            kTmt = pmt.tile([128, NT], BF16, tag="mt512")
            nc.scalar.activation(kTmt[:], pk[:], AF.Copy)
            ksqmt = pmt.tile([128, NT], BF16, tag="mt512")
            nc.scalar.square(ksqmt[:], pk[:])
            kTs[pr] = (kTmt, ksqmt)

        def s2(pr):  # squared-norm row + FAVOR features
            kTmt, ksqmt = kTs[pr]
            pks = PR.tile([2, NT], F32, tag="prow")
            nc.tensor.matmul(pks[:], cAPs["sel2"][:], ksqmt[:], start=True,
                             stop=True)
            ksq2 = psm.tile([2, NT], BF16, tag="ksq2")
            nc.scalar.activation(ksq2[:], pks[:], AF.Copy)
            ksq2s[pr] = ksq2

            # One accumulation group for the whole bank: only the FIRST matmul
            # carries start=True (whole-bank has_written clear); later slices
            # overwrite-and-set, negsels then accumulate onto set bits. The
            # raw-projection max is one whole-bank reduce between the phases.
            Ek = psm.tile([128, TT, 128], BF16, tag="Ek")
            pdd = PD.tile([128, NT], F32, tag="pdd")
            sls = [slice(tt * 128, (tt + 1) * 128) for tt in range(TT)]
            for tt in range(TT):
                nc.tensor.matmul(pdd[:, sls[tt]], kTmt[:, sls[tt]],
                                 cAPs["projbd"][:], start=tt == 0, stop=False,
                                 skip_group_check=True)
            c = b * PAIRS + pr
            nc.vector.tensor_reduce(mxcols[:, c:c + 1], pdd[:], axis=AX.X,
                                    op=ALU.max)
            for tt in range(TT):
                nc.tensor.matmul(pdd[:, sls[tt]], ksq2[:, sls[tt]],
                                 cAPs["negselF"][:], start=False,
                                 stop=tt == TT - 1, skip_group_check=True)
            nc.scalar.activation(Ek[:], pdd[:], AF.Exp)
            Eks[pr] = Ek

        def s3(pr):  # token-contraction A matmuls + AR staging
            Ek = Eks[pr]
            pA = PA_.tile([128, 129], F32, tag="pA")
            for tt in range(TT):
                nc.tensor.matmul(pA[:], Ek[:, tt, :], vtok[:, tt, pr, :],
                                 start=tt == 0, stop=tt == TT - 1,
                                 skip_group_check=True)
            j = b * PAIRS + pr
            nc.vector.tensor_copy(arstage[0:64, j * 64:(j + 1) * 64], pA[0:64, 0:64])
            nc.vector.tensor_copy(arstage[64:128, j * 64:(j + 1) * 64],
                                  pA[64:128, 64:128])
            nc.vector.tensor_copy(arstage[:, AC_A + j:AC_A + j + 1], pA[:, 128:129])

        for step in range(PAIRS + 2):
            if step < PAIRS:
                s1(step)
            if 1 <= step <= PAIRS:
                s2(step - 1)
            if step >= 2:
                s3(step - 2)

    # ---- fire AllReduce ----
    mxr = pone.tile([128, 1], F32, tag="mxr")
    nc.vector.tensor_reduce(mxr[:], mxcols[:], axis=AX.X, op=ALU.max)
    mx1 = pone.tile([1, 1], F32, tag="mx1")
    nc.gpsimd.tensor_reduce(mx1[:], mxr[:], axis=AX.C, op=ALU.max)
    nc.vector.tensor_scalar(arstage[0:1, AC_A + AC_U:AC], cAPs["onehot"][:],
                            mx1[:], None, op0=ALU.mult)
    arin = pdram.tile([128, AC], F32, tag="arin")
    arout = pdram.tile([128, AC], F32, tag="arout")
    nc.sync.dma_start(arin[:], arstage[:])
    if os.environ.get("KERNEL_NOCOLL"):
        nc.sync.dma_start(arout[:], arin[:])
    else:
        nc.gpsimd.collective_compute("AllReduce", ALU.add,
                                     replica_groups=[list(range(NC))],
                                     ins=[arin[:]], outs=[arout[:]])
    arres = pone.tile([128, AC], F32, tag="arbuf")
    nc.sync.dma_start(arres[:], arout[:])

    # ================= q-side features (overlap AR) =================
    Eq_all = {}

    def emit_qside(b):
        qxbf = pxa.tile([128, KT_D, NT], BF16, tag="qxbf")
        nc.sync.dma_start(qxbf[:], x_bf[b].rearrange("(kt p) t -> p kt t", p=128))
        Eqs = []
        for pr in range(PAIRS):
            wqmt = pstream.tile([128, KT_D, 128], BF16, tag="wmt")
            nc.sync.dma_start(wqmt[:], wqs[pr])
            pq_ = PP.tile([128, NT], F32, tag="pbig")
            for kt in range(KT_D):
                nc.tensor.matmul(pq_[:], wqmt[:, kt, :], qxbf[:, kt, :],
                                 start=kt == 0, stop=kt == KT_D - 1)
            qTmt = pmt.tile([128, NT], BF16, tag="mt512")
            nc.scalar.activation(qTmt[:], pq_[:], AF.Copy)
            qsqmt = pmt.tile([128, NT], BF16, tag="mt512")
            nc.scalar.square(qsqmt[:], pq_[:])
            pqs = PR.tile([2, NT], F32, tag="prow")
            nc.tensor.matmul(pqs[:], cAPs["sel2"][:], qsqmt[:], start=True, stop=True)
            qsq2 = psm.tile([2, NT], BF16, tag="qsq2")
            nc.scalar.activation(qsq2[:], pqs[:], AF.Copy)

            pdq = PP.tile([128, NT], F32, tag="pbig")
            nc.tensor.matmul(pdq[:], cAPs["projbd"][:], qTmt[:], start=True,
                             stop=False)
            nc.tensor.matmul(pdq[:], cAPs["negselF"][:], qsq2[:], start=False,
                             stop=True, skip_group_check=True)
            Etmp = pmt.tile([128, NT], BF16, tag="t512bf")
            nc.scalar.activation(Etmp[:], pdq[:], AF.Exp)

            pS = PR.tile([2, NT], F32, tag="prow")
            nc.tensor.matmul(pS[:], cAPs["sel2"][:], Etmp[:], start=True, stop=True)
            ediag = psm.tile([2, NT], BF16, tag="ediag")
            nc.scalar.activation(ediag[:], qsq2[:], AF.Exp, scale=float(DN2H))
            wrow = psm.tile([2, NT], BF16, tag="wrow")
            nc.vector.scalar_tensor_tensor(wrow[:], ediag[:], EPS_KERN, pS[:],
                                           op0=ALU.mult, op1=ALU.mult)
            pwB = PP.tile([128, NT], F32, tag="pbig")
            nc.tensor.matmul(pwB[:], sel2b_bf[:], wrow[:], start=True, stop=True)
            Eq = peq.tile([128, NT], BF16, tag=f"Eq{b % 2}_{pr}")
            nc.vector.tensor_tensor(Eq[:], Etmp[:], pwB[:], op=ALU.add)
            Eqs.append(Eq)
        Eq_all[b] = Eqs

    emit_qside(0)
    emit_qside(1)

    # ---- kv / ksum assembly ----
    mx8 = pone.tile([1, 1], F32, tag="mx8")
    nc.vector.tensor_reduce(mx8[:], arres[0:1, AC_A + AC_U:AC], axis=AX.X, op=ALU.max)
    emxf = pone.tile([1, 1], F32, tag="emxf")
    nc.scalar.activation(emxf[:], mx8[:], AF.Exp, scale=-1.0)
    emxrow = psm.tile([1, NT], F32R, tag="lnrow")
    nc.vector.tensor_scalar(emxrow[:], onesrow_c[:], emxf[:], None, op0=ALU.mult)
    pex = PP.tile([128, NT], F32, tag="pbig")
    nc.tensor.matmul(pex[:], ones1x128_r[:], emxrow[:], start=True, stop=True)
    emxc = pone.tile([128, 1], F32, tag="emxc")
    nc.vector.tensor_copy(emxc[:], pex[:, 0:1])

    epsv_sb = pbig.tile([128, B * PAIRS, 128], BF16, tag="big32")
    nc.sync.dma_start(epsv_sb[:], epsvB[:])

    # bulk kv assembly: one strided op per quadrant across all 32 (b, pair)
    # summaries instead of 128 tiny vector ops — this chain gates stage B.
    kvBall = pone.tile([128, B * PAIRS, 130], BF16, tag="kvBall")
    nc.vector.memset(kvBall[:], 0.0)
    arA0 = arres[0:64, 0:AC_A].rearrange("p (j c) -> p j c", j=B * PAIRS)
    nc.vector.scalar_tensor_tensor(kvBall[0:64, :, 0:64], arA0, emxc[0:64, :],
                                   epsv_sb[0:64, :, 0:64],
                                   op0=ALU.mult, op1=ALU.add)
    arA1 = arres[64:128, 0:AC_A].rearrange("p (j c) -> p j c", j=B * PAIRS)
    nc.vector.scalar_tensor_tensor(kvBall[64:128, :, 64:128], arA1,
                                   emxc[64:128, :], epsv_sb[64:128, :, 64:128],
                                   op0=ALU.mult, op1=ALU.add)
    ksfall = pone.tile([128, B * PAIRS], F32, tag="ksfall")
    nc.vector.tensor_scalar(ksfall[:], arres[:, AC_A:AC_A + B * PAIRS],
                            emxc[:], float(EPS_KERN * N),
                            op0=ALU.mult, op1=ALU.add)
    kvB_all = {}
    for j in range(B * PAIRS):
        nc.vector.tensor_scalar(kvBall[:, j, 128:130], cAPs["headmask2"][:],
                                ksfall[:, j:j + 1], None, op0=ALU.mult)
        kvB_all[j] = kvBall[:, j, :]

    # ================= stage B (software-pipelined across batches) ==========
    def attn(b):
        Eqs = Eq_all[b]
        attnT = pbig.tile([128, KT_D, NT], BF16, tag="attnT")
        rdens = {}

        def pass1(pr):
            kvB = kvB_all[b * PAIRS + pr]
            pnum = PP.tile([128, NT], F32, tag="pbig")
            nc.tensor.matmul(pnum[:], kvB[:, 0:128], Eqs[pr][:], start=True,
                             stop=True)
            pden = PD.tile([2, NT], F32, tag="pdd")
            nc.tensor.matmul(pden[:], kvB[:, 128:130], Eqs[pr][:], start=True,
                             stop=True)
            nc.scalar.activation(attnT[:, pr, :], pnum[:], AF.Copy)
            rdf = psm.tile([2, NT], F32, tag="rdf")
            nc.vector.reciprocal_approx_fast(rdf[:], pden[:])
            rden = psm.tile([2, NT], F32R, tag="rden", bufs=4)
            nc.vector.tensor_copy(rden[:], rdf[:])
            rdens[pr] = rden

        def pass2(pr):
            prdB = PP.tile([128, NT], F32, tag="pbig")
            nc.tensor.matmul(prdB[:], sel2b_r[:], rdens[pr][:], start=True,
                             stop=True)
            nc.vector.tensor_tensor(attnT[:, pr, :], attnT[:, pr, :], prdB[:],
                                    op=ALU.mult)

        # lag-2 interleave keeps the PE dense while the reciprocal of pair p
        # finishes during pairs p+1/p+2's matmuls.
        for step in range(PAIRS + 2):
            if step < PAIRS:
                pass1(step)
            if step >= 2:
                pass2(step - 2)
        return attnT

    def wo_res1(b, attnT):
        res1 = pbig.tile([128, KT_D, NT], F32R, tag="resX")
        for mt in range(KT_D):
            womt = pstream.tile([128, KT_D, 128], BF16, tag="wmt")
            nc.sync.dma_start(womt[:], wos[mt])
            po = PP.tile([128, NT], F32, tag="pbig")
            for kt in range(KT_D):
                nc.tensor.matmul(po[:], womt[:, kt, :], attnT[:, kt, :],
                                 start=kt == 0, stop=kt == KT_D - 1)
            xf = psm.tile([128, NT], F32, tag="xf")
            nc.sync.dma_start(xf[:], x_f32[b, mt * 128:(mt + 1) * 128, :])
            nc.vector.tensor_tensor(res1[:, mt, :], xf[:], po[:], op=ALU.add)
        return res1

    def ffn(b, out1):
        hsb = pbig.tile([128, MT_FF, NT], BF16, tag="big32")
        for mt in range(MT_FF):
            w1mt = pstream.tile([128, KT_D, 128], BF16, tag="wmt")
            nc.sync.dma_start(w1mt[:], w1s[mt])
            pz = PP.tile([128, NT], F32, tag="pbig", name="pz")
            for kt in range(KT_D):
                nc.tensor.matmul(pz[:], w1mt[:, kt, :], out1[:, kt, :],
                                 start=kt == 0, stop=kt == KT_D - 1)
            # elu(z')+1 = max(z'+1, min(exp(z'), 1)): one scalar Exp, with the
            # min on the (FFN-idle) vector engine in 16-bit 2x mode.
            eraw = pmt.tile([128, NT], BF16, tag="t512bf")
            nc.scalar.activation(eraw[:], pz[:], AF.Exp,
                                 bias=cAPs["b1c"][:, mt:mt + 1])
            emin = pmt.tile([128, NT], BF16, tag="t512bf")
            nc.vector.tensor_scalar(emin[:], eraw[:], 1.0, None, op0=ALU.min)
            nc.vector.scalar_tensor_tensor(hsb[:, mt, :], pz[:],
                                           cAPs["b1p1c"][:, mt:mt + 1], emin[:],
                                           op0=ALU.add, op1=ALU.max)

        res2 = pbig.tile([128, KT_D, NT], F32R, tag="resX")
        for mt in range(KT_D):
            w2a = pw2s.tile([128, MT_FF // 2, 128], BF16, tag="w2mt")
            nc.sync.dma_start(w2a[:], w2s[mt, :, 0:MT_FF // 2])
            w2b = pw2s.tile([128, MT_FF // 2, 128], BF16, tag="w2mt")
            nc.sync.dma_start(w2b[:], w2s[mt, :, MT_FF // 2:MT_FF])
            pf = PP.tile([128, NT], F32, tag="pbig")
            for kt in range(MT_FF):
                w2h = w2a if kt < MT_FF // 2 else w2b
                nc.tensor.matmul(pf[:], w2h[:, kt % (MT_FF // 2), :],
                                 hsb[:, kt, :],
                                 start=kt == 0, stop=kt == MT_FF - 1)
            nc.vector.scalar_tensor_tensor(res2[:, mt, :], pf[:],
                                           cAPs["b2adjc"][:, mt:mt + 1],
                                           out1[:, mt, :], op0=ALU.add, op1=ALU.add)
        return res2

    res1 = wo_res1(0, attn(0))
    for b in range(B):
        mu1, rstd1 = ln_stats(res1)
        if b + 2 < B:
            # qside PE work covers the LN1 stat chain + apply window
            emit_qside(b + 2)
        out1 = ln_apply(mu1, rstd1, res1, cAPs["g1c"], cAPs["be1c"],
                        mybir.dt.bfloat16)
        res2 = ffn(b, out1)
        mu2, rstd2 = ln_stats(res2)
        if b + 1 < B:
            # next batch's attention matmuls cover the LN2 chain; its Wo
            # stream then runs while LN2 applies and the output DMAs out.
            attnT = attn(b + 1)
        ln_apply(mu2, rstd2, res2, cAPs["g2c"], cAPs["be2c"], None,
                 dma_out=(out, b))
        if b + 1 < B:
            res1 = wo_res1(b + 1, attnT)

    ctx.close()


_CACHE = {}


def _build():
    import concourse.tile as tile
    from concourse import bacc
    nc = bacc.Bacc("TRN2", target_bir_lowering=False, debug=False, num_devices=NC)
    with tile.TileContext(nc) as tc:
        _emit(nc, tc)
    nc.compile()
    return nc


def _host_inputs(x, Wq, Wk, Wv, Wo, proj, W1, b1, W2, b2,
                 ln1_g, ln1_b, ln2_g, ln2_b):
    bf = ml_dtypes.bfloat16
    f32 = np.float32
    d = {}

    def chunked(w):  # [D, X] -> [X/128 mt, 128 p, D/128 kt, 128]
        Dk, X = w.shape
        r = w.reshape(Dk // 128, 128, X // 128, 128)
        return np.ascontiguousarray(r.transpose(2, 1, 0, 3)).astype(bf)

    d["wqs"] = chunked(Wq.reshape(D, D))
    d["wks"] = chunked(Wk.reshape(D, D))
    d["wv"] = np.ascontiguousarray(Wv.reshape(D, D)).astype(bf)
    d["wos"] = chunked(Wo.reshape(D, D))
    d["w1s"] = chunked(W1)
    d["w2s"] = chunked(W2)

    projT_s = (proj * DN).T.astype(f32)
    pbd = np.zeros((128, 128), f32)
    pbd[0:64, 0:64] = projT_s
    pbd[64:128, 64:128] = projT_s
    d["projbd"] = pbd.astype(bf)
    nsF = np.zeros((2, 128), f32)
    nsF[0, 0:64] = -DN2H
    nsF[1, 64:128] = -DN2H
    d["negselF"] = nsF.astype(bf)
    s2 = np.zeros((128, 2), f32)
    s2[0:64, 0] = 1.0
    s2[64:128, 1] = 1.0
    d["sel2"] = s2.astype(bf)
    s2b = np.zeros((2, 128), f32)
    s2b[0, 0:64] = 1.0
    s2b[1, 64:128] = 1.0
    d["sel2b"] = s2b
    d["ones128"] = np.ones((128, 1), f32).astype(bf)
    d["ones1x128"] = np.ones((1, 128), f32)
    d["mean1"] = np.full((128, 1), 1.0 / D, f32)
    hm2 = np.zeros((128, 2), f32)
    hm2[0:64, 0] = 1.0
    hm2[64:128, 1] = 1.0
    d["headmask2"] = hm2

    xsum = x.sum(axis=1, dtype=np.float64)
    vsum = xsum @ Wv.reshape(D, D).astype(np.float64)
    epsv = (EPS_KERN * vsum).astype(f32)
    d["epsvB"] = np.ascontiguousarray(
        np.broadcast_to(epsv.reshape(1, B * D), (128, B * D))).astype(bf)

    d["b1c"] = np.ascontiguousarray(b1.reshape(MT_FF, 128).T).astype(f32)
    d["nb1c"] = np.ascontiguousarray((-b1).reshape(MT_FF, 128).T).astype(f32)
    d["b1p1c"] = np.ascontiguousarray((b1 + 1.0).reshape(MT_FF, 128).T).astype(f32)
    b2adj = b2.astype(np.float64) - W2.astype(np.float64).sum(axis=0)
    d["b2adjc"] = np.ascontiguousarray(b2adj.reshape(KT_D, 128).T).astype(f32)
    d["g1c"] = np.ascontiguousarray(ln1_g.reshape(KT_D, 128).T).astype(f32)
    d["be1c"] = np.ascontiguousarray(ln1_b.reshape(KT_D, 128).T).astype(f32)
    d["g2c"] = np.ascontiguousarray(ln2_g.reshape(KT_D, 128).T).astype(f32)
    d["be2c"] = np.ascontiguousarray(ln2_b.reshape(KT_D, 128).T).astype(f32)
    return d


def kernel(x, Wq, Wk, Wv, Wo, proj, W1, b1, W2, b2, ln1_g, ln1_b, ln2_g, ln2_b):
    from concourse import bass_utils

    x = np.asarray(x, np.float32)
    shared = _host_inputs(x, np.asarray(Wq), np.asarray(Wk), np.asarray(Wv),
                          np.asarray(Wo), np.asarray(proj), np.asarray(W1),
                          np.asarray(b1), np.asarray(W2), np.asarray(b2),
                          np.asarray(ln1_g), np.asarray(ln1_b),
                          np.asarray(ln2_g), np.asarray(ln2_b))

    if "nc" not in _CACHE:
        _CACHE["nc"] = _build()
    nc = _CACHE["nc"]

    in_maps = []
    for c in range(NC):
        xs = x[:, c * NT:(c + 1) * NT, :]
        xT = np.ascontiguousarray(xs.transpose(0, 2, 1))
        oh = np.zeros((1, NC), np.float32)
        oh[0, c] = 1.0
        m = dict(shared)
        m["x_f32"] = xT
        m["x_bf"] = xT.astype(ml_dtypes.bfloat16)
        m["onehot"] = oh
        in_maps.append(m)

    trace = bool(int(os.environ.get("KERNEL_TRACE", "0")))
    res = bass_utils.run_bass_kernel_spmd(nc, in_maps, core_ids=list(range(NC)),
                                          trace=trace)
    if trace and res.exec_time_ns is not None:
        print(f"HW exec time: {res.exec_time_ns} ns")
        if res.instructions_and_trace is not None:
            print("trace:", res.instructions_and_trace[1])

    outp = np.empty((B, N, D), np.float32)
    for c in range(NC):
        oT = res.results[c]["out"]
        outp[:, c * NT:(c + 1) * NT, :] = oT.transpose(0, 2, 1)
    return outp



# revision 43
# speedup vs baseline: 1.7568x; 1.0060x over previous
"""Performer (FAVOR+) encoder layer on 8 trn2 NeuronCores.

Sharding: data-parallel over sequence (512 positions per core x 4 batches).
The linear-attention summaries (A = E_k^T v per (batch, head), usum) and the
global key-feature max (via one-hot slots) are combined in ONE packed
AllReduce, overlapped with the Q-side feature compute.

Layout: activations feature-major (xT = [D, tokens]) so every GEMM's
stationary operand is a natural weight slice; per-token reductions and
broadcasts are small PE matmuls (selector / ones / f32r broadcast matmuls).
E_k and v are produced token-major directly by matmuls so the token-
contraction A-matmul needs no transposes.
"""
import os
import numpy as np
import ml_dtypes

B, N, D = 4, 4096, 1024
H, DH = 16, 64
DFF = 4096
M = 64
EPS_KERN = 1e-6
EPS_LN = 1e-6
NC = 8
NT = N // NC                # 512 positions per core per batch
PAIRS = H // 2              # 8 head-pairs
KT_D = D // 128             # 8
MT_FF = DFF // 128          # 32
TT = NT // 128              # 4
DN = 1.0 / np.sqrt(np.sqrt(DH))
DN2H = DN * DN / 2.0


def _emit(nc, tc):
    import concourse.mybir as mybir
    from contextlib import ExitStack
    F32 = mybir.dt.float32
    F32R = mybir.dt.float32r
    BF16 = mybir.dt.bfloat16
    AF = mybir.ActivationFunctionType
    ALU = mybir.AluOpType
    AX = mybir.AxisListType

    dram = lambda name, shape, dt, kind: nc.dram_tensor(name, shape, dt, kind=kind).ap()

    x_bf = dram("x_bf", [B, D, NT], BF16, "ExternalInput")
    x_f32 = dram("x_f32", [B, D, NT], F32, "ExternalInput")
    wqs = dram("wqs", [KT_D, 128, KT_D, 128], BF16, "ExternalInput")
    wks = dram("wks", [KT_D, 128, KT_D, 128], BF16, "ExternalInput")
    wv = dram("wv", [D, D], BF16, "ExternalInput")
    wos = dram("wos", [KT_D, 128, KT_D, 128], BF16, "ExternalInput")
    w1s = dram("w1s", [MT_FF, 128, KT_D, 128], BF16, "ExternalInput")
    w2s = dram("w2s", [KT_D, 128, MT_FF, 128], BF16, "ExternalInput")
    projbd = dram("projbd", [128, 128], BF16, "ExternalInput")
    negselF = dram("negselF", [2, 128], BF16, "ExternalInput")
    sel2 = dram("sel2", [128, 2], BF16, "ExternalInput")
    sel2b = dram("sel2b", [2, 128], F32, "ExternalInput")
    ones128 = dram("ones128", [128, 1], BF16, "ExternalInput")
    ones1x128 = dram("ones1x128", [1, 128], F32, "ExternalInput")
    mean1 = dram("mean1", [128, 1], F32, "ExternalInput")
    headmask2 = dram("headmask2", [128, 2], F32, "ExternalInput")
    epsvB = dram("epsvB", [128, B * D], BF16, "ExternalInput")
    onehot = dram("onehot", [1, NC], F32, "ExternalInput")
    b1c = dram("b1c", [128, MT_FF], F32, "ExternalInput")
    nb1c = dram("nb1c", [128, MT_FF], F32, "ExternalInput")
    b1p1c = dram("b1p1c", [128, MT_FF], F32, "ExternalInput")
    b2adjc = dram("b2adjc", [128, KT_D], F32, "ExternalInput")
    g1c = dram("g1c", [128, KT_D], F32, "ExternalInput")
    be1c = dram("be1c", [128, KT_D], F32, "ExternalInput")
    g2c = dram("g2c", [128, KT_D], F32, "ExternalInput")
    be2c = dram("be2c", [128, KT_D], F32, "ExternalInput")
    out = dram("out", [B, D, NT], F32, "ExternalOutput")

    AC_A = B * PAIRS * 64
    AC_U = B * PAIRS
    AC = AC_A + AC_U + NC

    ctx = ExitStack()
    pconst = ctx.enter_context(tc.tile_pool(name="pconst", bufs=1))
    pstream = ctx.enter_context(tc.tile_pool(name="pstream", bufs=2))
    pw2s = ctx.enter_context(tc.tile_pool(name="pw2s", bufs=2))
    pxa = ctx.enter_context(tc.tile_pool(name="pxa", bufs=1))
    pmt = ctx.enter_context(tc.tile_pool(name="pmt", bufs=4))
    psm = ctx.enter_context(tc.tile_pool(name="psm", bufs=2))
    peq = ctx.enter_context(tc.tile_pool(name="peq", bufs=1))
    pbig = ctx.enter_context(tc.tile_pool(name="pbig", bufs=1))
    pone = ctx.enter_context(tc.tile_pool(name="pone", bufs=1))
    pdram = ctx.enter_context(tc.tile_pool(name="pdram", bufs=1, space="DRAM"))
    PP = ctx.enter_context(tc.tile_pool(name="PP", bufs=4, space="PSUM"))
    PD = ctx.enter_context(tc.tile_pool(name="PD", bufs=2, space="PSUM"))
    PA_ = ctx.enter_context(tc.tile_pool(name="PA", bufs=1, space="PSUM"))
    PR = ctx.enter_context(tc.tile_pool(name="PR", bufs=1, space="PSUM"))

    # ---- constants ----
    # wv shares the big32 slot with epsv/hsb: wv is only read in stage A,
    # epsv only in kv assembly, hsb only from FFN1 onward — disjoint uses.
    wv_sb = pbig.tile([128, KT_D, D], BF16, tag="big32")
    nc.sync.dma_start(wv_sb[:], wv.rearrange("(kt p) m -> p kt m", p=128))
    cAPs = {}
    for name, ap, shape, dt in (
        ("projbd", projbd, [128, 128], BF16), ("negselF", negselF, [2, 128], BF16),
        ("sel2", sel2, [128, 2], BF16), ("sel2b", sel2b, [2, 128], F32),
        ("ones128", ones128, [128, 1], BF16), ("ones1x128", ones1x128, [1, 128], F32),
        ("mean1", mean1, [128, 1], F32), ("headmask2", headmask2, [128, 2], F32),
        ("onehot", onehot, [1, NC], F32), ("b1c", b1c, [128, MT_FF], F32),
        ("nb1c", nb1c, [128, MT_FF], F32),
        ("b1p1c", b1p1c, [128, MT_FF], F32), ("b2adjc", b2adjc, [128, KT_D], F32),
        ("g1c", g1c, [128, KT_D], F32), ("be1c", be1c, [128, KT_D], F32),
        ("g2c", g2c, [128, KT_D], F32), ("be2c", be2c, [128, KT_D], F32),
    ):
        t = pconst.tile(shape, dt, tag=name)
        nc.sync.dma_start(t[:], ap[:])
        cAPs[name] = t
    sel2b_r = pconst.tile([2, 128], F32R, tag="sel2br")
    ones1x128_r = pconst.tile([1, 128], F32R, tag="ones1x128r")
    mean1_r = pconst.tile([128, 1], F32R, tag="mean1r")
    mean1_bf = pconst.tile([128, 1], BF16, tag="mean1bf")
    sel2b_bf = pconst.tile([2, 128], BF16, tag="sel2bbf")
    nc.vector.tensor_copy(sel2b_r[:], cAPs["sel2b"][:])
    nc.vector.tensor_copy(ones1x128_r[:], cAPs["ones1x128"][:])
    nc.vector.tensor_copy(mean1_r[:], cAPs["mean1"][:])
    nc.vector.tensor_copy(mean1_bf[:], cAPs["mean1"][:])
    nc.vector.tensor_copy(sel2b_bf[:], cAPs["sel2b"][:])

    epsln_c = pconst.tile([1, 1], F32, tag="epslnc")
    nc.vector.memset(epsln_c[:], float(EPS_LN))
    onesrow_c = pconst.tile([1, NT], F32, tag="onesrowc")
    nc.vector.memset(onesrow_c[:], 1.0)
    magicrow = pconst.tile([1, NT], mybir.dt.int32, tag="magicrow")
    nc.vector.memset(magicrow[:], 0x5f3759df)
    oneirow = pconst.tile([1, NT], mybir.dt.int32, tag="oneirow")
    nc.vector.memset(oneirow[:], 1)
    arstage = pone.tile([128, AC], F32, tag="arbuf")
    mxcols = pone.tile([128, B * PAIRS], BF16, tag="mxcols")

    def ln_stats(res):
        """Feature-major LN stats for a [128, KT_D, NT] f32r residual.
        rstd = 1/sqrt(var+eps) entirely on the vector engine (bit-magic seed +
        2 Newton steps) — the scalar Ln/Exp pair forced an ACT_TABLE_LOAD per
        LN. Returns (mu f32r row, rstd f32r row)."""
        pm0 = PR.tile([1, NT], F32, tag="prow")
        pm1 = PA_.tile([1, NT], F32, tag="pA")
        for kt in range(KT_D):
            sqt = psm.tile([128, NT], BF16, tag="lnsq")
            nc.scalar.square(sqt[:], res[:, kt, :])
            nc.tensor.matmul(pm0[:], mean1_bf[:], res[:, kt, :],
                             start=kt == 0, stop=kt == KT_D - 1,
                             skip_group_check=True)
            nc.tensor.matmul(pm1[:], mean1_bf[:], sqt[:],
                             start=kt == 0, stop=kt == KT_D - 1,
                             skip_group_check=True)
        mu = psm.tile([1, NT], F32R, tag="lnmu", bufs=1)
        nc.vector.tensor_copy(mu[:], pm0[:])
        muf = mu[:].bitcast(F32)
        mu2 = psm.tile([1, NT], F32, tag="lnrow")
        nc.vector.tensor_tensor(mu2[:], muf, muf, op=ALU.mult)
        var = psm.tile([1, NT], F32, tag="lnrow")
        nc.vector.tensor_tensor(var[:], pm1[:], mu2[:], op=ALU.subtract)
        vare = psm.tile([1, NT], F32, tag="lnvare", bufs=1)
        nc.vector.tensor_scalar(vare[:], var[:], float(EPS_LN), None,
                                op0=ALU.add)
        I32 = mybir.dt.int32
        sh = psm.tile([1, NT], I32, tag="lnrow")
        nc.vector.tensor_tensor(sh[:], vare[:].bitcast(I32), oneirow[:],
                                op=ALU.arith_shift_right)
        y0 = psm.tile([1, NT], I32, tag="lnya", bufs=1)
        nc.vector.tensor_tensor(y0[:], magicrow[:], sh[:], op=ALU.subtract)
        y = y0[:].bitcast(F32)
        for it in range(1):
            t1 = psm.tile([1, NT], F32, tag="lnrow")
            nc.vector.tensor_tensor(t1[:], y, y, op=ALU.mult)
            t2 = psm.tile([1, NT], F32, tag="lnrow")
            nc.vector.tensor_tensor(t2[:], t1[:], vare[:], op=ALU.mult)
            t3 = psm.tile([1, NT], F32, tag="lnrow")
            nc.vector.tensor_scalar(t3[:], t2[:], -0.5, 1.5,
                                    op0=ALU.mult, op1=ALU.add)
            yn = psm.tile([1, NT], F32, tag=f"lny{it}", bufs=1)
            nc.vector.tensor_tensor(yn[:], y, t3[:], op=ALU.mult)
            y = yn[:]
        rstd = psm.tile([1, NT], F32R, tag="lnrstd", bufs=1)
        nc.vector.tensor_copy(rstd[:], y)
        return mu, rstd

    def ln_apply(mu, rstd, res, gc, bc, odt, dma_out=None):
        # pmu/prs live in the PD banks (idle during LN windows) so the PP
        # rotation used by the surrounding matmul streams never stalls on
        # the long apply reads.
        pmu = PD.tile([128, NT], F32, tag="pdd")
        nc.tensor.matmul(pmu[:], ones1x128_r[:], mu[:], start=True, stop=True)
        pmu_bf = psm.tile([128, NT], BF16, tag="lnpmubf", bufs=1)
        nc.scalar.activation(pmu_bf[:], pmu[:], AF.Copy)
        prs = PD.tile([128, NT], F32, tag="pdd")
        nc.tensor.matmul(prs[:], ones1x128_r[:], rstd[:], start=True, stop=True)
        prs_bf = psm.tile([128, NT], BF16, tag="lnprsbf", bufs=1)
        nc.scalar.activation(prs_bf[:], prs[:], AF.Copy)
        o = None
        if odt is not None:
            o = pbig.tile([128, KT_D, NT], odt, tag="bf8")
        cen = psm.tile([128, NT], BF16, tag="lncen")
        nrm = psm.tile([128, NT], BF16, tag="lnnrm")
        for kt in range(KT_D):
            nc.vector.tensor_tensor(cen[:], res[:, kt, :], pmu_bf[:],
                                    op=ALU.subtract)
            nc.vector.scalar_tensor_tensor(nrm[:], cen[:], gc[:, kt:kt + 1],
                                           prs_bf[:], op0=ALU.mult, op1=ALU.mult)
            if o is not None:
                nc.scalar.activation(o[:, kt, :], nrm[:], AF.Identity,
                                     bias=bc[:, kt:kt + 1])
            else:
                ot = psm.tile([128, NT], F32, tag="lnot")
                nc.scalar.activation(ot[:], nrm[:], AF.Identity,
                                     bias=bc[:, kt:kt + 1])
                nc.sync.dma_start(dma_out[0][dma_out[1], kt * 128:(kt + 1) * 128, :],
                                  ot[:])
        return o

    # ================= stage A =================
    for b in range(B):
        xbf = pxa.tile([128, KT_D, NT], BF16, tag="xbf", bufs=2)
        nc.sync.dma_start(xbf[:], x_bf[b].rearrange("(kt p) t -> p kt t", p=128))

        vtok = pxa.tile([128, TT, PAIRS, 129], BF16, tag="vtok", bufs=2)
        nc.vector.memset(vtok[:, :, :, 128:129], 1.0)
        for tt in range(TT):
            for nh in range(2):
                pv = PP.tile([128, 4, 128], F32, tag="pbig")
                for kt in range(KT_D):
                    nc.tensor.matmul(pv[:], xbf[:, kt, tt * 128:(tt + 1) * 128],
                                     wv_sb[:, kt, nh * 512:(nh + 1) * 512],
                                     start=kt == 0, stop=kt == KT_D - 1)
                nc.vector.tensor_copy(vtok[:, tt, nh * 4:(nh + 1) * 4, 0:128],
                                      pv[:])

        # per-pair chain software-pipelined across pairs (depth 2) so the PE
        # never waits on the scalar/vector singletons between sub-phases.
        kTs, ksq2s, Eks = {}, {}, {}

        def s1(pr):  # K projection -> token-major k + k^2 (scalar trails)
            wkmt = pstream.tile([128, KT_D, 128], BF16, tag="wmt")
            nc.sync.dma_start(wkmt[:], wks[pr])
            pk = PP.tile([128, NT], F32, tag="pbig")
            for kt in range(KT_D):
                nc.tensor.matmul(pk[:], wkmt[:, kt, :], xbf[:, kt, :],
                                 start=kt == 0, stop=kt == KT_D - 1)
            kTmt = pmt.tile([128, NT], BF16, tag="mt512")
            nc.scalar.activation(kTmt[:], pk[:], AF.Copy)
            ksqmt = pmt.tile([128, NT], BF16, tag="mt512")
            nc.scalar.square(ksqmt[:], pk[:])
            kTs[pr] = (kTmt, ksqmt)

        def s2(pr):  # squared-norm row + FAVOR features
            kTmt, ksqmt = kTs[pr]
            pks = PR.tile([2, NT], F32, tag="prow")
            nc.tensor.matmul(pks[:], cAPs["sel2"][:], ksqmt[:], start=True,
                             stop=True)
            ksq2 = psm.tile([2, NT], BF16, tag="ksq2")
            nc.scalar.activation(ksq2[:], pks[:], AF.Copy)
            ksq2s[pr] = ksq2

            # One accumulation group for the whole bank: only the FIRST matmul
            # carries start=True (whole-bank has_written clear); later slices
            # overwrite-and-set, negsels then accumulate onto set bits. The
            # raw-projection max is one whole-bank reduce between the phases.
            Ek = psm.tile([128, TT, 128], BF16, tag="Ek")
            pdd = PD.tile([128, NT], F32, tag="pdd")
            sls = [slice(tt * 128, (tt + 1) * 128) for tt in range(TT)]
            for tt in range(TT):
                nc.tensor.matmul(pdd[:, sls[tt]], kTmt[:, sls[tt]],
                                 cAPs["projbd"][:], start=tt == 0, stop=False,
                                 skip_group_check=True)
            c = b * PAIRS + pr
            nc.vector.tensor_reduce(mxcols[:, c:c + 1], pdd[:], axis=AX.X,
                                    op=ALU.max)
            for tt in range(TT):
                nc.tensor.matmul(pdd[:, sls[tt]], ksq2[:, sls[tt]],
                                 cAPs["negselF"][:], start=False,
                                 stop=tt == TT - 1, skip_group_check=True)
            nc.scalar.activation(Ek[:], pdd[:], AF.Exp)
            Eks[pr] = Ek

        def s3(pr):  # token-contraction A matmuls + AR staging
            Ek = Eks[pr]
            pA = PA_.tile([128, 129], F32, tag="pA")
            for tt in range(TT):
                nc.tensor.matmul(pA[:], Ek[:, tt, :], vtok[:, tt, pr, :],
                                 start=tt == 0, stop=tt == TT - 1,
                                 skip_group_check=True)
            j = b * PAIRS + pr
            nc.vector.tensor_copy(arstage[0:64, j * 64:(j + 1) * 64], pA[0:64, 0:64])
            nc.vector.tensor_copy(arstage[64:128, j * 64:(j + 1) * 64],
                                  pA[64:128, 64:128])
            nc.vector.tensor_copy(arstage[:, AC_A + j:AC_A + j + 1], pA[:, 128:129])

        for step in range(PAIRS + 2):
            if step < PAIRS:
                s1(step)
            if 1 <= step <= PAIRS:
                s2(step - 1)
            if step >= 2:
                s3(step - 2)

    # ---- fire AllReduce ----
    mxr = pone.tile([128, 1], F32, tag="mxr")
    nc.vector.tensor_reduce(mxr[:], mxcols[:], axis=AX.X, op=ALU.max)
    mx1 = pone.tile([1, 1], F32, tag="mx1")
    nc.gpsimd.tensor_reduce(mx1[:], mxr[:], axis=AX.C, op=ALU.max)
    nc.vector.tensor_scalar(arstage[0:1, AC_A + AC_U:AC], cAPs["onehot"][:],
                            mx1[:], None, op0=ALU.mult)
    arin = pdram.tile([128, AC], F32, tag="arin")
    arout = pdram.tile([128, AC], F32, tag="arout")
    nc.sync.dma_start(arin[:], arstage[:])
    if os.environ.get("KERNEL_NOCOLL"):
        nc.sync.dma_start(arout[:], arin[:])
    else:
        nc.gpsimd.collective_compute("AllReduce", ALU.add,
                                     replica_groups=[list(range(NC))],
                                     ins=[arin[:]], outs=[arout[:]])
    arres = pone.tile([128, AC], F32, tag="arbuf")
    nc.sync.dma_start(arres[:], arout[:])

    # ================= q-side features (overlap AR) =================
    Eq_all = {}

    def emit_qside(b):
        qxbf = pxa.tile([128, KT_D, NT], BF16, tag="qxbf")
        nc.sync.dma_start(qxbf[:], x_bf[b].rearrange("(kt p) t -> p kt t", p=128))
        Eqs = []
        for pr in range(PAIRS):
            wqmt = pstream.tile([128, KT_D, 128], BF16, tag="wmt")
            nc.sync.dma_start(wqmt[:], wqs[pr])
            pq_ = PP.tile([128, NT], F32, tag="pbig")
            for kt in range(KT_D):
                nc.tensor.matmul(pq_[:], wqmt[:, kt, :], qxbf[:, kt, :],
                                 start=kt == 0, stop=kt == KT_D - 1)
            qTmt = pmt.tile([128, NT], BF16, tag="mt512")
            nc.scalar.activation(qTmt[:], pq_[:], AF.Copy)
            qsqmt = pmt.tile([128, NT], BF16, tag="mt512")
            nc.scalar.square(qsqmt[:], pq_[:])
            pqs = PR.tile([2, NT], F32, tag="prow")
            nc.tensor.matmul(pqs[:], cAPs["sel2"][:], qsqmt[:], start=True, stop=True)
            qsq2 = psm.tile([2, NT], BF16, tag="qsq2")
            nc.scalar.activation(qsq2[:], pqs[:], AF.Copy)

            pdq = PP.tile([128, NT], F32, tag="pbig")
            nc.tensor.matmul(pdq[:], cAPs["projbd"][:], qTmt[:], start=True,
                             stop=False)
            nc.tensor.matmul(pdq[:], cAPs["negselF"][:], qsq2[:], start=False,
                             stop=True, skip_group_check=True)
            Etmp = pmt.tile([128, NT], BF16, tag="t512bf")
            nc.scalar.activation(Etmp[:], pdq[:], AF.Exp)

            pS = PR.tile([2, NT], F32, tag="prow")
            nc.tensor.matmul(pS[:], cAPs["sel2"][:], Etmp[:], start=True, stop=True)
            ediag = psm.tile([2, NT], BF16, tag="ediag")
            nc.scalar.activation(ediag[:], qsq2[:], AF.Exp, scale=float(DN2H))
            wrow = psm.tile([2, NT], BF16, tag="wrow")
            nc.vector.scalar_tensor_tensor(wrow[:], ediag[:], EPS_KERN, pS[:],
                                           op0=ALU.mult, op1=ALU.mult)
            pwB = PP.tile([128, NT], F32, tag="pbig")
            nc.tensor.matmul(pwB[:], sel2b_bf[:], wrow[:], start=True, stop=True)
            Eq = peq.tile([128, NT], BF16, tag=f"Eq{b % 2}_{pr}")
            nc.vector.tensor_tensor(Eq[:], Etmp[:], pwB[:], op=ALU.add)
            Eqs.append(Eq)
        Eq_all[b] = Eqs

    emit_qside(0)
    emit_qside(1)

    # ---- kv / ksum assembly ----
    mx8 = pone.tile([1, 1], F32, tag="mx8")
    nc.vector.tensor_reduce(mx8[:], arres[0:1, AC_A + AC_U:AC], axis=AX.X, op=ALU.max)
    emxf = pone.tile([1, 1], F32, tag="emxf")
    nc.scalar.activation(emxf[:], mx8[:], AF.Exp, scale=-1.0)
    emxrow = psm.tile([1, NT], F32R, tag="lnrow")
    nc.vector.tensor_scalar(emxrow[:], onesrow_c[:], emxf[:], None, op0=ALU.mult)
    pex = PP.tile([128, NT], F32, tag="pbig")
    nc.tensor.matmul(pex[:], ones1x128_r[:], emxrow[:], start=True, stop=True)
    emxc = pone.tile([128, 1], F32, tag="emxc")
    nc.vector.tensor_copy(emxc[:], pex[:, 0:1])

    epsv_sb = pbig.tile([128, B * PAIRS, 128], BF16, tag="big32")
    nc.sync.dma_start(epsv_sb[:], epsvB[:])

    # bulk kv assembly: one strided op per quadrant across all 32 (b, pair)
    # summaries instead of 128 tiny vector ops — this chain gates stage B.
    kvBall = pone.tile([128, B * PAIRS, 130], BF16, tag="kvBall")
    nc.vector.memset(kvBall[:], 0.0)
    arA0 = arres[0:64, 0:AC_A].rearrange("p (j c) -> p j c", j=B * PAIRS)
    nc.vector.scalar_tensor_tensor(kvBall[0:64, :, 0:64], arA0, emxc[0:64, :],
                                   epsv_sb[0:64, :, 0:64],
                                   op0=ALU.mult, op1=ALU.add)
    arA1 = arres[64:128, 0:AC_A].rearrange("p (j c) -> p j c", j=B * PAIRS)
    nc.vector.scalar_tensor_tensor(kvBall[64:128, :, 64:128], arA1,
                                   emxc[64:128, :], epsv_sb[64:128, :, 64:128],
                                   op0=ALU.mult, op1=ALU.add)
    ksfall = pone.tile([128, B * PAIRS], F32, tag="ksfall")
    nc.vector.tensor_scalar(ksfall[:], arres[:, AC_A:AC_A + B * PAIRS],
                            emxc[:], float(EPS_KERN * N),
                            op0=ALU.mult, op1=ALU.add)
    kvB_all = {}
    for j in range(B * PAIRS):
        nc.vector.tensor_scalar(kvBall[:, j, 128:130], cAPs["headmask2"][:],
                                ksfall[:, j:j + 1], None, op0=ALU.mult)
        kvB_all[j] = kvBall[:, j, :]

    # ================= stage B (software-pipelined across batches) ==========
    def attn(b):
        Eqs = Eq_all[b]
        attnT = pbig.tile([128, KT_D, NT], BF16, tag="attnT")
        rdens = {}

        def pass1(pr):
            kvB = kvB_all[b * PAIRS + pr]
            pnum = PP.tile([128, NT], F32, tag="pbig")
            nc.tensor.matmul(pnum[:], kvB[:, 0:128], Eqs[pr][:], start=True,
                             stop=True)
            pden = PD.tile([2, NT], F32, tag="pdd")
            nc.tensor.matmul(pden[:], kvB[:, 128:130], Eqs[pr][:], start=True,
                             stop=True)
            nc.scalar.activation(attnT[:, pr, :], pnum[:], AF.Copy)
            rdf = psm.tile([2, NT], F32, tag="rden", bufs=4)
            nc.vector.reciprocal_approx_fast(rdf[:], pden[:])
            rdens[pr] = rdf

        def pass2(pr):
            prdB = PP.tile([128, NT], F32, tag="pbig")
            nc.tensor.matmul(prdB[:], cAPs["sel2b"][:], rdens[pr][:],
                             start=True, stop=True)
            nc.vector.tensor_tensor(attnT[:, pr, :], attnT[:, pr, :], prdB[:],
                                    op=ALU.mult)

        # lag-2 interleave keeps the PE dense while the reciprocal of pair p
        # finishes during pairs p+1/p+2's matmuls.
        for step in range(PAIRS + 2):
            if step < PAIRS:
                pass1(step)
            if step >= 2:
                pass2(step - 2)
        return attnT

    def wo_res1(b, attnT):
        res1 = pbig.tile([128, KT_D, NT], BF16, tag="resX")
        for mt in range(KT_D):
            womt = pstream.tile([128, KT_D, 128], BF16, tag="wmt")
            nc.sync.dma_start(womt[:], wos[mt])
            po = PP.tile([128, NT], F32, tag="pbig")
            for kt in range(KT_D):
                nc.tensor.matmul(po[:], womt[:, kt, :], attnT[:, kt, :],
                                 start=kt == 0, stop=kt == KT_D - 1)
            xf = psm.tile([128, NT], F32, tag="xf")
            nc.sync.dma_start(xf[:], x_f32[b, mt * 128:(mt + 1) * 128, :])
            nc.vector.tensor_tensor(res1[:, mt, :], xf[:], po[:], op=ALU.add)
        return res1

    def ffn(b, out1):
        hsb = pbig.tile([128, MT_FF, NT], BF16, tag="big32")
        for mt in range(MT_FF):
            w1mt = pstream.tile([128, KT_D, 128], BF16, tag="wmt")
            nc.sync.dma_start(w1mt[:], w1s[mt])
            pz = PP.tile([128, NT], F32, tag="pbig", name="pz")
            for kt in range(KT_D):
                nc.tensor.matmul(pz[:], w1mt[:, kt, :], out1[:, kt, :],
                                 start=kt == 0, stop=kt == KT_D - 1)
            # elu(z')+1 = max(z'+1, min(exp(z'), 1)): one scalar Exp, with the
            # min on the (FFN-idle) vector engine in 16-bit 2x mode.
            eraw = pmt.tile([128, NT], BF16, tag="t512bf")
            nc.scalar.activation(eraw[:], pz[:], AF.Exp,
                                 bias=cAPs["b1c"][:, mt:mt + 1])
            emin = pmt.tile([128, NT], BF16, tag="t512bf")
            nc.vector.tensor_scalar(emin[:], eraw[:], 1.0, None, op0=ALU.min)
            nc.vector.scalar_tensor_tensor(hsb[:, mt, :], pz[:],
                                           cAPs["b1p1c"][:, mt:mt + 1], emin[:],
                                           op0=ALU.add, op1=ALU.max)

        res2 = pbig.tile([128, KT_D, NT], BF16, tag="resX")
        for mt in range(KT_D):
            w2a = pw2s.tile([128, MT_FF // 2, 128], BF16, tag="w2mt")
            nc.sync.dma_start(w2a[:], w2s[mt, :, 0:MT_FF // 2])
            w2b = pw2s.tile([128, MT_FF // 2, 128], BF16, tag="w2mt")
            nc.sync.dma_start(w2b[:], w2s[mt, :, MT_FF // 2:MT_FF])
            pf = PP.tile([128, NT], F32, tag="pbig")
            for kt in range(MT_FF):
                w2h = w2a if kt < MT_FF // 2 else w2b
                nc.tensor.matmul(pf[:], w2h[:, kt % (MT_FF // 2), :],
                                 hsb[:, kt, :],
                                 start=kt == 0, stop=kt == MT_FF - 1)
            nc.vector.scalar_tensor_tensor(res2[:, mt, :], pf[:],
                                           cAPs["b2adjc"][:, mt:mt + 1],
                                           out1[:, mt, :], op0=ALU.add, op1=ALU.add)
        return res2

    res1 = wo_res1(0, attn(0))
    for b in range(B):
        mu1, rstd1 = ln_stats(res1)
        if b + 2 < B:
            # qside PE work covers the LN1 stat chain + apply window
            emit_qside(b + 2)
        out1 = ln_apply(mu1, rstd1, res1, cAPs["g1c"], cAPs["be1c"],
                        mybir.dt.bfloat16)
        res2 = ffn(b, out1)
        mu2, rstd2 = ln_stats(res2)
        if b + 1 < B:
            # next batch's attention matmuls cover the LN2 chain; its Wo
            # stream then runs while LN2 applies and the output DMAs out.
            attnT = attn(b + 1)
        ln_apply(mu2, rstd2, res2, cAPs["g2c"], cAPs["be2c"], None,
                 dma_out=(out, b))
        if b + 1 < B:
            res1 = wo_res1(b + 1, attnT)

    ctx.close()


_CACHE = {}


def _build():
    import concourse.tile as tile
    from concourse import bacc
    nc = bacc.Bacc("TRN2", target_bir_lowering=False, debug=False, num_devices=NC)
    with tile.TileContext(nc) as tc:
        _emit(nc, tc)
    nc.compile()
    return nc


def _host_inputs(x, Wq, Wk, Wv, Wo, proj, W1, b1, W2, b2,
                 ln1_g, ln1_b, ln2_g, ln2_b):
    bf = ml_dtypes.bfloat16
    f32 = np.float32
    d = {}

    def chunked(w):  # [D, X] -> [X/128 mt, 128 p, D/128 kt, 128]
        Dk, X = w.shape
        r = w.reshape(Dk // 128, 128, X // 128, 128)
        return np.ascontiguousarray(r.transpose(2, 1, 0, 3)).astype(bf)

    d["wqs"] = chunked(Wq.reshape(D, D))
    d["wks"] = chunked(Wk.reshape(D, D))
    d["wv"] = np.ascontiguousarray(Wv.reshape(D, D)).astype(bf)
    d["wos"] = chunked(Wo.reshape(D, D))
    d["w1s"] = chunked(W1)
    d["w2s"] = chunked(W2)

    projT_s = (proj * DN).T.astype(f32)
    pbd = np.zeros((128, 128), f32)
    pbd[0:64, 0:64] = projT_s
    pbd[64:128, 64:128] = projT_s
    d["projbd"] = pbd.astype(bf)
    nsF = np.zeros((2, 128), f32)
    nsF[0, 0:64] = -DN2H
    nsF[1, 64:128] = -DN2H
    d["negselF"] = nsF.astype(bf)
    s2 = np.zeros((128, 2), f32)
    s2[0:64, 0] = 1.0
    s2[64:128, 1] = 1.0
    d["sel2"] = s2.astype(bf)
    s2b = np.zeros((2, 128), f32)
    s2b[0, 0:64] = 1.0
    s2b[1, 64:128] = 1.0
    d["sel2b"] = s2b
    d["ones128"] = np.ones((128, 1), f32).astype(bf)
    d["ones1x128"] = np.ones((1, 128), f32)
    d["mean1"] = np.full((128, 1), 1.0 / D, f32)
    hm2 = np.zeros((128, 2), f32)
    hm2[0:64, 0] = 1.0
    hm2[64:128, 1] = 1.0
    d["headmask2"] = hm2

    xsum = x.sum(axis=1, dtype=np.float64)
    vsum = xsum @ Wv.reshape(D, D).astype(np.float64)
    epsv = (EPS_KERN * vsum).astype(f32)
    d["epsvB"] = np.ascontiguousarray(
        np.broadcast_to(epsv.reshape(1, B * D), (128, B * D))).astype(bf)

    d["b1c"] = np.ascontiguousarray(b1.reshape(MT_FF, 128).T).astype(f32)
    d["nb1c"] = np.ascontiguousarray((-b1).reshape(MT_FF, 128).T).astype(f32)
    d["b1p1c"] = np.ascontiguousarray((b1 + 1.0).reshape(MT_FF, 128).T).astype(f32)
    b2adj = b2.astype(np.float64) - W2.astype(np.float64).sum(axis=0)
    d["b2adjc"] = np.ascontiguousarray(b2adj.reshape(KT_D, 128).T).astype(f32)
    d["g1c"] = np.ascontiguousarray(ln1_g.reshape(KT_D, 128).T).astype(f32)
    d["be1c"] = np.ascontiguousarray(ln1_b.reshape(KT_D, 128).T).astype(f32)
    d["g2c"] = np.ascontiguousarray(ln2_g.reshape(KT_D, 128).T).astype(f32)
    d["be2c"] = np.ascontiguousarray(ln2_b.reshape(KT_D, 128).T).astype(f32)
    return d


def kernel(x, Wq, Wk, Wv, Wo, proj, W1, b1, W2, b2, ln1_g, ln1_b, ln2_g, ln2_b):
    from concourse import bass_utils

    x = np.asarray(x, np.float32)
    shared = _host_inputs(x, np.asarray(Wq), np.asarray(Wk), np.asarray(Wv),
                          np.asarray(Wo), np.asarray(proj), np.asarray(W1),
                          np.asarray(b1), np.asarray(W2), np.asarray(b2),
                          np.asarray(ln1_g), np.asarray(ln1_b),
                          np.asarray(ln2_g), np.asarray(ln2_b))

    if "nc" not in _CACHE:
        _CACHE["nc"] = _build()
    nc = _CACHE["nc"]

    in_maps = []
    for c in range(NC):
        xs = x[:, c * NT:(c + 1) * NT, :]
        xT = np.ascontiguousarray(xs.transpose(0, 2, 1))
        oh = np.zeros((1, NC), np.float32)
        oh[0, c] = 1.0
        m = dict(shared)
        m["x_f32"] = xT
        m["x_bf"] = xT.astype(ml_dtypes.bfloat16)
        m["onehot"] = oh
        in_maps.append(m)

    trace = bool(int(os.environ.get("KERNEL_TRACE", "0")))
    res = bass_utils.run_bass_kernel_spmd(nc, in_maps, core_ids=list(range(NC)),
                                          trace=trace)
    if trace and res.exec_time_ns is not None:
        print(f"HW exec time: {res.exec_time_ns} ns")
        if res.instructions_and_trace is not None:
            print("trace:", res.instructions_and_trace[1])

    outp = np.empty((B, N, D), np.float32)
    for c in range(NC):
        oT = res.results[c]["out"]
        outp[:, c * NT:(c + 1) * NT, :] = oT.transpose(0, 2, 1)
    return outp



# revision 47
# speedup vs baseline: 1.9020x; 1.0826x over previous
"""Performer (FAVOR+) encoder layer on 8 trn2 NeuronCores.

Sharding: data-parallel over sequence (512 positions per core x 4 batches).
The linear-attention summaries (A = E_k^T v per (batch, head), usum) and the
global key-feature max (via one-hot slots) are combined in ONE packed
AllReduce, overlapped with the Q-side feature compute.

Layout: activations feature-major (xT = [D, tokens]) so every GEMM's
stationary operand is a natural weight slice; per-token reductions and
broadcasts are small PE matmuls (selector / ones / f32r broadcast matmuls).
E_k and v are produced token-major directly by matmuls so the token-
contraction A-matmul needs no transposes.
"""
import os
import numpy as np
import ml_dtypes

B, N, D = 4, 4096, 1024
H, DH = 16, 64
DFF = 4096
M = 64
EPS_KERN = 1e-6
EPS_LN = 1e-6
NC = 8
NT = N // NC                # 512 positions per core per batch
PAIRS = H // 2              # 8 head-pairs
KT_D = D // 128             # 8
MT_FF = DFF // 128          # 32
TT = NT // 128              # 4
DN = 1.0 / np.sqrt(np.sqrt(DH))
DN2H = DN * DN / 2.0


def _emit(nc, tc):
    import concourse.mybir as mybir
    from contextlib import ExitStack
    F32 = mybir.dt.float32
    F32R = mybir.dt.float32r
    BF16 = mybir.dt.bfloat16
    AF = mybir.ActivationFunctionType
    ALU = mybir.AluOpType
    AX = mybir.AxisListType

    dram = lambda name, shape, dt, kind: nc.dram_tensor(name, shape, dt, kind=kind).ap()

    x_bf = dram("x_bf", [B, D, NT], BF16, "ExternalInput")
    x_f32 = dram("x_f32", [B, D, NT], F32, "ExternalInput")
    wqs = dram("wqs", [KT_D, 128, KT_D, 128], BF16, "ExternalInput")
    wks = dram("wks", [KT_D, 128, KT_D, 128], BF16, "ExternalInput")
    wv = dram("wv", [D, D], BF16, "ExternalInput")
    wos = dram("wos", [KT_D, 128, KT_D, 128], BF16, "ExternalInput")
    w1s = dram("w1s", [MT_FF, 128, KT_D, 128], BF16, "ExternalInput")
    w2s = dram("w2s", [KT_D, 128, MT_FF, 128], BF16, "ExternalInput")
    projbd = dram("projbd", [128, 128], BF16, "ExternalInput")
    negselF = dram("negselF", [2, 128], BF16, "ExternalInput")
    sel2 = dram("sel2", [128, 2], BF16, "ExternalInput")
    sel2b = dram("sel2b", [2, 128], F32, "ExternalInput")
    ones128 = dram("ones128", [128, 1], BF16, "ExternalInput")
    ones1x128 = dram("ones1x128", [1, 128], F32, "ExternalInput")
    mean1 = dram("mean1", [128, 1], F32, "ExternalInput")
    headmask2 = dram("headmask2", [128, 2], F32, "ExternalInput")
    epsvB = dram("epsvB", [128, B * D], BF16, "ExternalInput")
    onehot = dram("onehot", [1, NC], F32, "ExternalInput")
    b1c = dram("b1c", [128, MT_FF], F32, "ExternalInput")
    nb1c = dram("nb1c", [128, MT_FF], F32, "ExternalInput")
    b1p1c = dram("b1p1c", [128, MT_FF], F32, "ExternalInput")
    b2adjc = dram("b2adjc", [128, KT_D], F32, "ExternalInput")
    g1c = dram("g1c", [128, KT_D], F32, "ExternalInput")
    be1c = dram("be1c", [128, KT_D], F32, "ExternalInput")
    g2c = dram("g2c", [128, KT_D], F32, "ExternalInput")
    be2c = dram("be2c", [128, KT_D], F32, "ExternalInput")
    out = dram("out", [B, D, NT], F32, "ExternalOutput")

    AC_A = B * PAIRS * 64
    AC_U = B * PAIRS
    AC = AC_A + AC_U + NC

    ctx = ExitStack()
    pconst = ctx.enter_context(tc.tile_pool(name="pconst", bufs=1))
    pstream = ctx.enter_context(tc.tile_pool(name="pstream", bufs=2))
    pw2s = ctx.enter_context(tc.tile_pool(name="pw2s", bufs=2))
    pxa = ctx.enter_context(tc.tile_pool(name="pxa", bufs=1))
    pmt = ctx.enter_context(tc.tile_pool(name="pmt", bufs=4))
    psm = ctx.enter_context(tc.tile_pool(name="psm", bufs=2))
    peq = ctx.enter_context(tc.tile_pool(name="peq", bufs=1))
    pbig = ctx.enter_context(tc.tile_pool(name="pbig", bufs=1))
    pone = ctx.enter_context(tc.tile_pool(name="pone", bufs=1))
    pdram = ctx.enter_context(tc.tile_pool(name="pdram", bufs=1, space="DRAM"))
    PP = ctx.enter_context(tc.tile_pool(name="PP", bufs=4, space="PSUM"))
    PD = ctx.enter_context(tc.tile_pool(name="PD", bufs=2, space="PSUM"))
    PA_ = ctx.enter_context(tc.tile_pool(name="PA", bufs=1, space="PSUM"))
    PR = ctx.enter_context(tc.tile_pool(name="PR", bufs=1, space="PSUM"))

    # ---- constants ----
    # wv shares the big32 slot with epsv/hsb: wv is only read in stage A,
    # epsv only in kv assembly, hsb only from FFN1 onward — disjoint uses.
    wv_sb = pbig.tile([128, KT_D, D], BF16, tag="big32")
    nc.sync.dma_start(wv_sb[:], wv.rearrange("(kt p) m -> p kt m", p=128))
    cAPs = {}
    for name, ap, shape, dt in (
        ("projbd", projbd, [128, 128], BF16), ("negselF", negselF, [2, 128], BF16),
        ("sel2", sel2, [128, 2], BF16), ("sel2b", sel2b, [2, 128], F32),
        ("ones128", ones128, [128, 1], BF16), ("ones1x128", ones1x128, [1, 128], F32),
        ("mean1", mean1, [128, 1], F32), ("headmask2", headmask2, [128, 2], F32),
        ("onehot", onehot, [1, NC], F32), ("b1c", b1c, [128, MT_FF], F32),
        ("nb1c", nb1c, [128, MT_FF], F32),
        ("b1p1c", b1p1c, [128, MT_FF], F32), ("b2adjc", b2adjc, [128, KT_D], F32),
        ("g1c", g1c, [128, KT_D], F32), ("be1c", be1c, [128, KT_D], F32),
        ("g2c", g2c, [128, KT_D], F32), ("be2c", be2c, [128, KT_D], F32),
    ):
        t = pconst.tile(shape, dt, tag=name)
        nc.sync.dma_start(t[:], ap[:])
        cAPs[name] = t
    sel2b_r = pconst.tile([2, 128], F32R, tag="sel2br")
    ones1x128_r = pconst.tile([1, 128], F32R, tag="ones1x128r")
    mean1_r = pconst.tile([128, 1], F32R, tag="mean1r")
    mean1_bf = pconst.tile([128, 1], BF16, tag="mean1bf")
    sel2b_bf = pconst.tile([2, 128], BF16, tag="sel2bbf")
    nc.vector.tensor_copy(sel2b_r[:], cAPs["sel2b"][:])
    nc.vector.tensor_copy(ones1x128_r[:], cAPs["ones1x128"][:])
    nc.vector.tensor_copy(mean1_r[:], cAPs["mean1"][:])
    nc.vector.tensor_copy(mean1_bf[:], cAPs["mean1"][:])
    nc.vector.tensor_copy(sel2b_bf[:], cAPs["sel2b"][:])

    epsln_c = pconst.tile([1, 1], F32, tag="epslnc")
    nc.vector.memset(epsln_c[:], float(EPS_LN))
    onesrow_c = pconst.tile([1, NT], F32, tag="onesrowc")
    nc.vector.memset(onesrow_c[:], 1.0)
    magicrow = pconst.tile([1, NT], mybir.dt.int32, tag="magicrow")
    nc.vector.memset(magicrow[:], 0x5f3759df)
    oneirow = pconst.tile([1, NT], mybir.dt.int32, tag="oneirow")
    nc.vector.memset(oneirow[:], 1)
    arstage = pone.tile([128, AC], F32, tag="arbuf")
    mxcols = pone.tile([128, B * PAIRS], BF16, tag="mxcols")

    def ln_stats(res):
        """Feature-major LN stats for a [128, KT_D, NT] f32r residual.
        rstd = 1/sqrt(var+eps) entirely on the vector engine (bit-magic seed +
        2 Newton steps) — the scalar Ln/Exp pair forced an ACT_TABLE_LOAD per
        LN. Returns (mu f32r row, rstd f32r row)."""
        pm0 = PR.tile([1, NT], F32, tag="prow")
        pm1 = PA_.tile([1, NT], F32, tag="pA")
        for kt in range(KT_D):
            sqt = psm.tile([128, NT], BF16, tag="lnsq")
            nc.scalar.square(sqt[:], res[:, kt, :])
            nc.tensor.matmul(pm0[:], mean1_bf[:], res[:, kt, :],
                             start=kt == 0, stop=kt == KT_D - 1,
                             skip_group_check=True)
            nc.tensor.matmul(pm1[:], mean1_bf[:], sqt[:],
                             start=kt == 0, stop=kt == KT_D - 1,
                             skip_group_check=True)
        mu = psm.tile([1, NT], F32R, tag="lnmu", bufs=1)
        nc.vector.tensor_copy(mu[:], pm0[:])
        muf = mu[:].bitcast(F32)
        mu2 = psm.tile([1, NT], F32, tag="lnrow")
        nc.vector.tensor_tensor(mu2[:], muf, muf, op=ALU.mult)
        var = psm.tile([1, NT], F32, tag="lnrow")
        nc.vector.tensor_tensor(var[:], pm1[:], mu2[:], op=ALU.subtract)
        vare = psm.tile([1, NT], F32, tag="lnvare", bufs=1)
        nc.vector.tensor_scalar(vare[:], var[:], float(EPS_LN), None,
                                op0=ALU.add)
        I32 = mybir.dt.int32
        sh = psm.tile([1, NT], I32, tag="lnrow")
        nc.vector.tensor_tensor(sh[:], vare[:].bitcast(I32), oneirow[:],
                                op=ALU.arith_shift_right)
        y0 = psm.tile([1, NT], I32, tag="lnya", bufs=1)
        nc.vector.tensor_tensor(y0[:], magicrow[:], sh[:], op=ALU.subtract)
        y = y0[:].bitcast(F32)
        for it in range(1):
            t1 = psm.tile([1, NT], F32, tag="lnrow")
            nc.vector.tensor_tensor(t1[:], y, y, op=ALU.mult)
            t2 = psm.tile([1, NT], F32, tag="lnrow")
            nc.vector.tensor_tensor(t2[:], t1[:], vare[:], op=ALU.mult)
            t3 = psm.tile([1, NT], F32, tag="lnrow")
            nc.vector.tensor_scalar(t3[:], t2[:], -0.5, 1.5,
                                    op0=ALU.mult, op1=ALU.add)
            yn = psm.tile([1, NT], F32, tag=f"lny{it}", bufs=1)
            nc.vector.tensor_tensor(yn[:], y, t3[:], op=ALU.mult)
            y = yn[:]
        rstd = psm.tile([1, NT], F32R, tag="lnrstd", bufs=1)
        nc.vector.tensor_copy(rstd[:], y)
        return mu, rstd

    def ln_apply(mu, rstd, res, gc, bc, odt, dma_out=None):
        # pmu/prs live in the PD banks (idle during LN windows) so the PP
        # rotation used by the surrounding matmul streams never stalls on
        # the long apply reads.
        pmu = PD.tile([128, NT], F32, tag="pdd")
        nc.tensor.matmul(pmu[:], ones1x128_r[:], mu[:], start=True, stop=True)
        pmu_bf = psm.tile([128, NT], BF16, tag="lnpmubf", bufs=1)
        nc.scalar.activation(pmu_bf[:], pmu[:], AF.Copy)
        prs = PD.tile([128, NT], F32, tag="pdd")
        nc.tensor.matmul(prs[:], ones1x128_r[:], rstd[:], start=True, stop=True)
        prs_bf = psm.tile([128, NT], BF16, tag="lnprsbf", bufs=1)
        nc.scalar.activation(prs_bf[:], prs[:], AF.Copy)
        o = None
        if odt is not None:
            o = pbig.tile([128, KT_D, NT], odt, tag="bf8")
        cen = psm.tile([128, NT], BF16, tag="lncen")
        nrm = psm.tile([128, NT], BF16, tag="lnnrm")
        for kt in range(KT_D):
            nc.vector.tensor_tensor(cen[:], res[:, kt, :], pmu_bf[:],
                                    op=ALU.subtract)
            nc.vector.scalar_tensor_tensor(nrm[:], cen[:], gc[:, kt:kt + 1],
                                           prs_bf[:], op0=ALU.mult, op1=ALU.mult)
            if o is not None:
                nc.scalar.activation(o[:, kt, :], nrm[:], AF.Identity,
                                     bias=bc[:, kt:kt + 1])
            else:
                ot = psm.tile([128, NT], F32, tag="lnot")
                nc.scalar.activation(ot[:], nrm[:], AF.Identity,
                                     bias=bc[:, kt:kt + 1])
                nc.sync.dma_start(dma_out[0][dma_out[1], kt * 128:(kt + 1) * 128, :],
                                  ot[:])
        return o

    # ================= stage A =================
    for b in range(B):
        xbf = pxa.tile([128, KT_D, NT], BF16, tag="xbf", bufs=2)
        nc.sync.dma_start(xbf[:], x_bf[b].rearrange("(kt p) t -> p kt t", p=128))

        vtok = pxa.tile([128, TT, PAIRS, 129], BF16, tag="vtok", bufs=1)
        nc.vector.memset(vtok[:, :, :, 128:129], 1.0)
        for tt in range(TT):
            for nh in range(2):
                pv = PP.tile([128, 4, 128], F32, tag="pbig")
                for kt in range(KT_D):
                    nc.tensor.matmul(pv[:], xbf[:, kt, tt * 128:(tt + 1) * 128],
                                     wv_sb[:, kt, nh * 512:(nh + 1) * 512],
                                     start=kt == 0, stop=kt == KT_D - 1)
                nc.vector.tensor_copy(vtok[:, tt, nh * 4:(nh + 1) * 4, 0:128],
                                      pv[:])

        # per-pair chain software-pipelined across pairs (depth 2) so the PE
        # never waits on the scalar/vector singletons between sub-phases.
        kTs, ksq2s, Eks = {}, {}, {}

        def s1(pr):  # K projection -> token-major k + k^2 (scalar trails)
            wkmt = pstream.tile([128, KT_D, 128], BF16, tag="wmt")
            nc.sync.dma_start(wkmt[:], wks[pr])
            pk = PP.tile([128, NT], F32, tag="pbig")
            for kt in range(KT_D):
                nc.tensor.matmul(pk[:], wkmt[:, kt, :], xbf[:, kt, :],
                                 start=kt == 0, stop=kt == KT_D - 1)
            kTmt = pmt.tile([128, NT], BF16, tag="mt512", bufs=6)
            nc.scalar.activation(kTmt[:], pk[:], AF.Copy)
            ksqmt = pmt.tile([128, NT], BF16, tag="mt512", bufs=6)
            nc.scalar.square(ksqmt[:], pk[:])
            kTs[pr] = (kTmt, ksqmt)

        def s2(pr):  # squared-norm row + FAVOR features
            kTmt, ksqmt = kTs[pr]
            pks = PR.tile([2, NT], F32, tag="prow")
            nc.tensor.matmul(pks[:], cAPs["sel2"][:], ksqmt[:], start=True,
                             stop=True)
            ksq2 = psm.tile([2, NT], BF16, tag="ksq2")
            nc.scalar.activation(ksq2[:], pks[:], AF.Copy)
            ksq2s[pr] = ksq2

            # One accumulation group for the whole bank: only the FIRST matmul
            # carries start=True (whole-bank has_written clear); later slices
            # overwrite-and-set, negsels then accumulate onto set bits. The
            # raw-projection max is one whole-bank reduce between the phases.
            Ek = psm.tile([128, TT, 128], BF16, tag="Ek")
            pdd = PD.tile([128, NT], F32, tag="pdd")
            sls = [slice(tt * 128, (tt + 1) * 128) for tt in range(TT)]
            for tt in range(TT):
                nc.tensor.matmul(pdd[:, sls[tt]], kTmt[:, sls[tt]],
                                 cAPs["projbd"][:], start=tt == 0, stop=False,
                                 skip_group_check=True)
            c = b * PAIRS + pr
            nc.vector.tensor_reduce(mxcols[:, c:c + 1], pdd[:], axis=AX.X,
                                    op=ALU.max)
            for tt in range(TT):
                nc.tensor.matmul(pdd[:, sls[tt]], ksq2[:, sls[tt]],
                                 cAPs["negselF"][:], start=False,
                                 stop=tt == TT - 1, skip_group_check=True)
            nc.scalar.activation(Ek[:], pdd[:], AF.Exp)
            Eks[pr] = Ek

        def s3(pr):  # token-contraction A matmuls + AR staging
            Ek = Eks[pr]
            pA = PA_.tile([128, 129], F32, tag="pA")
            for tt in range(TT):
                nc.tensor.matmul(pA[:], Ek[:, tt, :], vtok[:, tt, pr, :],
                                 start=tt == 0, stop=tt == TT - 1,
                                 skip_group_check=True)
            j = b * PAIRS + pr
            nc.vector.tensor_copy(arstage[0:64, j * 64:(j + 1) * 64], pA[0:64, 0:64])
            nc.vector.tensor_copy(arstage[64:128, j * 64:(j + 1) * 64],
                                  pA[64:128, 64:128])
            nc.vector.tensor_copy(arstage[:, AC_A + j:AC_A + j + 1], pA[:, 128:129])

        for step in range(PAIRS + 2):
            if step < PAIRS:
                s1(step)
            if 1 <= step <= PAIRS:
                s2(step - 1)
            if step >= 2:
                s3(step - 2)

    # ---- fire AllReduce ----
    mxr = pone.tile([128, 1], F32, tag="mxr")
    nc.vector.tensor_reduce(mxr[:], mxcols[:], axis=AX.X, op=ALU.max)
    mx1 = pone.tile([1, 1], F32, tag="mx1")
    nc.gpsimd.tensor_reduce(mx1[:], mxr[:], axis=AX.C, op=ALU.max)
    nc.vector.tensor_scalar(arstage[0:1, AC_A + AC_U:AC], cAPs["onehot"][:],
                            mx1[:], None, op0=ALU.mult)
    arin = pdram.tile([128, AC], F32, tag="arin")
    arout = pdram.tile([128, AC], F32, tag="arout")
    nc.sync.dma_start(arin[:], arstage[:])
    if os.environ.get("KERNEL_NOCOLL"):
        nc.sync.dma_start(arout[:], arin[:])
    else:
        nc.gpsimd.collective_compute("AllReduce", ALU.add,
                                     replica_groups=[list(range(NC))],
                                     ins=[arin[:]], outs=[arout[:]])
    arres = pone.tile([128, AC], F32, tag="arbuf")
    nc.sync.dma_start(arres[:], arout[:])

    # ================= q-side features (overlap AR) =================
    Eq_all = {}

    def emit_qside(b):
        """Q-side features, software-pipelined across pairs (5 sub-stages,
        one-step lags) so the PE never waits on the scalar/vector singletons."""
        qxbf = pxa.tile([128, KT_D, NT], BF16, tag="qxbf", bufs=1)
        nc.sync.dma_start(qxbf[:], x_bf[b].rearrange("(kt p) t -> p kt t", p=128))
        Eqs = [None] * PAIRS
        Eq_all[b] = Eqs
        st = {}

        def q1(pr):
            wqmt = pstream.tile([128, KT_D, 128], BF16, tag="wmt")
            nc.sync.dma_start(wqmt[:], wqs[pr])
            pq_ = PP.tile([128, NT], F32, tag="pbig")
            for kt in range(KT_D):
                nc.tensor.matmul(pq_[:], wqmt[:, kt, :], qxbf[:, kt, :],
                                 start=kt == 0, stop=kt == KT_D - 1)
            qTmt = pmt.tile([128, NT], BF16, tag="mt512", bufs=6)
            nc.scalar.activation(qTmt[:], pq_[:], AF.Copy)
            qsqmt = pmt.tile([128, NT], BF16, tag="mt512", bufs=6)
            nc.scalar.square(qsqmt[:], pq_[:])
            st[pr] = [qTmt, qsqmt]

        def q2(pr):
            pqs = PR.tile([2, NT], F32, tag="prow")
            nc.tensor.matmul(pqs[:], cAPs["sel2"][:], st[pr][1][:], start=True,
                             stop=True)
            qsq2 = psm.tile([2, NT], BF16, tag="qsq2")
            nc.scalar.activation(qsq2[:], pqs[:], AF.Copy)
            st[pr].append(qsq2)

        def q3(pr):
            qTmt, _, qsq2 = st[pr]
            pdq = PP.tile([128, NT], F32, tag="pbig")
            nc.tensor.matmul(pdq[:], cAPs["projbd"][:], qTmt[:], start=True,
                             stop=False)
            nc.tensor.matmul(pdq[:], cAPs["negselF"][:], qsq2[:], start=False,
                             stop=True, skip_group_check=True)
            Etmp = pmt.tile([128, NT], BF16, tag="t512bf")
            nc.scalar.activation(Etmp[:], pdq[:], AF.Exp)
            ediag = psm.tile([2, NT], BF16, tag="ediag")
            nc.scalar.activation(ediag[:], qsq2[:], AF.Exp, scale=float(DN2H))
            st[pr] += [Etmp, ediag]

        def q4(pr):
            _, _, _, Etmp, ediag = st[pr]
            pS = PR.tile([2, NT], F32, tag="prow")
            nc.tensor.matmul(pS[:], cAPs["sel2"][:], Etmp[:], start=True,
                             stop=True)
            wrow = psm.tile([2, NT], BF16, tag="wrow")
            nc.vector.scalar_tensor_tensor(wrow[:], ediag[:], EPS_KERN, pS[:],
                                           op0=ALU.mult, op1=ALU.mult)
            st[pr].append(wrow)

        def q5(pr):
            Etmp, wrow = st[pr][3], st[pr][5]
            pwB = PP.tile([128, NT], F32, tag="pbig")
            nc.tensor.matmul(pwB[:], sel2b_bf[:], wrow[:], start=True, stop=True)
            Eq = peq.tile([128, NT], BF16, tag=f"Eq{b % 2}_{pr}")
            nc.vector.tensor_tensor(Eq[:], Etmp[:], pwB[:], op=ALU.add)
            Eqs[pr] = Eq

        for step in range(PAIRS + 4):
            if step < PAIRS:
                q1(step)
            if 1 <= step <= PAIRS:
                q2(step - 1)
            if 2 <= step <= PAIRS + 1:
                q3(step - 2)
            if 3 <= step <= PAIRS + 2:
                q4(step - 3)
            if step >= 4:
                q5(step - 4)

    emit_qside(0)
    emit_qside(1)

    # ---- kv / ksum assembly ----
    mx8 = pone.tile([1, 1], F32, tag="mx8")
    nc.vector.tensor_reduce(mx8[:], arres[0:1, AC_A + AC_U:AC], axis=AX.X, op=ALU.max)
    emxf = pone.tile([1, 1], F32, tag="emxf")
    nc.scalar.activation(emxf[:], mx8[:], AF.Exp, scale=-1.0)
    emxrow = psm.tile([1, NT], F32R, tag="lnrow")
    nc.vector.tensor_scalar(emxrow[:], onesrow_c[:], emxf[:], None, op0=ALU.mult)
    pex = PP.tile([128, NT], F32, tag="pbig")
    nc.tensor.matmul(pex[:], ones1x128_r[:], emxrow[:], start=True, stop=True)
    emxc = pone.tile([128, 1], F32, tag="emxc")
    nc.vector.tensor_copy(emxc[:], pex[:, 0:1])

    epsv_sb = pbig.tile([128, B * PAIRS, 128], BF16, tag="big32")
    nc.sync.dma_start(epsv_sb[:], epsvB[:])

    # bulk kv assembly: one strided op per quadrant across all 32 (b, pair)
    # summaries instead of 128 tiny vector ops — this chain gates stage B.
    kvBall = pone.tile([128, B * PAIRS, 130], BF16, tag="kvBall")
    nc.vector.memset(kvBall[:], 0.0)
    arA0 = arres[0:64, 0:AC_A].rearrange("p (j c) -> p j c", j=B * PAIRS)
    nc.vector.scalar_tensor_tensor(kvBall[0:64, :, 0:64], arA0, emxc[0:64, :],
                                   epsv_sb[0:64, :, 0:64],
                                   op0=ALU.mult, op1=ALU.add)
    arA1 = arres[64:128, 0:AC_A].rearrange("p (j c) -> p j c", j=B * PAIRS)
    nc.vector.scalar_tensor_tensor(kvBall[64:128, :, 64:128], arA1,
                                   emxc[64:128, :], epsv_sb[64:128, :, 64:128],
                                   op0=ALU.mult, op1=ALU.add)
    ksfall = pone.tile([128, B * PAIRS], F32, tag="ksfall")
    nc.vector.tensor_scalar(ksfall[:], arres[:, AC_A:AC_A + B * PAIRS],
                            emxc[:], float(EPS_KERN * N),
                            op0=ALU.mult, op1=ALU.add)
    kvB_all = {}
    for j in range(B * PAIRS):
        nc.vector.tensor_scalar(kvBall[:, j, 128:130], cAPs["headmask2"][:],
                                ksfall[:, j:j + 1], None, op0=ALU.mult)
        kvB_all[j] = kvBall[:, j, :]

    # ================= stage B (software-pipelined across batches) ==========
    def attn(b):
        Eqs = Eq_all[b]
        attnT = pbig.tile([128, KT_D, NT], BF16, tag="attnT")
        rdens = {}

        def pass1(pr):
            kvB = kvB_all[b * PAIRS + pr]
            pnum = PP.tile([128, NT], F32, tag="pbig")
            nc.tensor.matmul(pnum[:], kvB[:, 0:128], Eqs[pr][:], start=True,
                             stop=True)
            pden = PD.tile([2, NT], F32, tag="pdd")
            nc.tensor.matmul(pden[:], kvB[:, 128:130], Eqs[pr][:], start=True,
                             stop=True)
            nc.scalar.activation(attnT[:, pr, :], pnum[:], AF.Copy)
            rdf = psm.tile([2, NT], F32, tag="rden", bufs=4)
            nc.vector.reciprocal_approx_fast(rdf[:], pden[:])
            rdens[pr] = rdf

        def pass2(pr):
            prdB = PP.tile([128, NT], F32, tag="pbig")
            nc.tensor.matmul(prdB[:], cAPs["sel2b"][:], rdens[pr][:],
                             start=True, stop=True)
            nc.vector.tensor_tensor(attnT[:, pr, :], attnT[:, pr, :], prdB[:],
                                    op=ALU.mult)

        # lag-2 interleave keeps the PE dense while the reciprocal of pair p
        # finishes during pairs p+1/p+2's matmuls.
        for step in range(PAIRS + 2):
            if step < PAIRS:
                pass1(step)
            if step >= 2:
                pass2(step - 2)
        return attnT

    def wo_res1(b, attnT):
        res1 = pbig.tile([128, KT_D, NT], BF16, tag="resX")
        for mt in range(KT_D):
            womt = pstream.tile([128, KT_D, 128], BF16, tag="wmt")
            nc.sync.dma_start(womt[:], wos[mt])
            po = PP.tile([128, NT], F32, tag="pbig")
            for kt in range(KT_D):
                nc.tensor.matmul(po[:], womt[:, kt, :], attnT[:, kt, :],
                                 start=kt == 0, stop=kt == KT_D - 1)
            xf = psm.tile([128, NT], F32, tag="xf")
            nc.sync.dma_start(xf[:], x_f32[b, mt * 128:(mt + 1) * 128, :])
            nc.vector.tensor_tensor(res1[:, mt, :], xf[:], po[:], op=ALU.add)
        return res1

    def ffn(b, out1):
        hsb = pbig.tile([128, MT_FF, NT], BF16, tag="big32")
        for mt in range(MT_FF):
            w1mt = pstream.tile([128, KT_D, 128], BF16, tag="wmt")
            nc.sync.dma_start(w1mt[:], w1s[mt])
            pz = PP.tile([128, NT], F32, tag="pbig", name="pz")
            for kt in range(KT_D):
                nc.tensor.matmul(pz[:], w1mt[:, kt, :], out1[:, kt, :],
                                 start=kt == 0, stop=kt == KT_D - 1)
            # elu(z')+1 = max(z'+1, min(exp(z'), 1)): one scalar Exp, with the
            # min on the (FFN-idle) vector engine in 16-bit 2x mode.
            eraw = pmt.tile([128, NT], BF16, tag="t512bf")
            nc.scalar.activation(eraw[:], pz[:], AF.Exp,
                                 bias=cAPs["b1c"][:, mt:mt + 1])
            emin = pmt.tile([128, NT], BF16, tag="t512bf")
            nc.vector.tensor_scalar(emin[:], eraw[:], 1.0, None, op0=ALU.min)
            nc.vector.scalar_tensor_tensor(hsb[:, mt, :], pz[:],
                                           cAPs["b1p1c"][:, mt:mt + 1], emin[:],
                                           op0=ALU.add, op1=ALU.max)

        res2 = pbig.tile([128, KT_D, NT], BF16, tag="resX")
        for mt in range(KT_D):
            w2a = pw2s.tile([128, MT_FF // 2, 128], BF16, tag="w2mt")
            nc.sync.dma_start(w2a[:], w2s[mt, :, 0:MT_FF // 2])
            w2b = pw2s.tile([128, MT_FF // 2, 128], BF16, tag="w2mt")
            nc.sync.dma_start(w2b[:], w2s[mt, :, MT_FF // 2:MT_FF])
            pf = PP.tile([128, NT], F32, tag="pbig")
            for kt in range(MT_FF):
                w2h = w2a if kt < MT_FF // 2 else w2b
                nc.tensor.matmul(pf[:], w2h[:, kt % (MT_FF // 2), :],
                                 hsb[:, kt, :],
                                 start=kt == 0, stop=kt == MT_FF - 1)
            nc.vector.scalar_tensor_tensor(res2[:, mt, :], pf[:],
                                           cAPs["b2adjc"][:, mt:mt + 1],
                                           out1[:, mt, :], op0=ALU.add, op1=ALU.add)
        return res2

    res1 = wo_res1(0, attn(0))
    for b in range(B):
        mu1, rstd1 = ln_stats(res1)
        if b + 2 < B:
            # qside PE work covers the LN1 stat chain + apply window
            emit_qside(b + 2)
        out1 = ln_apply(mu1, rstd1, res1, cAPs["g1c"], cAPs["be1c"],
                        mybir.dt.bfloat16)
        res2 = ffn(b, out1)
        mu2, rstd2 = ln_stats(res2)
        if b + 1 < B:
            # next batch's attention matmuls cover the LN2 chain; its Wo
            # stream then runs while LN2 applies and the output DMAs out.
            attnT = attn(b + 1)
        ln_apply(mu2, rstd2, res2, cAPs["g2c"], cAPs["be2c"], None,
                 dma_out=(out, b))
        if b + 1 < B:
            res1 = wo_res1(b + 1, attnT)

    ctx.close()


_CACHE = {}


def _build():
    import concourse.tile as tile
    from concourse import bacc
    nc = bacc.Bacc("TRN2", target_bir_lowering=False, debug=False, num_devices=NC)
    with tile.TileContext(nc) as tc:
        _emit(nc, tc)
    nc.compile()
    return nc


def _host_inputs(x, Wq, Wk, Wv, Wo, proj, W1, b1, W2, b2,
                 ln1_g, ln1_b, ln2_g, ln2_b):
    bf = ml_dtypes.bfloat16
    f32 = np.float32
    d = {}

    def chunked(w):  # [D, X] -> [X/128 mt, 128 p, D/128 kt, 128]
        Dk, X = w.shape
        r = w.reshape(Dk // 128, 128, X // 128, 128)
        return np.ascontiguousarray(r.transpose(2, 1, 0, 3)).astype(bf)

    d["wqs"] = chunked(Wq.reshape(D, D))
    d["wks"] = chunked(Wk.reshape(D, D))
    d["wv"] = np.ascontiguousarray(Wv.reshape(D, D)).astype(bf)
    d["wos"] = chunked(Wo.reshape(D, D))
    d["w1s"] = chunked(W1)
    d["w2s"] = chunked(W2)

    projT_s = (proj * DN).T.astype(f32)
    pbd = np.zeros((128, 128), f32)
    pbd[0:64, 0:64] = projT_s
    pbd[64:128, 64:128] = projT_s
    d["projbd"] = pbd.astype(bf)
    nsF = np.zeros((2, 128), f32)
    nsF[0, 0:64] = -DN2H
    nsF[1, 64:128] = -DN2H
    d["negselF"] = nsF.astype(bf)
    s2 = np.zeros((128, 2), f32)
    s2[0:64, 0] = 1.0
    s2[64:128, 1] = 1.0
    d["sel2"] = s2.astype(bf)
    s2b = np.zeros((2, 128), f32)
    s2b[0, 0:64] = 1.0
    s2b[1, 64:128] = 1.0
    d["sel2b"] = s2b
    d["ones128"] = np.ones((128, 1), f32).astype(bf)
    d["ones1x128"] = np.ones((1, 128), f32)
    d["mean1"] = np.full((128, 1), 1.0 / D, f32)
    hm2 = np.zeros((128, 2), f32)
    hm2[0:64, 0] = 1.0
    hm2[64:128, 1] = 1.0
    d["headmask2"] = hm2

    xsum = x.sum(axis=1, dtype=np.float64)
    vsum = xsum @ Wv.reshape(D, D).astype(np.float64)
    epsv = (EPS_KERN * vsum).astype(f32)
    d["epsvB"] = np.ascontiguousarray(
        np.broadcast_to(epsv.reshape(1, B * D), (128, B * D))).astype(bf)

    d["b1c"] = np.ascontiguousarray(b1.reshape(MT_FF, 128).T).astype(f32)
    d["nb1c"] = np.ascontiguousarray((-b1).reshape(MT_FF, 128).T).astype(f32)
    d["b1p1c"] = np.ascontiguousarray((b1 + 1.0).reshape(MT_FF, 128).T).astype(f32)
    b2adj = b2.astype(np.float64) - W2.astype(np.float64).sum(axis=0)
    d["b2adjc"] = np.ascontiguousarray(b2adj.reshape(KT_D, 128).T).astype(f32)
    d["g1c"] = np.ascontiguousarray(ln1_g.reshape(KT_D, 128).T).astype(f32)
    d["be1c"] = np.ascontiguousarray(ln1_b.reshape(KT_D, 128).T).astype(f32)
    d["g2c"] = np.ascontiguousarray(ln2_g.reshape(KT_D, 128).T).astype(f32)
    d["be2c"] = np.ascontiguousarray(ln2_b.reshape(KT_D, 128).T).astype(f32)
    return d


def kernel(x, Wq, Wk, Wv, Wo, proj, W1, b1, W2, b2, ln1_g, ln1_b, ln2_g, ln2_b):
    from concourse import bass_utils

    x = np.asarray(x, np.float32)
    shared = _host_inputs(x, np.asarray(Wq), np.asarray(Wk), np.asarray(Wv),
                          np.asarray(Wo), np.asarray(proj), np.asarray(W1),
                          np.asarray(b1), np.asarray(W2), np.asarray(b2),
                          np.asarray(ln1_g), np.asarray(ln1_b),
                          np.asarray(ln2_g), np.asarray(ln2_b))

    if "nc" not in _CACHE:
        _CACHE["nc"] = _build()
    nc = _CACHE["nc"]

    in_maps = []
    for c in range(NC):
        xs = x[:, c * NT:(c + 1) * NT, :]
        xT = np.ascontiguousarray(xs.transpose(0, 2, 1))
        oh = np.zeros((1, NC), np.float32)
        oh[0, c] = 1.0
        m = dict(shared)
        m["x_f32"] = xT
        m["x_bf"] = xT.astype(ml_dtypes.bfloat16)
        m["onehot"] = oh
        in_maps.append(m)

    trace = bool(int(os.environ.get("KERNEL_TRACE", "0")))
    res = bass_utils.run_bass_kernel_spmd(nc, in_maps, core_ids=list(range(NC)),
                                          trace=trace)
    if trace and res.exec_time_ns is not None:
        print(f"HW exec time: {res.exec_time_ns} ns")
        if res.instructions_and_trace is not None:
            print("trace:", res.instructions_and_trace[1])

    outp = np.empty((B, N, D), np.float32)
    for c in range(NC):
        oT = res.results[c]["out"]
        outp[:, c * NT:(c + 1) * NT, :] = oT.transpose(0, 2, 1)
    return outp



# revision 48
# speedup vs baseline: 1.9081x; 1.0032x over previous
"""Performer (FAVOR+) encoder layer on 8 trn2 NeuronCores.

Sharding: data-parallel over sequence (512 positions per core x 4 batches).
The linear-attention summaries (A = E_k^T v per (batch, head), usum) and the
global key-feature max (via one-hot slots) are combined in ONE packed
AllReduce, overlapped with the Q-side feature compute.

Layout: activations feature-major (xT = [D, tokens]) so every GEMM's
stationary operand is a natural weight slice; per-token reductions and
broadcasts are small PE matmuls (selector / ones / f32r broadcast matmuls).
E_k and v are produced token-major directly by matmuls so the token-
contraction A-matmul needs no transposes.
"""
import os
import numpy as np
import ml_dtypes

B, N, D = 4, 4096, 1024
H, DH = 16, 64
DFF = 4096
M = 64
EPS_KERN = 1e-6
EPS_LN = 1e-6
NC = 8
NT = N // NC                # 512 positions per core per batch
PAIRS = H // 2              # 8 head-pairs
KT_D = D // 128             # 8
MT_FF = DFF // 128          # 32
TT = NT // 128              # 4
DN = 1.0 / np.sqrt(np.sqrt(DH))
DN2H = DN * DN / 2.0


def _emit(nc, tc):
    import concourse.mybir as mybir
    from contextlib import ExitStack
    F32 = mybir.dt.float32
    F32R = mybir.dt.float32r
    BF16 = mybir.dt.bfloat16
    AF = mybir.ActivationFunctionType
    ALU = mybir.AluOpType
    AX = mybir.AxisListType

    dram = lambda name, shape, dt, kind: nc.dram_tensor(name, shape, dt, kind=kind).ap()

    x_bf = dram("x_bf", [B, D, NT], BF16, "ExternalInput")
    x_f32 = dram("x_f32", [B, D, NT], F32, "ExternalInput")
    wqs = dram("wqs", [KT_D, 128, KT_D, 128], BF16, "ExternalInput")
    wks = dram("wks", [KT_D, 128, KT_D, 128], BF16, "ExternalInput")
    wv = dram("wv", [D, D], BF16, "ExternalInput")
    wos = dram("wos", [KT_D, 128, KT_D, 128], BF16, "ExternalInput")
    w1s = dram("w1s", [MT_FF, 128, KT_D, 128], BF16, "ExternalInput")
    w2s = dram("w2s", [KT_D, 128, MT_FF, 128], BF16, "ExternalInput")
    projbd = dram("projbd", [128, 128], BF16, "ExternalInput")
    negselF = dram("negselF", [2, 128], BF16, "ExternalInput")
    sel2 = dram("sel2", [128, 2], BF16, "ExternalInput")
    sel2b = dram("sel2b", [2, 128], F32, "ExternalInput")
    ones128 = dram("ones128", [128, 1], BF16, "ExternalInput")
    ones1x128 = dram("ones1x128", [1, 128], F32, "ExternalInput")
    mean1 = dram("mean1", [128, 1], F32, "ExternalInput")
    headmask2 = dram("headmask2", [128, 2], F32, "ExternalInput")
    epsvB = dram("epsvB", [128, B * D], BF16, "ExternalInput")
    onehot = dram("onehot", [1, NC], F32, "ExternalInput")
    b1c = dram("b1c", [128, MT_FF], F32, "ExternalInput")
    nb1c = dram("nb1c", [128, MT_FF], F32, "ExternalInput")
    b1p1c = dram("b1p1c", [128, MT_FF], F32, "ExternalInput")
    b2adjc = dram("b2adjc", [128, KT_D], F32, "ExternalInput")
    g1c = dram("g1c", [128, KT_D], F32, "ExternalInput")
    be1c = dram("be1c", [128, KT_D], F32, "ExternalInput")
    g2c = dram("g2c", [128, KT_D], F32, "ExternalInput")
    be2c = dram("be2c", [128, KT_D], F32, "ExternalInput")
    out = dram("out", [B, D, NT], F32, "ExternalOutput")

    AC_A = B * PAIRS * 64
    AC_U = B * PAIRS
    AC = AC_A + AC_U + NC

    ctx = ExitStack()
    pconst = ctx.enter_context(tc.tile_pool(name="pconst", bufs=1))
    pstream = ctx.enter_context(tc.tile_pool(name="pstream", bufs=2))
    pw2s = ctx.enter_context(tc.tile_pool(name="pw2s", bufs=2))
    pxa = ctx.enter_context(tc.tile_pool(name="pxa", bufs=1))
    pmt = ctx.enter_context(tc.tile_pool(name="pmt", bufs=4))
    psm = ctx.enter_context(tc.tile_pool(name="psm", bufs=2))
    peq = ctx.enter_context(tc.tile_pool(name="peq", bufs=1))
    pbig = ctx.enter_context(tc.tile_pool(name="pbig", bufs=1))
    pone = ctx.enter_context(tc.tile_pool(name="pone", bufs=1))
    pdram = ctx.enter_context(tc.tile_pool(name="pdram", bufs=1, space="DRAM"))
    PP = ctx.enter_context(tc.tile_pool(name="PP", bufs=4, space="PSUM"))
    PD = ctx.enter_context(tc.tile_pool(name="PD", bufs=2, space="PSUM"))
    PA_ = ctx.enter_context(tc.tile_pool(name="PA", bufs=1, space="PSUM"))
    PR = ctx.enter_context(tc.tile_pool(name="PR", bufs=1, space="PSUM"))

    # ---- constants ----
    # wv shares the big32 slot with epsv/hsb: wv is only read in stage A,
    # epsv only in kv assembly, hsb only from FFN1 onward — disjoint uses.
    wv_sb = pbig.tile([128, KT_D, D], BF16, tag="big32")
    nc.sync.dma_start(wv_sb[:], wv.rearrange("(kt p) m -> p kt m", p=128))
    cAPs = {}
    for name, ap, shape, dt in (
        ("projbd", projbd, [128, 128], BF16), ("negselF", negselF, [2, 128], BF16),
        ("sel2", sel2, [128, 2], BF16), ("sel2b", sel2b, [2, 128], F32),
        ("ones128", ones128, [128, 1], BF16), ("ones1x128", ones1x128, [1, 128], F32),
        ("mean1", mean1, [128, 1], F32), ("headmask2", headmask2, [128, 2], F32),
        ("onehot", onehot, [1, NC], F32), ("b1c", b1c, [128, MT_FF], F32),
        ("nb1c", nb1c, [128, MT_FF], F32),
        ("b1p1c", b1p1c, [128, MT_FF], F32), ("b2adjc", b2adjc, [128, KT_D], F32),
        ("g1c", g1c, [128, KT_D], F32), ("be1c", be1c, [128, KT_D], F32),
        ("g2c", g2c, [128, KT_D], F32), ("be2c", be2c, [128, KT_D], F32),
    ):
        t = pconst.tile(shape, dt, tag=name)
        nc.sync.dma_start(t[:], ap[:])
        cAPs[name] = t
    sel2b_r = pconst.tile([2, 128], F32R, tag="sel2br")
    ones1x128_r = pconst.tile([1, 128], F32R, tag="ones1x128r")
    mean1_r = pconst.tile([128, 1], F32R, tag="mean1r")
    mean1_bf = pconst.tile([128, 1], BF16, tag="mean1bf")
    sel2b_bf = pconst.tile([2, 128], BF16, tag="sel2bbf")
    nc.vector.tensor_copy(sel2b_r[:], cAPs["sel2b"][:])
    nc.vector.tensor_copy(ones1x128_r[:], cAPs["ones1x128"][:])
    nc.vector.tensor_copy(mean1_r[:], cAPs["mean1"][:])
    nc.vector.tensor_copy(mean1_bf[:], cAPs["mean1"][:])
    nc.vector.tensor_copy(sel2b_bf[:], cAPs["sel2b"][:])

    epsln_c = pconst.tile([1, 1], F32, tag="epslnc")
    nc.vector.memset(epsln_c[:], float(EPS_LN))
    onesrow_c = pconst.tile([1, NT], F32, tag="onesrowc")
    nc.vector.memset(onesrow_c[:], 1.0)
    magicrow = pconst.tile([1, NT], mybir.dt.int32, tag="magicrow")
    nc.vector.memset(magicrow[:], 0x5f3759df)
    oneirow = pconst.tile([1, NT], mybir.dt.int32, tag="oneirow")
    nc.vector.memset(oneirow[:], 1)
    arstage = pone.tile([128, AC], F32, tag="arbuf")
    mxcols = pone.tile([128, B * PAIRS], BF16, tag="mxcols")

    def ln_stats(res):
        """Feature-major LN stats for a [128, KT_D, NT] f32r residual.
        rstd = 1/sqrt(var+eps) entirely on the vector engine (bit-magic seed +
        2 Newton steps) — the scalar Ln/Exp pair forced an ACT_TABLE_LOAD per
        LN. Returns (mu f32r row, rstd f32r row)."""
        pm0 = PR.tile([1, NT], F32, tag="prow")
        pm1 = PA_.tile([1, NT], F32, tag="pA")
        for kt in range(KT_D):
            sqt = psm.tile([128, NT], BF16, tag="lnsq")
            nc.scalar.square(sqt[:], res[:, kt, :])
            nc.tensor.matmul(pm0[:], mean1_bf[:], res[:, kt, :],
                             start=kt == 0, stop=kt == KT_D - 1,
                             skip_group_check=True)
            nc.tensor.matmul(pm1[:], mean1_bf[:], sqt[:],
                             start=kt == 0, stop=kt == KT_D - 1,
                             skip_group_check=True)
        mu = psm.tile([1, NT], F32R, tag="lnmu", bufs=1)
        nc.vector.tensor_copy(mu[:], pm0[:])
        muf = mu[:].bitcast(F32)
        mu2 = psm.tile([1, NT], F32, tag="lnrow")
        nc.vector.tensor_tensor(mu2[:], muf, muf, op=ALU.mult)
        var = psm.tile([1, NT], F32, tag="lnrow")
        nc.vector.tensor_tensor(var[:], pm1[:], mu2[:], op=ALU.subtract)
        vare = psm.tile([1, NT], F32, tag="lnvare", bufs=1)
        nc.vector.tensor_scalar(vare[:], var[:], float(EPS_LN), None,
                                op0=ALU.add)
        I32 = mybir.dt.int32
        sh = psm.tile([1, NT], I32, tag="lnrow")
        nc.vector.tensor_tensor(sh[:], vare[:].bitcast(I32), oneirow[:],
                                op=ALU.arith_shift_right)
        y0 = psm.tile([1, NT], I32, tag="lnya", bufs=1)
        nc.vector.tensor_tensor(y0[:], magicrow[:], sh[:], op=ALU.subtract)
        y = y0[:].bitcast(F32)
        for it in range(1):
            t1 = psm.tile([1, NT], F32, tag="lnrow")
            nc.vector.tensor_tensor(t1[:], y, y, op=ALU.mult)
            t2 = psm.tile([1, NT], F32, tag="lnrow")
            nc.vector.tensor_tensor(t2[:], t1[:], vare[:], op=ALU.mult)
            t3 = psm.tile([1, NT], F32, tag="lnrow")
            nc.vector.tensor_scalar(t3[:], t2[:], -0.5, 1.5,
                                    op0=ALU.mult, op1=ALU.add)
            yn = psm.tile([1, NT], F32, tag=f"lny{it}", bufs=1)
            nc.vector.tensor_tensor(yn[:], y, t3[:], op=ALU.mult)
            y = yn[:]
        rstd = psm.tile([1, NT], F32R, tag="lnrstd", bufs=1)
        nc.vector.tensor_copy(rstd[:], y)
        return mu, rstd

    def ln_apply(mu, rstd, res, gc, bc, odt, dma_out=None):
        # pmu/prs live in the PD banks (idle during LN windows) so the PP
        # rotation used by the surrounding matmul streams never stalls on
        # the long apply reads.
        pmu = PD.tile([128, NT], F32, tag="pdd")
        nc.tensor.matmul(pmu[:], ones1x128_r[:], mu[:], start=True, stop=True)
        pmu_bf = psm.tile([128, NT], BF16, tag="lnpmubf", bufs=1)
        nc.scalar.activation(pmu_bf[:], pmu[:], AF.Copy)
        prs = PD.tile([128, NT], F32, tag="pdd")
        nc.tensor.matmul(prs[:], ones1x128_r[:], rstd[:], start=True, stop=True)
        prs_bf = psm.tile([128, NT], BF16, tag="lnprsbf", bufs=1)
        nc.scalar.activation(prs_bf[:], prs[:], AF.Copy)
        o = None
        if odt is not None:
            o = pbig.tile([128, KT_D, NT], odt, tag="bf8")
        cen = psm.tile([128, NT], BF16, tag="lncen")
        nrm = psm.tile([128, NT], BF16, tag="lnnrm")
        for kt in range(KT_D):
            nc.vector.tensor_tensor(cen[:], res[:, kt, :], pmu_bf[:],
                                    op=ALU.subtract)
            nc.vector.scalar_tensor_tensor(nrm[:], cen[:], gc[:, kt:kt + 1],
                                           prs_bf[:], op0=ALU.mult, op1=ALU.mult)
            if o is not None:
                nc.scalar.activation(o[:, kt, :], nrm[:], AF.Identity,
                                     bias=bc[:, kt:kt + 1])
            else:
                ot = psm.tile([128, NT], F32, tag="lnot")
                nc.scalar.activation(ot[:], nrm[:], AF.Identity,
                                     bias=bc[:, kt:kt + 1])
                nc.sync.dma_start(dma_out[0][dma_out[1], kt * 128:(kt + 1) * 128, :],
                                  ot[:])
        return o

    # ================= stage A =================
    for b in range(B):
        xbf = pxa.tile([128, KT_D, NT], BF16, tag="xbf", bufs=2)
        nc.sync.dma_start(xbf[:], x_bf[b].rearrange("(kt p) t -> p kt t", p=128))

        vtok = pxa.tile([128, TT, PAIRS, 129], BF16, tag="vtok", bufs=1)
        nc.vector.memset(vtok[:, :, :, 128:129], 1.0)
        for tt in range(TT):
            for nh in range(2):
                pv = PP.tile([128, 4, 128], F32, tag="pbig")
                for kt in range(KT_D):
                    nc.tensor.matmul(pv[:], xbf[:, kt, tt * 128:(tt + 1) * 128],
                                     wv_sb[:, kt, nh * 512:(nh + 1) * 512],
                                     start=kt == 0, stop=kt == KT_D - 1)
                nc.vector.tensor_copy(vtok[:, tt, nh * 4:(nh + 1) * 4, 0:128],
                                      pv[:])

        # per-pair chain software-pipelined across pairs (depth 2) so the PE
        # never waits on the scalar/vector singletons between sub-phases.
        kTs, ksq2s, Eks = {}, {}, {}

        def s1(pr):  # K projection -> token-major k + k^2 (scalar trails)
            wkmt = pstream.tile([128, KT_D, 128], BF16, tag="wmt")
            nc.sync.dma_start(wkmt[:], wks[pr])
            pk = PP.tile([128, NT], F32, tag="pbig")
            for kt in range(KT_D):
                nc.tensor.matmul(pk[:], wkmt[:, kt, :], xbf[:, kt, :],
                                 start=kt == 0, stop=kt == KT_D - 1)
            kTmt = pmt.tile([128, NT], BF16, tag="mt512", bufs=6)
            nc.scalar.activation(kTmt[:], pk[:], AF.Copy)
            ksqmt = pmt.tile([128, NT], BF16, tag="mt512", bufs=6)
            nc.scalar.square(ksqmt[:], pk[:])
            kTs[pr] = (kTmt, ksqmt)

        def s2(pr):  # squared-norm row + FAVOR features
            kTmt, ksqmt = kTs[pr]
            pks = PR.tile([2, NT], F32, tag="prow")
            nc.tensor.matmul(pks[:], cAPs["sel2"][:], ksqmt[:], start=True,
                             stop=True)
            ksq2 = psm.tile([2, NT], BF16, tag="ksq2")
            nc.scalar.activation(ksq2[:], pks[:], AF.Copy)
            ksq2s[pr] = ksq2

            # One accumulation group for the whole bank: only the FIRST matmul
            # carries start=True (whole-bank has_written clear); later slices
            # overwrite-and-set, negsels then accumulate onto set bits. The
            # raw-projection max is one whole-bank reduce between the phases.
            Ek = psm.tile([128, TT, 128], BF16, tag="Ek")
            pdd = PD.tile([128, NT], F32, tag="pdd")
            sls = [slice(tt * 128, (tt + 1) * 128) for tt in range(TT)]
            for tt in range(TT):
                nc.tensor.matmul(pdd[:, sls[tt]], kTmt[:, sls[tt]],
                                 cAPs["projbd"][:], start=tt == 0, stop=False,
                                 skip_group_check=True)
            c = b * PAIRS + pr
            nc.vector.tensor_reduce(mxcols[:, c:c + 1], pdd[:], axis=AX.X,
                                    op=ALU.max)
            for tt in range(TT):
                nc.tensor.matmul(pdd[:, sls[tt]], ksq2[:, sls[tt]],
                                 cAPs["negselF"][:], start=False,
                                 stop=tt == TT - 1, skip_group_check=True)
            nc.scalar.activation(Ek[:], pdd[:], AF.Exp)
            Eks[pr] = Ek

        def s3(pr):  # token-contraction A matmuls + AR staging
            Ek = Eks[pr]
            pA = PA_.tile([128, 129], F32, tag="pA")
            for tt in range(TT):
                nc.tensor.matmul(pA[:], Ek[:, tt, :], vtok[:, tt, pr, :],
                                 start=tt == 0, stop=tt == TT - 1,
                                 skip_group_check=True)
            j = b * PAIRS + pr
            nc.vector.tensor_copy(arstage[0:64, j * 64:(j + 1) * 64], pA[0:64, 0:64])
            nc.vector.tensor_copy(arstage[64:128, j * 64:(j + 1) * 64],
                                  pA[64:128, 64:128])
            nc.vector.tensor_copy(arstage[:, AC_A + j:AC_A + j + 1], pA[:, 128:129])

        for step in range(PAIRS + 2):
            if step < PAIRS:
                s1(step)
            if 1 <= step <= PAIRS:
                s2(step - 1)
            if step >= 2:
                s3(step - 2)

    # ---- fire AllReduce ----
    from concourse import bass_isa
    mxr = pone.tile([128, 1], F32, tag="mxr")
    nc.vector.tensor_reduce(mxr[:], mxcols[:], axis=AX.X, op=ALU.max)
    mxall = pone.tile([128, 1], F32, tag="mxall")
    nc.gpsimd.partition_all_reduce(mxall[:], mxr[:], channels=128,
                                   reduce_op=bass_isa.ReduceOp.max)
    nc.vector.tensor_scalar(arstage[0:1, AC_A + AC_U:AC], cAPs["onehot"][:],
                            mxall[0:1, :], None, op0=ALU.mult)
    arin = pdram.tile([128, AC], F32, tag="arin")
    arout = pdram.tile([128, AC], F32, tag="arout")
    nc.sync.dma_start(arin[:], arstage[:])
    if os.environ.get("KERNEL_NOCOLL"):
        nc.sync.dma_start(arout[:], arin[:])
    else:
        nc.gpsimd.collective_compute("AllReduce", ALU.add,
                                     replica_groups=[list(range(NC))],
                                     ins=[arin[:]], outs=[arout[:]])
    arres = pone.tile([128, AC], F32, tag="arbuf")
    nc.sync.dma_start(arres[:], arout[:])

    # ================= q-side features (overlap AR) =================
    Eq_all = {}

    def emit_qside(b):
        """Q-side features, software-pipelined across pairs (5 sub-stages,
        one-step lags) so the PE never waits on the scalar/vector singletons."""
        qxbf = pxa.tile([128, KT_D, NT], BF16, tag="qxbf", bufs=1)
        nc.sync.dma_start(qxbf[:], x_bf[b].rearrange("(kt p) t -> p kt t", p=128))
        Eqs = [None] * PAIRS
        Eq_all[b] = Eqs
        st = {}

        def q1(pr):
            wqmt = pstream.tile([128, KT_D, 128], BF16, tag="wmt")
            nc.sync.dma_start(wqmt[:], wqs[pr])
            pq_ = PP.tile([128, NT], F32, tag="pbig")
            for kt in range(KT_D):
                nc.tensor.matmul(pq_[:], wqmt[:, kt, :], qxbf[:, kt, :],
                                 start=kt == 0, stop=kt == KT_D - 1)
            qTmt = pmt.tile([128, NT], BF16, tag="mt512", bufs=6)
            nc.scalar.activation(qTmt[:], pq_[:], AF.Copy)
            qsqmt = pmt.tile([128, NT], BF16, tag="mt512", bufs=6)
            nc.scalar.square(qsqmt[:], pq_[:])
            st[pr] = [qTmt, qsqmt]

        def q2(pr):
            pqs = PR.tile([2, NT], F32, tag="prow")
            nc.tensor.matmul(pqs[:], cAPs["sel2"][:], st[pr][1][:], start=True,
                             stop=True)
            qsq2 = psm.tile([2, NT], BF16, tag="qsq2")
            nc.scalar.activation(qsq2[:], pqs[:], AF.Copy)
            st[pr].append(qsq2)

        def q3(pr):
            qTmt, _, qsq2 = st[pr]
            pdq = PP.tile([128, NT], F32, tag="pbig")
            nc.tensor.matmul(pdq[:], cAPs["projbd"][:], qTmt[:], start=True,
                             stop=False)
            nc.tensor.matmul(pdq[:], cAPs["negselF"][:], qsq2[:], start=False,
                             stop=True, skip_group_check=True)
            Etmp = pmt.tile([128, NT], BF16, tag="t512bf")
            nc.scalar.activation(Etmp[:], pdq[:], AF.Exp)
            ediag = psm.tile([2, NT], BF16, tag="ediag")
            nc.scalar.activation(ediag[:], qsq2[:], AF.Exp, scale=float(DN2H))
            st[pr] += [Etmp, ediag]

        def q4(pr):
            _, _, _, Etmp, ediag = st[pr]
            pS = PR.tile([2, NT], F32, tag="prow")
            nc.tensor.matmul(pS[:], cAPs["sel2"][:], Etmp[:], start=True,
                             stop=True)
            wrow = psm.tile([2, NT], BF16, tag="wrow")
            nc.vector.scalar_tensor_tensor(wrow[:], ediag[:], EPS_KERN, pS[:],
                                           op0=ALU.mult, op1=ALU.mult)
            st[pr].append(wrow)

        def q5(pr):
            Etmp, wrow = st[pr][3], st[pr][5]
            pwB = PP.tile([128, NT], F32, tag="pbig")
            nc.tensor.matmul(pwB[:], sel2b_bf[:], wrow[:], start=True, stop=True)
            Eq = peq.tile([128, NT], BF16, tag=f"Eq{b % 2}_{pr}")
            nc.vector.tensor_tensor(Eq[:], Etmp[:], pwB[:], op=ALU.add)
            Eqs[pr] = Eq

        for step in range(PAIRS + 4):
            if step < PAIRS:
                q1(step)
            if 1 <= step <= PAIRS:
                q2(step - 1)
            if 2 <= step <= PAIRS + 1:
                q3(step - 2)
            if 3 <= step <= PAIRS + 2:
                q4(step - 3)
            if step >= 4:
                q5(step - 4)

    emit_qside(0)
    emit_qside(1)

    # ---- kv / ksum assembly ----
    mx8 = pone.tile([1, 1], F32, tag="mx8")
    nc.vector.tensor_reduce(mx8[:], arres[0:1, AC_A + AC_U:AC], axis=AX.X, op=ALU.max)
    emxf = pone.tile([1, 1], F32, tag="emxf")
    nc.scalar.activation(emxf[:], mx8[:], AF.Exp, scale=-1.0)
    emxrow = psm.tile([1, NT], F32R, tag="lnrow")
    nc.vector.tensor_scalar(emxrow[:], onesrow_c[:], emxf[:], None, op0=ALU.mult)
    pex = PP.tile([128, NT], F32, tag="pbig")
    nc.tensor.matmul(pex[:], ones1x128_r[:], emxrow[:], start=True, stop=True)
    emxc = pone.tile([128, 1], F32, tag="emxc")
    nc.vector.tensor_copy(emxc[:], pex[:, 0:1])

    epsv_sb = pbig.tile([128, B * PAIRS, 128], BF16, tag="big32")
    nc.sync.dma_start(epsv_sb[:], epsvB[:])

    # bulk kv assembly: one strided op per quadrant across all 32 (b, pair)
    # summaries instead of 128 tiny vector ops — this chain gates stage B.
    kvBall = pone.tile([128, B * PAIRS, 130], BF16, tag="kvBall")
    nc.vector.memset(kvBall[:], 0.0)
    arA0 = arres[0:64, 0:AC_A].rearrange("p (j c) -> p j c", j=B * PAIRS)
    nc.vector.scalar_tensor_tensor(kvBall[0:64, :, 0:64], arA0, emxc[0:64, :],
                                   epsv_sb[0:64, :, 0:64],
                                   op0=ALU.mult, op1=ALU.add)
    arA1 = arres[64:128, 0:AC_A].rearrange("p (j c) -> p j c", j=B * PAIRS)
    nc.vector.scalar_tensor_tensor(kvBall[64:128, :, 64:128], arA1,
                                   emxc[64:128, :], epsv_sb[64:128, :, 64:128],
                                   op0=ALU.mult, op1=ALU.add)
    ksfall = pone.tile([128, B * PAIRS], F32, tag="ksfall")
    nc.vector.tensor_scalar(ksfall[:], arres[:, AC_A:AC_A + B * PAIRS],
                            emxc[:], float(EPS_KERN * N),
                            op0=ALU.mult, op1=ALU.add)
    kvB_all = {}
    for j in range(B * PAIRS):
        nc.vector.tensor_scalar(kvBall[:, j, 128:130], cAPs["headmask2"][:],
                                ksfall[:, j:j + 1], None, op0=ALU.mult)
        kvB_all[j] = kvBall[:, j, :]

    # ================= stage B (software-pipelined across batches) ==========
    def attn(b):
        Eqs = Eq_all[b]
        attnT = pbig.tile([128, KT_D, NT], BF16, tag="attnT")
        rdens = {}

        def pass1(pr):
            kvB = kvB_all[b * PAIRS + pr]
            pnum = PP.tile([128, NT], F32, tag="pbig")
            nc.tensor.matmul(pnum[:], kvB[:, 0:128], Eqs[pr][:], start=True,
                             stop=True)
            pden = PD.tile([2, NT], F32, tag="pdd")
            nc.tensor.matmul(pden[:], kvB[:, 128:130], Eqs[pr][:], start=True,
                             stop=True)
            nc.scalar.activation(attnT[:, pr, :], pnum[:], AF.Copy)
            rdf = psm.tile([2, NT], F32, tag="rden", bufs=4)
            nc.vector.reciprocal_approx_fast(rdf[:], pden[:])
            rdens[pr] = rdf

        def pass2(pr):
            prdB = PP.tile([128, NT], F32, tag="pbig")
            nc.tensor.matmul(prdB[:], cAPs["sel2b"][:], rdens[pr][:],
                             start=True, stop=True)
            nc.vector.tensor_tensor(attnT[:, pr, :], attnT[:, pr, :], prdB[:],
                                    op=ALU.mult)

        # lag-2 interleave keeps the PE dense while the reciprocal of pair p
        # finishes during pairs p+1/p+2's matmuls.
        for step in range(PAIRS + 2):
            if step < PAIRS:
                pass1(step)
            if step >= 2:
                pass2(step - 2)
        return attnT

    def wo_res1(b, attnT):
        res1 = pbig.tile([128, KT_D, NT], BF16, tag="resX")
        for mt in range(KT_D):
            womt = pstream.tile([128, KT_D, 128], BF16, tag="wmt")
            nc.sync.dma_start(womt[:], wos[mt])
            po = PP.tile([128, NT], F32, tag="pbig")
            for kt in range(KT_D):
                nc.tensor.matmul(po[:], womt[:, kt, :], attnT[:, kt, :],
                                 start=kt == 0, stop=kt == KT_D - 1)
            xf = psm.tile([128, NT], F32, tag="xf")
            nc.sync.dma_start(xf[:], x_f32[b, mt * 128:(mt + 1) * 128, :])
            nc.vector.tensor_tensor(res1[:, mt, :], xf[:], po[:], op=ALU.add)
        return res1

    def ffn(b, out1):
        hsb = pbig.tile([128, MT_FF, NT], BF16, tag="big32")
        for mt in range(MT_FF):
            w1mt = pstream.tile([128, KT_D, 128], BF16, tag="wmt")
            nc.sync.dma_start(w1mt[:], w1s[mt])
            pz = PP.tile([128, NT], F32, tag="pbig", name="pz")
            for kt in range(KT_D):
                nc.tensor.matmul(pz[:], w1mt[:, kt, :], out1[:, kt, :],
                                 start=kt == 0, stop=kt == KT_D - 1)
            # elu(z')+1 = max(z'+1, min(exp(z'), 1)): one scalar Exp, with the
            # min on the (FFN-idle) vector engine in 16-bit 2x mode.
            eraw = pmt.tile([128, NT], BF16, tag="t512bf")
            nc.scalar.activation(eraw[:], pz[:], AF.Exp,
                                 bias=cAPs["b1c"][:, mt:mt + 1])
            emin = pmt.tile([128, NT], BF16, tag="t512bf")
            nc.vector.tensor_scalar(emin[:], eraw[:], 1.0, None, op0=ALU.min)
            nc.vector.scalar_tensor_tensor(hsb[:, mt, :], pz[:],
                                           cAPs["b1p1c"][:, mt:mt + 1], emin[:],
                                           op0=ALU.add, op1=ALU.max)

        res2 = pbig.tile([128, KT_D, NT], BF16, tag="resX")
        for mt in range(KT_D):
            w2a = pw2s.tile([128, MT_FF // 2, 128], BF16, tag="w2mt")
            nc.sync.dma_start(w2a[:], w2s[mt, :, 0:MT_FF // 2])
            w2b = pw2s.tile([128, MT_FF // 2, 128], BF16, tag="w2mt")
            nc.sync.dma_start(w2b[:], w2s[mt, :, MT_FF // 2:MT_FF])
            pf = PP.tile([128, NT], F32, tag="pbig")
            for kt in range(MT_FF):
                w2h = w2a if kt < MT_FF // 2 else w2b
                nc.tensor.matmul(pf[:], w2h[:, kt % (MT_FF // 2), :],
                                 hsb[:, kt, :],
                                 start=kt == 0, stop=kt == MT_FF - 1)
            nc.vector.scalar_tensor_tensor(res2[:, mt, :], pf[:],
                                           cAPs["b2adjc"][:, mt:mt + 1],
                                           out1[:, mt, :], op0=ALU.add, op1=ALU.add)
        return res2

    res1 = wo_res1(0, attn(0))
    for b in range(B):
        mu1, rstd1 = ln_stats(res1)
        if b + 2 < B:
            # qside PE work covers the LN1 stat chain + apply window
            emit_qside(b + 2)
        out1 = ln_apply(mu1, rstd1, res1, cAPs["g1c"], cAPs["be1c"],
                        mybir.dt.bfloat16)
        res2 = ffn(b, out1)
        mu2, rstd2 = ln_stats(res2)
        if b + 1 < B:
            # next batch's attention matmuls cover the LN2 chain; its Wo
            # stream then runs while LN2 applies and the output DMAs out.
            attnT = attn(b + 1)
        ln_apply(mu2, rstd2, res2, cAPs["g2c"], cAPs["be2c"], None,
                 dma_out=(out, b))
        if b + 1 < B:
            res1 = wo_res1(b + 1, attnT)

    ctx.close()


_CACHE = {}


def _build():
    import concourse.tile as tile
    from concourse import bacc
    nc = bacc.Bacc("TRN2", target_bir_lowering=False, debug=False, num_devices=NC)
    with tile.TileContext(nc) as tc:
        _emit(nc, tc)
    nc.compile()
    return nc


def _host_inputs(x, Wq, Wk, Wv, Wo, proj, W1, b1, W2, b2,
                 ln1_g, ln1_b, ln2_g, ln2_b):
    bf = ml_dtypes.bfloat16
    f32 = np.float32
    d = {}

    def chunked(w):  # [D, X] -> [X/128 mt, 128 p, D/128 kt, 128]
        Dk, X = w.shape
        r = w.reshape(Dk // 128, 128, X // 128, 128)
        return np.ascontiguousarray(r.transpose(2, 1, 0, 3)).astype(bf)

    d["wqs"] = chunked(Wq.reshape(D, D))
    d["wks"] = chunked(Wk.reshape(D, D))
    d["wv"] = np.ascontiguousarray(Wv.reshape(D, D)).astype(bf)
    d["wos"] = chunked(Wo.reshape(D, D))
    d["w1s"] = chunked(W1)
    d["w2s"] = chunked(W2)

    projT_s = (proj * DN).T.astype(f32)
    pbd = np.zeros((128, 128), f32)
    pbd[0:64, 0:64] = projT_s
    pbd[64:128, 64:128] = projT_s
    d["projbd"] = pbd.astype(bf)
    nsF = np.zeros((2, 128), f32)
    nsF[0, 0:64] = -DN2H
    nsF[1, 64:128] = -DN2H
    d["negselF"] = nsF.astype(bf)
    s2 = np.zeros((128, 2), f32)
    s2[0:64, 0] = 1.0
    s2[64:128, 1] = 1.0
    d["sel2"] = s2.astype(bf)
    s2b = np.zeros((2, 128), f32)
    s2b[0, 0:64] = 1.0
    s2b[1, 64:128] = 1.0
    d["sel2b"] = s2b
    d["ones128"] = np.ones((128, 1), f32).astype(bf)
    d["ones1x128"] = np.ones((1, 128), f32)
    d["mean1"] = np.full((128, 1), 1.0 / D, f32)
    hm2 = np.zeros((128, 2), f32)
    hm2[0:64, 0] = 1.0
    hm2[64:128, 1] = 1.0
    d["headmask2"] = hm2

    xsum = x.sum(axis=1, dtype=np.float64)
    vsum = xsum @ Wv.reshape(D, D).astype(np.float64)
    epsv = (EPS_KERN * vsum).astype(f32)
    d["epsvB"] = np.ascontiguousarray(
        np.broadcast_to(epsv.reshape(1, B * D), (128, B * D))).astype(bf)

    d["b1c"] = np.ascontiguousarray(b1.reshape(MT_FF, 128).T).astype(f32)
    d["nb1c"] = np.ascontiguousarray((-b1).reshape(MT_FF, 128).T).astype(f32)
    d["b1p1c"] = np.ascontiguousarray((b1 + 1.0).reshape(MT_FF, 128).T).astype(f32)
    b2adj = b2.astype(np.float64) - W2.astype(np.float64).sum(axis=0)
    d["b2adjc"] = np.ascontiguousarray(b2adj.reshape(KT_D, 128).T).astype(f32)
    d["g1c"] = np.ascontiguousarray(ln1_g.reshape(KT_D, 128).T).astype(f32)
    d["be1c"] = np.ascontiguousarray(ln1_b.reshape(KT_D, 128).T).astype(f32)
    d["g2c"] = np.ascontiguousarray(ln2_g.reshape(KT_D, 128).T).astype(f32)
    d["be2c"] = np.ascontiguousarray(ln2_b.reshape(KT_D, 128).T).astype(f32)
    return d


def kernel(x, Wq, Wk, Wv, Wo, proj, W1, b1, W2, b2, ln1_g, ln1_b, ln2_g, ln2_b):
    from concourse import bass_utils

    x = np.asarray(x, np.float32)
    shared = _host_inputs(x, np.asarray(Wq), np.asarray(Wk), np.asarray(Wv),
                          np.asarray(Wo), np.asarray(proj), np.asarray(W1),
                          np.asarray(b1), np.asarray(W2), np.asarray(b2),
                          np.asarray(ln1_g), np.asarray(ln1_b),
                          np.asarray(ln2_g), np.asarray(ln2_b))

    if "nc" not in _CACHE:
        _CACHE["nc"] = _build()
    nc = _CACHE["nc"]

    in_maps = []
    for c in range(NC):
        xs = x[:, c * NT:(c + 1) * NT, :]
        xT = np.ascontiguousarray(xs.transpose(0, 2, 1))
        oh = np.zeros((1, NC), np.float32)
        oh[0, c] = 1.0
        m = dict(shared)
        m["x_f32"] = xT
        m["x_bf"] = xT.astype(ml_dtypes.bfloat16)
        m["onehot"] = oh
        in_maps.append(m)

    trace = bool(int(os.environ.get("KERNEL_TRACE", "0")))
    res = bass_utils.run_bass_kernel_spmd(nc, in_maps, core_ids=list(range(NC)),
                                          trace=trace)
    if trace and res.exec_time_ns is not None:
        print(f"HW exec time: {res.exec_time_ns} ns")
        if res.instructions_and_trace is not None:
            print("trace:", res.instructions_and_trace[1])

    outp = np.empty((B, N, D), np.float32)
    for c in range(NC):
        oT = res.results[c]["out"]
        outp[:, c * NT:(c + 1) * NT, :] = oT.transpose(0, 2, 1)
    return outp



# revision 49
# speedup vs baseline: 1.9247x; 1.0087x over previous
"""Performer (FAVOR+) encoder layer on 8 trn2 NeuronCores.

Sharding: data-parallel over sequence (512 positions per core x 4 batches).
The linear-attention summaries (A = E_k^T v per (batch, head), usum) and the
global key-feature max (via one-hot slots) are combined in ONE packed
AllReduce, overlapped with the Q-side feature compute.

Layout: activations feature-major (xT = [D, tokens]) so every GEMM's
stationary operand is a natural weight slice; per-token reductions and
broadcasts are small PE matmuls (selector / ones / f32r broadcast matmuls).
E_k and v are produced token-major directly by matmuls so the token-
contraction A-matmul needs no transposes.
"""
import os
import numpy as np
import ml_dtypes

B, N, D = 4, 4096, 1024
H, DH = 16, 64
DFF = 4096
M = 64
EPS_KERN = 1e-6
EPS_LN = 1e-6
NC = 8
NT = N // NC                # 512 positions per core per batch
PAIRS = H // 2              # 8 head-pairs
KT_D = D // 128             # 8
MT_FF = DFF // 128          # 32
TT = NT // 128              # 4
DN = 1.0 / np.sqrt(np.sqrt(DH))
DN2H = DN * DN / 2.0


def _emit(nc, tc):
    import concourse.mybir as mybir
    from contextlib import ExitStack
    F32 = mybir.dt.float32
    F32R = mybir.dt.float32r
    BF16 = mybir.dt.bfloat16
    AF = mybir.ActivationFunctionType
    ALU = mybir.AluOpType
    AX = mybir.AxisListType

    dram = lambda name, shape, dt, kind: nc.dram_tensor(name, shape, dt, kind=kind).ap()

    x_bf = dram("x_bf", [B, D, NT], BF16, "ExternalInput")
    x_f32 = dram("x_f32", [B, D, NT], F32, "ExternalInput")
    wqs = dram("wqs", [KT_D, 128, KT_D, 128], BF16, "ExternalInput")
    wks = dram("wks", [KT_D, 128, KT_D, 128], BF16, "ExternalInput")
    wv = dram("wv", [D, D], BF16, "ExternalInput")
    wos = dram("wos", [KT_D, 128, KT_D, 128], BF16, "ExternalInput")
    w1s = dram("w1s", [MT_FF, 128, KT_D, 128], BF16, "ExternalInput")
    w2s = dram("w2s", [KT_D, 128, MT_FF, 128], BF16, "ExternalInput")
    projbd = dram("projbd", [128, 128], BF16, "ExternalInput")
    negselF = dram("negselF", [2, 128], BF16, "ExternalInput")
    sel2 = dram("sel2", [128, 2], BF16, "ExternalInput")
    sel2b = dram("sel2b", [2, 128], F32, "ExternalInput")
    ones128 = dram("ones128", [128, 1], BF16, "ExternalInput")
    ones1x128 = dram("ones1x128", [1, 128], F32, "ExternalInput")
    mean1 = dram("mean1", [128, 1], F32, "ExternalInput")
    headmask2 = dram("headmask2", [128, 2], F32, "ExternalInput")
    epsvB = dram("epsvB", [128, B * D], BF16, "ExternalInput")
    onehot = dram("onehot", [1, NC], F32, "ExternalInput")
    b1c = dram("b1c", [128, MT_FF], F32, "ExternalInput")
    nb1c = dram("nb1c", [128, MT_FF], F32, "ExternalInput")
    b1p1c = dram("b1p1c", [128, MT_FF], F32, "ExternalInput")
    b2adjc = dram("b2adjc", [128, KT_D], F32, "ExternalInput")
    g1c = dram("g1c", [128, KT_D], F32, "ExternalInput")
    be1c = dram("be1c", [128, KT_D], F32, "ExternalInput")
    g2c = dram("g2c", [128, KT_D], F32, "ExternalInput")
    be2c = dram("be2c", [128, KT_D], F32, "ExternalInput")
    out = dram("out", [B, D, NT], F32, "ExternalOutput")

    AC_A = B * PAIRS * 64
    AC_U = B * PAIRS
    AC = AC_A + AC_U + NC

    ctx = ExitStack()
    pconst = ctx.enter_context(tc.tile_pool(name="pconst", bufs=1))
    pstream = ctx.enter_context(tc.tile_pool(name="pstream", bufs=2))
    pw2s = ctx.enter_context(tc.tile_pool(name="pw2s", bufs=2))
    pxa = ctx.enter_context(tc.tile_pool(name="pxa", bufs=1))
    pmt = ctx.enter_context(tc.tile_pool(name="pmt", bufs=4))
    psm = ctx.enter_context(tc.tile_pool(name="psm", bufs=2))
    peq = ctx.enter_context(tc.tile_pool(name="peq", bufs=1))
    pbig = ctx.enter_context(tc.tile_pool(name="pbig", bufs=1))
    pone = ctx.enter_context(tc.tile_pool(name="pone", bufs=1))
    pdram = ctx.enter_context(tc.tile_pool(name="pdram", bufs=1, space="DRAM"))
    PP = ctx.enter_context(tc.tile_pool(name="PP", bufs=4, space="PSUM"))
    PD = ctx.enter_context(tc.tile_pool(name="PD", bufs=2, space="PSUM"))
    PA_ = ctx.enter_context(tc.tile_pool(name="PA", bufs=1, space="PSUM"))
    PR = ctx.enter_context(tc.tile_pool(name="PR", bufs=1, space="PSUM"))

    # ---- constants ----
    # wv shares the big32 slot with epsv/hsb: wv is only read in stage A,
    # epsv only in kv assembly, hsb only from FFN1 onward — disjoint uses.
    wv_sb = pbig.tile([128, KT_D, D], BF16, tag="big32")
    nc.sync.dma_start(wv_sb[:], wv.rearrange("(kt p) m -> p kt m", p=128))
    cAPs = {}
    for name, ap, shape, dt in (
        ("projbd", projbd, [128, 128], BF16), ("negselF", negselF, [2, 128], BF16),
        ("sel2", sel2, [128, 2], BF16), ("sel2b", sel2b, [2, 128], F32),
        ("ones128", ones128, [128, 1], BF16), ("ones1x128", ones1x128, [1, 128], F32),
        ("mean1", mean1, [128, 1], F32), ("headmask2", headmask2, [128, 2], F32),
        ("onehot", onehot, [1, NC], F32), ("b1c", b1c, [128, MT_FF], F32),
        ("nb1c", nb1c, [128, MT_FF], F32),
        ("b1p1c", b1p1c, [128, MT_FF], F32), ("b2adjc", b2adjc, [128, KT_D], F32),
        ("g1c", g1c, [128, KT_D], F32), ("be1c", be1c, [128, KT_D], F32),
        ("g2c", g2c, [128, KT_D], F32), ("be2c", be2c, [128, KT_D], F32),
    ):
        t = pconst.tile(shape, dt, tag=name)
        nc.sync.dma_start(t[:], ap[:])
        cAPs[name] = t
    sel2b_r = pconst.tile([2, 128], F32R, tag="sel2br")
    ones1x128_r = pconst.tile([1, 128], F32R, tag="ones1x128r")
    mean1_r = pconst.tile([128, 1], F32R, tag="mean1r")
    mean1_bf = pconst.tile([128, 1], BF16, tag="mean1bf")
    sel2b_bf = pconst.tile([2, 128], BF16, tag="sel2bbf")
    nc.vector.tensor_copy(sel2b_r[:], cAPs["sel2b"][:])
    nc.vector.tensor_copy(ones1x128_r[:], cAPs["ones1x128"][:])
    nc.vector.tensor_copy(mean1_r[:], cAPs["mean1"][:])
    nc.vector.tensor_copy(mean1_bf[:], cAPs["mean1"][:])
    nc.vector.tensor_copy(sel2b_bf[:], cAPs["sel2b"][:])

    epsln_c = pconst.tile([1, 1], F32, tag="epslnc")
    nc.vector.memset(epsln_c[:], float(EPS_LN))
    onesrow_c = pconst.tile([1, NT], F32, tag="onesrowc")
    nc.vector.memset(onesrow_c[:], 1.0)
    magicrow = pconst.tile([1, NT], mybir.dt.int32, tag="magicrow")
    nc.vector.memset(magicrow[:], 0x5f3759df)
    oneirow = pconst.tile([1, NT], mybir.dt.int32, tag="oneirow")
    nc.vector.memset(oneirow[:], 1)
    arstage = pone.tile([128, AC], F32, tag="arbuf")
    mxcols = pone.tile([128, B * PAIRS], BF16, tag="mxcols")

    def ln_stats(res):
        """Feature-major LN stats for a [128, KT_D, NT] f32r residual.
        rstd = 1/sqrt(var+eps) entirely on the vector engine (bit-magic seed +
        2 Newton steps) — the scalar Ln/Exp pair forced an ACT_TABLE_LOAD per
        LN. Returns (mu f32r row, rstd f32r row)."""
        pm0 = PR.tile([1, NT], F32, tag="prow")
        pm1 = PA_.tile([1, NT], F32, tag="pA")
        for kt in range(KT_D):
            sqt = psm.tile([128, NT], BF16, tag="lnsq")
            nc.scalar.square(sqt[:], res[:, kt, :])
            nc.tensor.matmul(pm0[:], mean1_bf[:], res[:, kt, :],
                             start=kt == 0, stop=kt == KT_D - 1,
                             skip_group_check=True)
            nc.tensor.matmul(pm1[:], mean1_bf[:], sqt[:],
                             start=kt == 0, stop=kt == KT_D - 1,
                             skip_group_check=True)
        mu = psm.tile([1, NT], F32R, tag="lnmu", bufs=1)
        nc.vector.tensor_copy(mu[:], pm0[:])
        muf = mu[:].bitcast(F32)
        mu2 = psm.tile([1, NT], F32, tag="lnrow")
        nc.vector.tensor_tensor(mu2[:], muf, muf, op=ALU.mult)
        var = psm.tile([1, NT], F32, tag="lnrow")
        nc.vector.tensor_tensor(var[:], pm1[:], mu2[:], op=ALU.subtract)
        vare = psm.tile([1, NT], F32, tag="lnvare", bufs=1)
        nc.vector.tensor_scalar(vare[:], var[:], float(EPS_LN), None,
                                op0=ALU.add)
        I32 = mybir.dt.int32
        sh = psm.tile([1, NT], I32, tag="lnrow")
        nc.vector.tensor_tensor(sh[:], vare[:].bitcast(I32), oneirow[:],
                                op=ALU.arith_shift_right)
        y0 = psm.tile([1, NT], I32, tag="lnya", bufs=1)
        nc.vector.tensor_tensor(y0[:], magicrow[:], sh[:], op=ALU.subtract)
        y = y0[:].bitcast(F32)
        for it in range(1):
            t1 = psm.tile([1, NT], F32, tag="lnrow")
            nc.vector.tensor_tensor(t1[:], y, y, op=ALU.mult)
            t2 = psm.tile([1, NT], F32, tag="lnrow")
            nc.vector.tensor_tensor(t2[:], t1[:], vare[:], op=ALU.mult)
            t3 = psm.tile([1, NT], F32, tag="lnrow")
            nc.vector.tensor_scalar(t3[:], t2[:], -0.5, 1.5,
                                    op0=ALU.mult, op1=ALU.add)
            yn = psm.tile([1, NT], F32, tag=f"lny{it}", bufs=1)
            nc.vector.tensor_tensor(yn[:], y, t3[:], op=ALU.mult)
            y = yn[:]
        rstd = psm.tile([1, NT], F32R, tag="lnrstd", bufs=1)
        nc.vector.tensor_copy(rstd[:], y)
        return mu, rstd

    def ln_apply(mu, rstd, res, gc, bc, odt, dma_out=None):
        # pmu/prs live in the PD banks (idle during LN windows) so the PP
        # rotation used by the surrounding matmul streams never stalls on
        # the long apply reads.
        pmu = PD.tile([128, NT], F32, tag="pdd")
        nc.tensor.matmul(pmu[:], ones1x128_r[:], mu[:], start=True, stop=True)
        pmu_bf = psm.tile([128, NT], BF16, tag="lnpmubf", bufs=1)
        nc.scalar.activation(pmu_bf[:], pmu[:], AF.Copy)
        prs = PD.tile([128, NT], F32, tag="pdd")
        nc.tensor.matmul(prs[:], ones1x128_r[:], rstd[:], start=True, stop=True)
        prs_bf = psm.tile([128, NT], BF16, tag="lnprsbf", bufs=1)
        nc.scalar.activation(prs_bf[:], prs[:], AF.Copy)
        o = None
        if odt is not None:
            o = pbig.tile([128, KT_D, NT], odt, tag="bf8")
        cen = psm.tile([128, NT], BF16, tag="lncen")
        nrm = psm.tile([128, NT], BF16, tag="lnnrm")
        for kt in range(KT_D):
            nc.vector.tensor_tensor(cen[:], res[:, kt, :], pmu_bf[:],
                                    op=ALU.subtract)
            nc.vector.scalar_tensor_tensor(nrm[:], cen[:], gc[:, kt:kt + 1],
                                           prs_bf[:], op0=ALU.mult, op1=ALU.mult)
            if o is not None:
                nc.scalar.activation(o[:, kt, :], nrm[:], AF.Identity,
                                     bias=bc[:, kt:kt + 1])
            else:
                ot = psm.tile([128, NT], F32, tag="lnot")
                nc.scalar.activation(ot[:], nrm[:], AF.Identity,
                                     bias=bc[:, kt:kt + 1])
                nc.sync.dma_start(dma_out[0][dma_out[1], kt * 128:(kt + 1) * 128, :],
                                  ot[:])
        return o

    # ================= stage A =================
    for b in range(B):
        xbf = pxa.tile([128, KT_D, NT], BF16, tag="xbf", bufs=2)
        nc.sync.dma_start(xbf[:], x_bf[b].rearrange("(kt p) t -> p kt t", p=128))

        vtok = pxa.tile([128, TT, PAIRS, 129], BF16, tag="vtok", bufs=1)
        nc.vector.memset(vtok[:, :, :, 128:129], 1.0)
        for tt in range(TT):
            for nh in range(2):
                pv = PP.tile([128, 4, 128], F32, tag="pbig")
                for kt in range(KT_D):
                    nc.tensor.matmul(pv[:], xbf[:, kt, tt * 128:(tt + 1) * 128],
                                     wv_sb[:, kt, nh * 512:(nh + 1) * 512],
                                     start=kt == 0, stop=kt == KT_D - 1)
                nc.vector.tensor_copy(vtok[:, tt, nh * 4:(nh + 1) * 4, 0:128],
                                      pv[:])

        # per-pair chain software-pipelined across pairs (depth 2) so the PE
        # never waits on the scalar/vector singletons between sub-phases.
        kTs, ksq2s, Eks = {}, {}, {}

        def s1(pr):  # K projection -> token-major k + k^2 (scalar trails)
            wkmt = pstream.tile([128, KT_D, 128], BF16, tag="wmt")
            nc.sync.dma_start(wkmt[:], wks[pr])
            pk = PP.tile([128, NT], F32, tag="pbig")
            for kt in range(KT_D):
                nc.tensor.matmul(pk[:], wkmt[:, kt, :], xbf[:, kt, :],
                                 start=kt == 0, stop=kt == KT_D - 1)
            kTmt = pmt.tile([128, NT], BF16, tag="mt512", bufs=6)
            nc.scalar.activation(kTmt[:], pk[:], AF.Copy)
            ksqmt = pmt.tile([128, NT], BF16, tag="mt512", bufs=6)
            nc.scalar.square(ksqmt[:], pk[:])
            kTs[pr] = (kTmt, ksqmt)

        def s2(pr):  # squared-norm row + FAVOR features
            kTmt, ksqmt = kTs[pr]
            pks = PR.tile([2, NT], F32, tag="prow")
            nc.tensor.matmul(pks[:], cAPs["sel2"][:], ksqmt[:], start=True,
                             stop=True)
            ksq2 = psm.tile([2, NT], BF16, tag="ksq2")
            nc.scalar.activation(ksq2[:], pks[:], AF.Copy)
            ksq2s[pr] = ksq2

            # One accumulation group for the whole bank: only the FIRST matmul
            # carries start=True (whole-bank has_written clear); later slices
            # overwrite-and-set, negsels then accumulate onto set bits. The
            # raw-projection max is one whole-bank reduce between the phases.
            Ek = psm.tile([128, TT, 128], BF16, tag="Ek")
            pdd = PD.tile([128, NT], F32, tag="pdd")
            sls = [slice(tt * 128, (tt + 1) * 128) for tt in range(TT)]
            for tt in range(TT):
                nc.tensor.matmul(pdd[:, sls[tt]], kTmt[:, sls[tt]],
                                 cAPs["projbd"][:], start=tt == 0, stop=False,
                                 skip_group_check=True)
            c = b * PAIRS + pr
            nc.vector.tensor_reduce(mxcols[:, c:c + 1], pdd[:], axis=AX.X,
                                    op=ALU.max)
            for tt in range(TT):
                nc.tensor.matmul(pdd[:, sls[tt]], ksq2[:, sls[tt]],
                                 cAPs["negselF"][:], start=False,
                                 stop=tt == TT - 1, skip_group_check=True)
            nc.scalar.activation(Ek[:], pdd[:], AF.Exp)
            Eks[pr] = Ek

        def s3(pr):  # token-contraction A matmuls + AR staging
            Ek = Eks[pr]
            pA = PA_.tile([128, 129], F32, tag="pA")
            for tt in range(TT):
                nc.tensor.matmul(pA[:], Ek[:, tt, :], vtok[:, tt, pr, :],
                                 start=tt == 0, stop=tt == TT - 1,
                                 skip_group_check=True)
            j = b * PAIRS + pr
            nc.vector.tensor_copy(arstage[0:64, j * 64:(j + 1) * 64], pA[0:64, 0:64])
            nc.vector.tensor_copy(arstage[64:128, j * 64:(j + 1) * 64],
                                  pA[64:128, 64:128])
            nc.vector.tensor_copy(arstage[:, AC_A + j:AC_A + j + 1], pA[:, 128:129])

        for step in range(PAIRS + 2):
            if step < PAIRS:
                s1(step)
            if 1 <= step <= PAIRS:
                s2(step - 1)
            if step >= 2:
                s3(step - 2)

    # ---- fire AllReduce ----
    from concourse import bass_isa
    mxr = pone.tile([128, 1], F32, tag="mxr")
    nc.vector.tensor_reduce(mxr[:], mxcols[:], axis=AX.X, op=ALU.max)
    mxall = pone.tile([128, 1], F32, tag="mxall")
    nc.gpsimd.partition_all_reduce(mxall[:], mxr[:], channels=128,
                                   reduce_op=bass_isa.ReduceOp.max)
    nc.vector.tensor_scalar(arstage[0:1, AC_A + AC_U:AC], cAPs["onehot"][:],
                            mxall[0:1, :], None, op0=ALU.mult)
    arin = pdram.tile([128, AC], F32, tag="arin")
    arout = pdram.tile([128, AC], F32, tag="arout")
    nc.sync.dma_start(arin[:], arstage[:])
    if os.environ.get("KERNEL_NOCOLL"):
        nc.sync.dma_start(arout[:], arin[:])
    else:
        nc.gpsimd.collective_compute("AllReduce", ALU.add,
                                     replica_groups=[list(range(NC))],
                                     ins=[arin[:]], outs=[arout[:]])
    arres = pone.tile([128, AC], F32, tag="arbuf")
    nc.sync.dma_start(arres[:], arout[:])

    # ================= q-side features (overlap AR) =================
    Eq_all = {}

    def emit_qside(b):
        """Q-side features, software-pipelined across pairs (5 sub-stages,
        one-step lags) so the PE never waits on the scalar/vector singletons."""
        qxbf = pxa.tile([128, KT_D, NT], BF16, tag="qxbf", bufs=1)
        nc.sync.dma_start(qxbf[:], x_bf[b].rearrange("(kt p) t -> p kt t", p=128))
        Eqs = [None] * PAIRS
        Eq_all[b] = Eqs
        st = {}

        def q1(pr):
            wqmt = pstream.tile([128, KT_D, 128], BF16, tag="wmt")
            nc.sync.dma_start(wqmt[:], wqs[pr])
            pq_ = PP.tile([128, NT], F32, tag="pbig")
            for kt in range(KT_D):
                nc.tensor.matmul(pq_[:], wqmt[:, kt, :], qxbf[:, kt, :],
                                 start=kt == 0, stop=kt == KT_D - 1)
            qTmt = pmt.tile([128, NT], BF16, tag="mt512", bufs=6)
            nc.scalar.activation(qTmt[:], pq_[:], AF.Copy)
            qsqmt = pmt.tile([128, NT], BF16, tag="mt512", bufs=6)
            nc.scalar.square(qsqmt[:], pq_[:])
            st[pr] = [qTmt, qsqmt]

        def q2(pr):
            pqs = PR.tile([2, NT], F32, tag="prow")
            nc.tensor.matmul(pqs[:], cAPs["sel2"][:], st[pr][1][:], start=True,
                             stop=True)
            qsq2 = psm.tile([2, NT], BF16, tag="qsq2")
            nc.scalar.activation(qsq2[:], pqs[:], AF.Copy)
            st[pr].append(qsq2)

        def q3(pr):
            qTmt, _, qsq2 = st[pr]
            pdq = PP.tile([128, NT], F32, tag="pbig")
            nc.tensor.matmul(pdq[:], cAPs["projbd"][:], qTmt[:], start=True,
                             stop=False)
            nc.tensor.matmul(pdq[:], cAPs["negselF"][:], qsq2[:], start=False,
                             stop=True, skip_group_check=True)
            Etmp = pmt.tile([128, NT], BF16, tag="t512bf")
            nc.scalar.activation(Etmp[:], pdq[:], AF.Exp)
            ediag = psm.tile([2, NT], BF16, tag="ediag")
            nc.scalar.activation(ediag[:], qsq2[:], AF.Exp, scale=float(DN2H))
            st[pr] += [Etmp, ediag]

        def q4(pr):
            _, _, _, Etmp, ediag = st[pr]
            pS = PR.tile([2, NT], F32, tag="prow")
            nc.tensor.matmul(pS[:], cAPs["sel2"][:], Etmp[:], start=True,
                             stop=True)
            wrow = psm.tile([2, NT], BF16, tag="wrow")
            nc.vector.scalar_tensor_tensor(wrow[:], ediag[:], EPS_KERN, pS[:],
                                           op0=ALU.mult, op1=ALU.mult)
            st[pr].append(wrow)

        def q5(pr):
            Etmp, wrow = st[pr][3], st[pr][5]
            pwB = PP.tile([128, NT], F32, tag="pbig")
            nc.tensor.matmul(pwB[:], sel2b_bf[:], wrow[:], start=True, stop=True)
            Eq = peq.tile([128, NT], BF16, tag=f"Eq{b % 2}_{pr}")
            nc.vector.tensor_tensor(Eq[:], Etmp[:], pwB[:], op=ALU.add)
            Eqs[pr] = Eq

        for step in range(PAIRS + 4):
            if step < PAIRS:
                q1(step)
            if 1 <= step <= PAIRS:
                q2(step - 1)
            if 2 <= step <= PAIRS + 1:
                q3(step - 2)
            if 3 <= step <= PAIRS + 2:
                q4(step - 3)
            if step >= 4:
                q5(step - 4)

    emit_qside(0)
    emit_qside(1)

    # ---- kv / ksum assembly ----
    mx8 = pone.tile([1, 1], F32, tag="mx8")
    nc.vector.tensor_reduce(mx8[:], arres[0:1, AC_A + AC_U:AC], axis=AX.X, op=ALU.max)
    emxf = pone.tile([1, 1], F32, tag="emxf")
    nc.scalar.activation(emxf[:], mx8[:], AF.Exp, scale=-1.0)
    emxrow = psm.tile([1, NT], F32R, tag="lnrow")
    nc.vector.tensor_scalar(emxrow[:], onesrow_c[:], emxf[:], None, op0=ALU.mult)
    pex = PP.tile([128, NT], F32, tag="pbig")
    nc.tensor.matmul(pex[:], ones1x128_r[:], emxrow[:], start=True, stop=True)
    emxc = pone.tile([128, 1], F32, tag="emxc")
    nc.vector.tensor_copy(emxc[:], pex[:, 0:1])

    epsv_sb = pbig.tile([128, B * PAIRS, 128], BF16, tag="big32")
    nc.sync.dma_start(epsv_sb[:], epsvB[:])

    # bulk kv assembly: one strided op per quadrant across all 32 (b, pair)
    # summaries instead of 128 tiny vector ops — this chain gates stage B.
    kvBall = pone.tile([128, B * PAIRS, 130], BF16, tag="kvBall")
    nc.vector.memset(kvBall[:], 0.0)
    arA0 = arres[0:64, 0:AC_A].rearrange("p (j c) -> p j c", j=B * PAIRS)
    nc.vector.scalar_tensor_tensor(kvBall[0:64, :, 0:64], arA0, emxc[0:64, :],
                                   epsv_sb[0:64, :, 0:64],
                                   op0=ALU.mult, op1=ALU.add)
    arA1 = arres[64:128, 0:AC_A].rearrange("p (j c) -> p j c", j=B * PAIRS)
    nc.vector.scalar_tensor_tensor(kvBall[64:128, :, 64:128], arA1,
                                   emxc[64:128, :], epsv_sb[64:128, :, 64:128],
                                   op0=ALU.mult, op1=ALU.add)
    ksfall = pone.tile([128, B * PAIRS], F32, tag="ksfall")
    nc.vector.tensor_scalar(ksfall[:], arres[:, AC_A:AC_A + B * PAIRS],
                            emxc[:], float(EPS_KERN * N),
                            op0=ALU.mult, op1=ALU.add)
    kvB_all = {}
    for j in range(B * PAIRS):
        nc.vector.tensor_scalar(kvBall[:, j, 128:130], cAPs["headmask2"][:],
                                ksfall[:, j:j + 1], None, op0=ALU.mult)
        kvB_all[j] = kvBall[:, j, :]

    # ================= stage B (software-pipelined across batches) ==========
    def attn(b):
        Eqs = Eq_all[b]
        attnT = pbig.tile([128, KT_D, NT], BF16, tag="attnT")
        rdens = {}

        def pass1(pr):
            kvB = kvB_all[b * PAIRS + pr]
            pnum = PP.tile([128, NT], F32, tag="pbig")
            nc.tensor.matmul(pnum[:], kvB[:, 0:128], Eqs[pr][:], start=True,
                             stop=True)
            pden = PD.tile([2, NT], F32, tag="pdd")
            nc.tensor.matmul(pden[:], kvB[:, 128:130], Eqs[pr][:], start=True,
                             stop=True)
            nc.scalar.activation(attnT[:, pr, :], pnum[:], AF.Copy)
            rdf = psm.tile([2, NT], F32, tag="rden", bufs=4)
            nc.vector.reciprocal_approx_fast(rdf[:], pden[:])
            rdens[pr] = rdf

        def pass2(pr):
            prdB = PP.tile([128, NT], F32, tag="pbig")
            nc.tensor.matmul(prdB[:], cAPs["sel2b"][:], rdens[pr][:],
                             start=True, stop=True)
            nc.vector.tensor_tensor(attnT[:, pr, :], attnT[:, pr, :], prdB[:],
                                    op=ALU.mult)

        # lag-2 interleave keeps the PE dense while the reciprocal of pair p
        # finishes during pairs p+1/p+2's matmuls.
        for step in range(PAIRS + 2):
            if step < PAIRS:
                pass1(step)
            if step >= 2:
                pass2(step - 2)
        return attnT

    def wo_res1(b, attnT):
        res1 = pbig.tile([128, KT_D, NT], BF16, tag="resX")
        for mt in range(KT_D):
            womt = pstream.tile([128, KT_D, 128], BF16, tag="wmt")
            nc.sync.dma_start(womt[:], wos[mt])
            po = PP.tile([128, NT], F32, tag="pbig")
            for kt in range(KT_D):
                nc.tensor.matmul(po[:], womt[:, kt, :], attnT[:, kt, :],
                                 start=kt == 0, stop=kt == KT_D - 1)
            xf = psm.tile([128, NT], F32, tag="xf")
            nc.sync.dma_start(xf[:], x_f32[b, mt * 128:(mt + 1) * 128, :])
            nc.vector.tensor_tensor(res1[:, mt, :], xf[:], po[:], op=ALU.add)
        return res1

    def ffn(b, out1):
        hsb = pbig.tile([128, MT_FF, NT], BF16, tag="big32")
        for mt in range(MT_FF):
            w1mt = pstream.tile([128, KT_D, 128], BF16, tag="wmt")
            nc.sync.dma_start(w1mt[:], w1s[mt])
            # 6-deep pz rotation: 4 PP banks + the 2 PD banks (pmu/prs
            # reads are done before mt=2 needs a PD bank).
            if mt % 3 != 2:
                pz = PP.tile([128, NT], F32, tag="pbig", name="pz")
            else:
                pz = PD.tile([128, NT], F32, tag="pdd", name="pz")
            for kt in range(KT_D):
                nc.tensor.matmul(pz[:], w1mt[:, kt, :], out1[:, kt, :],
                                 start=kt == 0, stop=kt == KT_D - 1)
            # elu(z')+1 = max(z'+1, min(exp(z'), 1)): one scalar Exp, with the
            # min on the (FFN-idle) vector engine in 16-bit 2x mode.
            eraw = pmt.tile([128, NT], BF16, tag="t512bf")
            nc.scalar.activation(eraw[:], pz[:], AF.Exp,
                                 bias=cAPs["b1c"][:, mt:mt + 1])
            emin = pmt.tile([128, NT], BF16, tag="t512bf")
            nc.vector.tensor_scalar(emin[:], eraw[:], 1.0, None, op0=ALU.min)
            nc.vector.scalar_tensor_tensor(hsb[:, mt, :], pz[:],
                                           cAPs["b1p1c"][:, mt:mt + 1], emin[:],
                                           op0=ALU.add, op1=ALU.max)

        res2 = pbig.tile([128, KT_D, NT], BF16, tag="resX")
        for mt in range(KT_D):
            w2a = pw2s.tile([128, MT_FF // 2, 128], BF16, tag="w2mt")
            nc.sync.dma_start(w2a[:], w2s[mt, :, 0:MT_FF // 2])
            w2b = pw2s.tile([128, MT_FF // 2, 128], BF16, tag="w2mt")
            nc.sync.dma_start(w2b[:], w2s[mt, :, MT_FF // 2:MT_FF])
            pf = PP.tile([128, NT], F32, tag="pbig")
            for kt in range(MT_FF):
                w2h = w2a if kt < MT_FF // 2 else w2b
                nc.tensor.matmul(pf[:], w2h[:, kt % (MT_FF // 2), :],
                                 hsb[:, kt, :],
                                 start=kt == 0, stop=kt == MT_FF - 1)
            nc.vector.scalar_tensor_tensor(res2[:, mt, :], pf[:],
                                           cAPs["b2adjc"][:, mt:mt + 1],
                                           out1[:, mt, :], op0=ALU.add, op1=ALU.add)
        return res2

    res1 = wo_res1(0, attn(0))
    for b in range(B):
        mu1, rstd1 = ln_stats(res1)
        if b + 2 < B:
            # qside PE work covers the LN1 stat chain + apply window
            emit_qside(b + 2)
        out1 = ln_apply(mu1, rstd1, res1, cAPs["g1c"], cAPs["be1c"],
                        mybir.dt.bfloat16)
        res2 = ffn(b, out1)
        mu2, rstd2 = ln_stats(res2)
        if b + 1 < B:
            # next batch's attention matmuls cover the LN2 chain; its Wo
            # stream then runs while LN2 applies and the output DMAs out.
            attnT = attn(b + 1)
        ln_apply(mu2, rstd2, res2, cAPs["g2c"], cAPs["be2c"], None,
                 dma_out=(out, b))
        if b + 1 < B:
            res1 = wo_res1(b + 1, attnT)

    ctx.close()


_CACHE = {}


def _build():
    import concourse.tile as tile
    from concourse import bacc
    nc = bacc.Bacc("TRN2", target_bir_lowering=False, debug=False, num_devices=NC)
    with tile.TileContext(nc) as tc:
        _emit(nc, tc)
    nc.compile()
    return nc


def _host_inputs(x, Wq, Wk, Wv, Wo, proj, W1, b1, W2, b2,
                 ln1_g, ln1_b, ln2_g, ln2_b):
    bf = ml_dtypes.bfloat16
    f32 = np.float32
    d = {}

    def chunked(w):  # [D, X] -> [X/128 mt, 128 p, D/128 kt, 128]
        Dk, X = w.shape
        r = w.reshape(Dk // 128, 128, X // 128, 128)
        return np.ascontiguousarray(r.transpose(2, 1, 0, 3)).astype(bf)

    d["wqs"] = chunked(Wq.reshape(D, D))
    d["wks"] = chunked(Wk.reshape(D, D))
    d["wv"] = np.ascontiguousarray(Wv.reshape(D, D)).astype(bf)
    d["wos"] = chunked(Wo.reshape(D, D))
    d["w1s"] = chunked(W1)
    d["w2s"] = chunked(W2)

    projT_s = (proj * DN).T.astype(f32)
    pbd = np.zeros((128, 128), f32)
    pbd[0:64, 0:64] = projT_s
    pbd[64:128, 64:128] = projT_s
    d["projbd"] = pbd.astype(bf)
    nsF = np.zeros((2, 128), f32)
    nsF[0, 0:64] = -DN2H
    nsF[1, 64:128] = -DN2H
    d["negselF"] = nsF.astype(bf)
    s2 = np.zeros((128, 2), f32)
    s2[0:64, 0] = 1.0
    s2[64:128, 1] = 1.0
    d["sel2"] = s2.astype(bf)
    s2b = np.zeros((2, 128), f32)
    s2b[0, 0:64] = 1.0
    s2b[1, 64:128] = 1.0
    d["sel2b"] = s2b
    d["ones128"] = np.ones((128, 1), f32).astype(bf)
    d["ones1x128"] = np.ones((1, 128), f32)
    d["mean1"] = np.full((128, 1), 1.0 / D, f32)
    hm2 = np.zeros((128, 2), f32)
    hm2[0:64, 0] = 1.0
    hm2[64:128, 1] = 1.0
    d["headmask2"] = hm2

    xsum = x.sum(axis=1, dtype=np.float64)
    vsum = xsum @ Wv.reshape(D, D).astype(np.float64)
    epsv = (EPS_KERN * vsum).astype(f32)
    d["epsvB"] = np.ascontiguousarray(
        np.broadcast_to(epsv.reshape(1, B * D), (128, B * D))).astype(bf)

    d["b1c"] = np.ascontiguousarray(b1.reshape(MT_FF, 128).T).astype(f32)
    d["nb1c"] = np.ascontiguousarray((-b1).reshape(MT_FF, 128).T).astype(f32)
    d["b1p1c"] = np.ascontiguousarray((b1 + 1.0).reshape(MT_FF, 128).T).astype(f32)
    b2adj = b2.astype(np.float64) - W2.astype(np.float64).sum(axis=0)
    d["b2adjc"] = np.ascontiguousarray(b2adj.reshape(KT_D, 128).T).astype(f32)
    d["g1c"] = np.ascontiguousarray(ln1_g.reshape(KT_D, 128).T).astype(f32)
    d["be1c"] = np.ascontiguousarray(ln1_b.reshape(KT_D, 128).T).astype(f32)
    d["g2c"] = np.ascontiguousarray(ln2_g.reshape(KT_D, 128).T).astype(f32)
    d["be2c"] = np.ascontiguousarray(ln2_b.reshape(KT_D, 128).T).astype(f32)
    return d


def kernel(x, Wq, Wk, Wv, Wo, proj, W1, b1, W2, b2, ln1_g, ln1_b, ln2_g, ln2_b):
    from concourse import bass_utils

    x = np.asarray(x, np.float32)
    shared = _host_inputs(x, np.asarray(Wq), np.asarray(Wk), np.asarray(Wv),
                          np.asarray(Wo), np.asarray(proj), np.asarray(W1),
                          np.asarray(b1), np.asarray(W2), np.asarray(b2),
                          np.asarray(ln1_g), np.asarray(ln1_b),
                          np.asarray(ln2_g), np.asarray(ln2_b))

    if "nc" not in _CACHE:
        _CACHE["nc"] = _build()
    nc = _CACHE["nc"]

    in_maps = []
    for c in range(NC):
        xs = x[:, c * NT:(c + 1) * NT, :]
        xT = np.ascontiguousarray(xs.transpose(0, 2, 1))
        oh = np.zeros((1, NC), np.float32)
        oh[0, c] = 1.0
        m = dict(shared)
        m["x_f32"] = xT
        m["x_bf"] = xT.astype(ml_dtypes.bfloat16)
        m["onehot"] = oh
        in_maps.append(m)

    trace = bool(int(os.environ.get("KERNEL_TRACE", "0")))
    res = bass_utils.run_bass_kernel_spmd(nc, in_maps, core_ids=list(range(NC)),
                                          trace=trace)
    if trace and res.exec_time_ns is not None:
        print(f"HW exec time: {res.exec_time_ns} ns")
        if res.instructions_and_trace is not None:
            print("trace:", res.instructions_and_trace[1])

    outp = np.empty((B, N, D), np.float32)
    for c in range(NC):
        oT = res.results[c]["out"]
        outp[:, c * NT:(c + 1) * NT, :] = oT.transpose(0, 2, 1)
    return outp

